# revision 35
# baseline (speedup 1.0000x reference)
"""Trainium2 Bass kernel for a bidirectional linear-attention transformer layer.

Contract: kernel(**inputs) takes the FULL unsharded inputs (as produced by the
problem's setup_inputs()) and returns the FULL (B, T, D) float32 output.

Sharding: token-parallel across 8 NeuronCores. Core c handles batch c//4,
query tokens [ (c%4)*512, (c%4)*512+512 ), with a +-W token halo (W=128 for
decay=0.9). The decay mask d^|i-j| factorizes per 256-token query chunk into
  - a Q-side scale d^{+di} (past/diag) or d^{-di} (future), baked into two
    pre-scaled copies of Q,
  - a K-side scale folded into V rows and into an extra "ones" column of V
    that simultaneously produces the normalization denominator z,
  - an elementwise 256x256 mask for the diagonal zone only.

For the off-diagonal (fully factorized) zones the attention is linearized:
    out_chunk += (K_blk^T V_blk')^T q_chunk        (G-trick)
so only the diagonal zone materializes scores.  All heavy matmuls run in
bf16 (fp32 PSUM accumulate); the residual path and statistics stay fp32.

Structure (per core):
  A  LN1 + PE transposes            -> xsT (bf16, [d, tok])
  B  K^T (diag cols), Qp/Qf, V+decay variants, K_tok, G matrices
  C1 diagonal QK scores             -> st tiles (bf16, masked)
  C2 AV chains (diag st + G terms)  -> numT (normalized, bf16)
  D  O-proj + residual + LN2        -> x2 (f32), xs2T (bf16)
  E  FFN1 + gelu + FFN2             -> out
Head pairs (2c, 2c+1) occupy PE row groups 0-63 / 64-127, so their K=64
matmuls execute concurrently when emitted adjacently.
"""

import math
import os
from contextlib import ExitStack

import numpy as np
import ml_dtypes

import concourse.bacc as bacc
import concourse.mybir as mybir
import concourse.tile as tile
from concourse import bass_utils
from concourse.alu_op_type import AluOpType

F32 = mybir.dt.float32
BF16 = mybir.dt.bfloat16
AF = mybir.ActivationFunctionType

B, T, D, H, FF = 2, 2048, 512, 8, 2048
HD = D // H          # 64
NCORES = 8
SH = 4               # token shards per batch
TPC = T // SH        # 512 query tokens per core
CH = 256             # query chunk width
NQC = TPC // CH      # 2 query chunks per core
KC = D // 128        # 4 contraction chunks over D
FFC = FF // 128      # 16 ff chunks

_BUILD_CACHE = {}


def _build(W, uniform, nobias):
    nW = W // 128            # past/future 128-blocks per q-chunk
    L = TPC + 2 * W          # local tokens per core
    NT = L // 128            # 128-token tiles per core
    NQB = TPC // 128         # 4 query 128-blocks
    NHM = 1 if uniform else H    # distinct decay tables needed
    NQP = 1 if uniform else KC

    CS_COLS = 24 + 3 * NT * H + 2 * NT        # fp32 pack 1
    O_VN = 24
    O_MR = 24 + 3 * NT * H
    O_QP = 0                                  # fp32 pack 2 offsets
    O_QN = NQP * TPC
    CB_COLS = 2 * NQP * TPC
    O_DM = 2 * KC * 512                       # dmask lives in wa (bf16)
    WA_COLS = O_DM + NHM * 2 * CH

    nc = bacc.Bacc("TRN2", target_bir_lowering=False, debug=False)

    def dram_in(name, shape, dt=F32):
        return nc.dram_tensor(name, shape, dt, kind="ExternalInput").ap()

    # DMA issue order == declaration/use order below (HWDGE FIFO):
    xbig_a = dram_in("xh0", [128, (NT // 2) * D])     # x tokens, first half
    id_d = dram_in("ident", [128, 128], BF16)
    cs_d = dram_in("cs", [128, CS_COLS])              # small fp32 consts
    xbig_b = dram_in("xh1", [128, (NT - NT // 2) * D])
    wa_d = dram_in("wa", [128, WA_COLS], BF16)   # wk|wq|dmask
    cb_d = dram_in("cb", [128, CB_COLS])              # qpos|qneg|dmask
    wb_d = dram_in("wb", [128, 2 * KC * 512], BF16)   # wv|wo
    if not nobias:
        brow_d = dram_in("brow", [1, 5 * D], BF16)    # bv|bo|bf2|bk|bq rows
    w1_d = dram_in("w1", [128, KC * FF], BF16)
    w2_d = dram_in("w2", [128, FFC * D], BF16)

    out = nc.dram_tensor("out", [TPC, D], F32, kind="ExternalOutput").ap()

    dbg = os.environ.get("KDBG") == "1"
    dbg_d = {}
    if dbg:
        for nm, shape in [("d_xsT", [128, L]), ("d_kT", [128, TPC]),
                          ("d_qp", [128, TPC]), ("d_qf", [128, TPC]),
                          ("d_vaug", [128, H * (HD + 1)]),
                          ("d_ktok", [128, D]), ("d_numT", [128, TPC]),
                          ("d_x2", [128, D]), ("d_h1", [128, TPC])]:
            dbg_d[nm] = nc.dram_tensor(nm, shape, F32,
                                       kind="ExternalOutput").ap()

    with tile.TileContext(nc) as tc, ExitStack() as top:
        cpool = top.enter_context(tc.tile_pool(name="const", bufs=1))
        xpool = top.enter_context(tc.tile_pool(name="x", bufs=1))
        x2pool = top.enter_context(tc.tile_pool(name="x2", bufs=1))

        # ---- input DMAs (one per packed tensor) ------------------------
        xh0 = xpool.tile([128, (NT // 2) * D], F32, tag="xh0", name="xh0")
        nc.sync.dma_start(xh0[:], xbig_a[:])
        id_t = cpool.tile([128, 128], BF16, tag="ident", name="ident")
        nc.sync.dma_start(id_t[:], id_d[:])

        def x_t(i):
            if i < NT // 2:
                return xh0[:, i * D:(i + 1) * D]
            j = i - NT // 2
            return xh1[:, j * D:(j + 1) * D]

        cs_t = cpool.tile([128, CS_COLS], F32, tag="cs", name="cs")
        nc.sync.dma_start(cs_t[:], cs_d[:])
        xh1 = xpool.tile([128, (NT - NT // 2) * D], F32, tag="xh1", name="xh1")
        nc.sync.dma_start(xh1[:], xbig_b[:])
        wa_t = cpool.tile([128, WA_COLS], BF16, tag="wa", name="wa")
        nc.sync.dma_start(wa_t[:], wa_d[:])
        cb_t = cpool.tile([128, CB_COLS], F32, tag="cb", name="cb")
        nc.sync.dma_start(cb_t[:], cb_d[:])
        wb_t = cpool.tile([128, 2 * KC * 512], BF16, tag="wb", name="wb")
        nc.sync.dma_start(wb_t[:], wb_d[:])
        if not nobias:
            brow_t = cpool.tile([1, 5 * D], BF16, tag="brow", name="brow")
            nc.sync.dma_start(brow_t[:], brow_d[:])
        w1_t = cpool.tile([128, KC * FF], BF16, tag="w1", name="w1")
        nc.sync.dma_start(w1_t[:], w1_d[:])
        w2_t = cpool.tile([128, FFC * D], BF16, tag="w2", name="w2")
        nc.sync.dma_start(w2_t[:], w2_d[:])

        def wk_s(k, c):
            return wa_t[:, k * 512 + 128 * c:k * 512 + 128 * (c + 1)]

        def wq_s(k, c):
            return wa_t[:, 2048 + k * 512 + 128 * c:2048 + k * 512 + 128 * (c + 1)]

        def wk_full(k):
            return wa_t[:, k * 512:(k + 1) * 512]

        def wv_s(k):
            return wb_t[:, k * 512:(k + 1) * 512]

        def wo_s(k):
            return wb_t[:, 2048 + k * 512:2048 + k * 512 + 512]

        bf1_c = lambda f: cs_t[:, 8 + f:9 + f]
        vsc = {"n": lambda i, h: cs_t[:, O_VN + i * H + h:O_VN + i * H + h + 1],
               "f": lambda i, h: cs_t[:, O_VN + NT * H + i * H + h:
                                      O_VN + NT * H + i * H + h + 1],
               "p": lambda i, h: cs_t[:, O_VN + 2 * NT * H + i * H + h:
                                      O_VN + 2 * NT * H + i * H + h + 1]}
        mu_c = lambda i: cs_t[:, O_MR + 2 * i:O_MR + 2 * i + 1]
        rr_c = lambda i: cs_t[:, O_MR + 2 * i + 1:O_MR + 2 * i + 2]
        qpos_s = lambda c: cb_t[:, O_QP + (c % NQP) * TPC:
                                O_QP + (c % NQP) * TPC + TPC]
        qneg_s = lambda c: cb_t[:, O_QN + (c % NQP) * TPC:
                                O_QN + (c % NQP) * TPC + TPC]
        dm_s = lambda h, j: wa_t[:, O_DM + ((h % NHM) * 2 + j) * CH:
                                 O_DM + ((h % NHM) * 2 + j) * CH + CH]

        eps_t = cpool.tile([128, 1], F32, tag="eps", name="eps")
        nc.gpsimd.memset(eps_t[:], 1e-5)
        ones1b = cpool.tile([1, 512], BF16, tag="on1b", name="on1b")
        nc.gpsimd.memset(ones1b[:], 1.0)
        # warm the ACT LUT tables while input DMAs run
        warm_t = cpool.tile([1, 1], F32, tag="warm", name="warm")
        for fn in (AF.Copy, AF.Sqrt, AF.Exp, AF.Relu, AF.Gelu):
            nc.scalar.activation(warm_t[:], eps_t[0:1, :], fn)
        # PE warm-up: keep the PE busy (and the HAM un-throttled) during the
        # initial DMA wait so real matmuls run at 2.4 GHz from the start.
        wrm = cpool.tile([128, 512], BF16, tag="wrm", name="wrm")
        nc.gpsimd.memset(wrm[:], 0.0)

        def dump(nm, ap):
            if not dbg:
                return
            t = x2pool.tile(list(ap.shape), F32, tag="dbg_" + nm,
                            name="dbg_" + nm)
            nc.vector.tensor_copy(t[:], ap)
            nc.sync.dma_start(dbg_d[nm][:], t[:])

        x2_t = [x2pool.tile([128, D], F32, tag=f"x2_{q}", name=f"x2_{q}")
                for q in range(NQB)]
        xs2T = [x2pool.tile([128, TPC], BF16, tag=f"xs2T{c}", name=f"xs2T{c}")
                for c in range(KC)]

        with ExitStack() as attn:
            apool = attn.enter_context(tc.tile_pool(name="attn", bufs=1))
            spool = attn.enter_context(tc.tile_pool(name="st", bufs=1))
            tpool = attn.enter_context(tc.tile_pool(name="tmp", bufs=4))
            zpool = attn.enter_context(tc.tile_pool(name="ztmp", bufs=4))
            psS = attn.enter_context(
                tc.tile_pool(name="psS", bufs=2, space="PSUM"))
            psPJ = attn.enter_context(
                tc.tile_pool(name="psPJ", bufs=2, space="PSUM"))
            psAV = attn.enter_context(
                tc.tile_pool(name="psAV", bufs=2, space="PSUM"))
            psT = attn.enter_context(
                tc.tile_pool(name="psT", bufs=2, space="PSUM"))

            npj = [0]

            def pj_tile():
                pool = (psS, psPJ)[npj[0] % 2]
                npj[0] += 1
                return pool.tile([128, 512], F32,
                                 tag="s" if pool is psS else "pj",
                                 name="s" if pool is psS else "pj")

            for _ in range(8):
                wps = pj_tile()
                nc.tensor.matmul(wps[:, 0:512], wrm[:, 0:128], wrm[:, 0:512],
                                 start=True, stop=True)

            # ---- phase A: LN1 + transpose ------------------------------
            def layer_norm(src, xs_out):
                st6 = tpool.tile([128, 6], F32, tag="ln_st6", name="ln_st6")
                nc.vector.bn_stats(st6[:], src)
                mv = tpool.tile([128, 2], F32, tag="ln_mv", name="ln_mv")
                nc.vector.bn_aggr(mv[:], st6[:])
                sd = tpool.tile([128, 1], F32, tag="ln_sd", name="ln_sd")
                nc.scalar.activation(sd[:], mv[:, 1:2], AF.Sqrt, bias=eps_t[:])
                rr = tpool.tile([128, 1], F32, tag="ln_rr", name="ln_rr")
                nc.vector.reciprocal_approx_fast(rr[:], sd[:])
                nc.vector.tensor_scalar(xs_out, src, mv[:, 0:1], rr[:],
                                        AluOpType.subtract, AluOpType.mult)

            xsT = [apool.tile([128, L], BF16, tag=f"xsT{c}", name=f"xsT{c}")
                   for c in range(KC)]
            ntp = 0
            for i in range(NT):
                xs = tpool.tile([128, D], BF16, tag="ln_xs", name="ln_xs",
                                bufs=2)
                nc.vector.tensor_scalar(xs[:], x_t(i), mu_c(i), rr_c(i),
                                        AluOpType.subtract, AluOpType.mult)
                pt = psT.tile([128, 512], BF16, tag="tp", name="tp")
                for c in range(KC):
                    nc.tensor.transpose(pt[:, 128 * c:128 * (c + 1)],
                                        xs[:, 128 * c:128 * (c + 1)], id_t[:])
                    if ntp % 2 == 0:
                        nc.vector.tensor_copy(xsT[c][:, 128 * i:128 * (i + 1)],
                                              pt[:, 128 * c:128 * (c + 1)])
                    else:
                        nc.scalar.copy(xsT[c][:, 128 * i:128 * (i + 1)],
                                       pt[:, 128 * c:128 * (c + 1)])
                    ntp += 1

            dump("d_xsT", xsT[0][:])

            # ---- phase B: K^T(diag), Qp/Qf, V variants, K_tok, G -------
            nfg = [0]

            def feat_from_psum(ps, out_ap, span):
                """out = elu(ps) + 1 elementwise; out bf16 (no bias --
                biases are folded in as K=1 matmul rows upstream)."""
                m = tpool.tile([128, 512], BF16, tag="feat_m", name="feat_m",
                               bufs=2)
                nc.vector.tensor_scalar_min(m[:, :span], ps[:, :span], 0.0)
                r = tpool.tile([128, 512], BF16, tag="feat_r", name="feat_r",
                               bufs=2)
                if nfg[0] % 2 == 0:
                    nc.scalar.activation(r[:, :span], ps[:, :span], AF.Relu)
                else:
                    nc.vector.tensor_relu(r[:, :span], ps[:, :span])
                nfg[0] += 1
                e = tpool.tile([128, 512], BF16, tag="feat_e", name="feat_e",
                               bufs=2)
                nc.scalar.activation(e[:, :span], m[:, :span], AF.Exp)
                nc.vector.tensor_add(out_ap, e[:, :span], r[:, :span])

            # (kT is built later by transposing K_tok blocks)
            kT = [apool.tile([128, TPC], BF16, tag=f"kT{c}", name=f"kT{c}")
                  for c in range(KC)]

            qp = [apool.tile([128, TPC], BF16, tag=f"qp{c}", name=f"qp{c}")
                  for c in range(KC)]
            qf = [apool.tile([128, TPC], BF16, tag=f"qf{c}", name=f"qf{c}")
                  for c in range(KC)]
            for c in range(KC):
                ps = pj_tile()
                for k in range(KC):
                    nc.tensor.matmul(
                        ps[:], wq_s(k, c), xsT[k][:, W:W + TPC],
                        start=(k == 0), stop=(nobias and k == KC - 1))
                if not nobias:
                    nc.tensor.matmul(
                        ps[:], brow_t[0:1, 4 * D + 128 * c:4 * D + 128 * (c + 1)],
                        ones1b[0:1, 0:TPC], start=False, stop=True)
                ft = tpool.tile([128, TPC], F32, tag="feat_q", name="feat_q",
                                bufs=2)
                feat_from_psum(ps, ft[:], TPC)
                nc.vector.tensor_mul(qp[c][:], ft[:], qpos_s(c))
                nc.vector.tensor_mul(qf[c][:], ft[:], qneg_s(c))

            dump("d_qp", qp[0][:])
            dump("d_qf", qf[0][:])

            # role maps for key 128-blocks
            pastkb, diagkb, futkb = set(), set(), set()
            for qi in range(NQC):
                s = W + qi * CH
                for j in range(nW):
                    pastkb.add((s - W + 128 * j) // 128)
                    futkb.add((s + CH + 128 * j) // 128)
                diagkb.add(s // 128)
                diagkb.add(s // 128 + 1)
            plainkb = sorted(pastkb | futkb)

            # K in token-major layout for every local block; the diagonal
            # blocks are additionally transposed into kT (feature-major).
            allkb = sorted(pastkb | futkb | diagkb)
            ktok = {}
            for i in allkb:
                ps = pj_tile()
                for k in range(KC):
                    nc.tensor.matmul(ps[:],
                                     xsT[k][:, 128 * i:128 * (i + 1)],
                                     wk_full(k),
                                     start=(k == 0),
                                     stop=(nobias and k == KC - 1))
                if not nobias:
                    nc.tensor.matmul(ps[:], ones1b[0:1, 0:128],
                                     brow_t[0:1, 3 * D:4 * D],
                                     start=False, stop=True)
                kt = apool.tile([128, D], BF16, tag=f"ktok{i}",
                                name=f"ktok{i}")
                feat_from_psum(ps, kt[:], D)
                ktok[i] = kt
                if i in diagkb:
                    pt = psT.tile([128, 512], BF16, tag="tp", name="tp")
                    for c in range(KC):
                        nc.tensor.transpose(pt[:, 128 * c:128 * (c + 1)],
                                            kt[:, 128 * c:128 * (c + 1)],
                                            id_t[:])
                        dst = kT[c][:, 128 * i - W:128 * (i + 1) - W]
                        if ntp % 2 == 0:
                            nc.vector.tensor_copy(dst, pt[:, 128 * c:128 * (c + 1)])
                        else:
                            nc.scalar.copy(dst, pt[:, 128 * c:128 * (c + 1)])
                        ntp += 1

            dump("d_kT", kT[0][:])


            # V psum -> decay-scaled variants (vaug) per (role, block)
            vaug = {}
            for i in range(NT):
                roles = []
                if i in pastkb:
                    roles.append("n")
                if i in diagkb:
                    roles.append("p")
                if i in futkb:
                    roles.append("f")
                if not roles:
                    continue
                ps = pj_tile()
                for k in range(KC):
                    nc.tensor.matmul(ps[:],
                                     xsT[k][:, 128 * i:128 * (i + 1)],
                                     wv_s(k),
                                     start=(k == 0),
                                     stop=(nobias and k == KC - 1))
                if not nobias:
                    nc.tensor.matmul(ps[:], ones1b[0:1, 0:128],
                                     brow_t[0:1, 0:D], start=False, stop=True)
                for nv, rname in enumerate(roles):
                    va = apool.tile([128, H * (HD + 1)], BF16,
                                    tag=f"vaug_{rname}{i}",
                                    name=f"vaug_{rname}{i}")
                    vaug[(rname, i)] = va
                    if uniform:
                        sc = vsc[rname](i, 0)
                        if (i + nv) % 2 == 0:
                            nc.scalar.activation(
                                va[:].rearrange("p (h d) -> p h d", h=H)[:, :, 0:64],
                                ps[:].rearrange("p (h d) -> p h d", h=H),
                                AF.Copy, scale=sc)
                        else:
                            nc.vector.tensor_scalar_mul(
                                va[:].rearrange("p (h d) -> p h d", h=H)[:, :, 0:64],
                                ps[:].rearrange("p (h d) -> p h d", h=H), sc)
                        nc.scalar.copy(
                            va[:].rearrange("p (h d) -> p h d", h=H)[:, :, 64:65],
                            sc.broadcast_to((128, H, 1)))
                    else:
                        for h in range(H):
                            sc = vsc[rname](i, h)
                            nc.scalar.activation(va[:, h * 65:h * 65 + 64],
                                                 ps[:, h * 64:(h + 1) * 64],
                                                 AF.Copy, scale=sc)
                            nc.scalar.copy(va[:, h * 65 + 64:h * 65 + 65], sc)

            if dbg:
                dump("d_vaug", vaug[("p", W // 128)][:])

            if dbg:
                dump("d_ktok", ktok[plainkb[0]][:])

            # G matrices: G[(role, kb, c)] [128, 65] bf16 -- head 2c in
            # partitions 0:64, head 2c+1 in 64:128.
            # G[f, m] = sum_key K_tok[key, f] * vaug[key, m]
            groles = []
            for i in sorted(pastkb):
                groles.append(("n", i))
            for i in sorted(futkb):
                groles.append(("f", i))
            gmat = {}
            ngc = 0
            for role, kb in groles:
                for c in range(KC):
                    gt = spool.tile([128, 65], BF16, tag=f"g{role}{kb}_{c}",
                                    name=f"g{role}{kb}_{c}")
                    gmat[(role, kb, c)] = gt
                    pg = pj_tile()
                    for hh in range(2):
                        h = 2 * c + hh
                        nc.tensor.matmul(
                            pg[64 * hh:64 * hh + 64, 0:65],
                            ktok[kb][:, 64 * h:64 * h + 64],
                            vaug[(role, kb)][:, h * 65:(h + 1) * 65],
                            start=True, stop=True,
                            tile_position=(0, 64 * hh))
                    if ngc % 2 == 0:
                        nc.vector.tensor_copy(gt[:], pg[0:128, 0:65])
                    else:
                        nc.scalar.copy(gt[:], pg[0:128, 0:65])
                    ngc += 1

            # ---- phase C1: diagonal QK scores (masked) -----------------
            # per qi the two diagonal blocks; head pairs emitted adjacently
            # so their K=64 matmuls run in different PE row groups.
            diag = []                     # (qi, kb, mi)
            for qi in range(NQC):
                s = W + qi * CH
                diag.append((qi, s // 128, 0))
                diag.append((qi, s // 128 + 1, 1))

            sts = {}
            nst = 0
            nc1 = [0]

            def c1_tile():
                k = nc1[0] % 3
                nc1[0] += 1
                if k == 2:
                    return psAV.tile([128, CH], F32, tag="av", name="av")
                return pj_tile()

            for c in range(KC):
                for qi, kb, mi in diag:
                    pss = [c1_tile(), c1_tile()]
                    for hh in range(2):
                        po = 64 * hh
                        nc.tensor.matmul(
                            pss[hh][:, 0:CH],
                            kT[c][po:po + 64,
                                  128 * kb - W:128 * (kb + 1) - W],
                            qp[c][po:po + 64, qi * CH:(qi + 1) * CH],
                            start=True, stop=True)
                    for hh in range(2):
                        h = 2 * c + hh
                        sr = tpool.tile([128, CH], BF16, tag="st_raw",
                                        name="st_raw", bufs=3)
                        if nst % 2 == 0:
                            nc.scalar.copy(sr[:], pss[hh][:, 0:CH])
                        else:
                            nc.vector.tensor_copy(sr[:], pss[hh][:, 0:CH])
                        st = spool.tile([128, CH], BF16,
                                        tag=f"st{h}_{kb}",
                                        name=f"st{h}_{kb}")
                        if nst % 2 == 0:
                            nc.vector.tensor_mul(st[:], sr[:], dm_s(h, mi))
                        else:
                            nc.gpsimd.tensor_mul(st[:], sr[:], dm_s(h, mi))
                        nst += 1
                        sts[(h, kb)] = st

            # ---- phase C2: AV chains ----------------------------------
            # chain(h, qi): 2 diag terms (vaug^T st) + 2*nW plain terms
            # (G^T q).  z is the 65th row; recip is batched per head pair
            # and the 1/z broadcast (gpsimd) overlaps the next pair's
            # chains.
            numT = [apool.tile([128, TPC], BF16, tag=f"numT{c}",
                               name=f"numT{c}") for c in range(KC)]
            zall = zpool.tile([1, 16 * CH], F32, tag="zall", name="zall",
                              bufs=1)
            nav = 0

            def av_chain_pair(c, qi):
                nonlocal nav
                s = W + qi * CH
                qsl = slice(qi * CH, (qi + 1) * CH)
                pavs = []
                for hh in range(2):
                    sel = nav % 4
                    nav += 1
                    if sel < 2:
                        pav = psAV.tile([128, CH], F32, tag="av", name="av")
                    else:
                        pav = pj_tile()
                    pavs.append(pav)
                kbs_d = [s // 128, s // 128 + 1]
                kbs_n = [(s - W) // 128 + j for j in range(nW)]
                kbs_f = [(s + CH) // 128 + j for j in range(nW)]
                nterm = 2 + 2 * nW
                for hh in range(2):
                    h = 2 * c + hh
                    po = 64 * hh
                    it = 0
                    for kb in kbs_d:
                        nc.tensor.matmul(
                            pavs[hh][0:65, 0:CH],
                            vaug[("p", kb)][:, h * 65:(h + 1) * 65],
                            sts[(h, kb)][:],
                            start=(it == 0), stop=(it == nterm - 1))
                        it += 1
                    for kb in kbs_n:
                        nc.tensor.matmul(
                            pavs[hh][0:65, 0:CH],
                            gmat[("n", kb, c)][po:po + 64, :],
                            qp[c][po:po + 64, qsl],
                            start=(it == 0), stop=(it == nterm - 1))
                        it += 1
                    for kb in kbs_f:
                        nc.tensor.matmul(
                            pavs[hh][0:65, 0:CH],
                            gmat[("f", kb, c)][po:po + 64, :],
                            qf[c][po:po + 64, qsl],
                            start=(it == 0), stop=(it == nterm - 1))
                        it += 1
                for hh in range(2):
                    zi = 4 * c + 2 * hh + qi
                    nc.scalar.copy(zall[0:1, zi * CH:(zi + 1) * CH],
                                   pavs[hh][64:65, 0:CH])
                return pavs

            for c in range(KC):
                pav_q = [av_chain_pair(c, qi) for qi in range(NQC)]
                zlo = 4 * c * CH
                nc.vector.tensor_scalar_max(
                    zall[0:1, zlo:zlo + 4 * CH], zall[0:1, zlo:zlo + 4 * CH],
                    1e-6)
                nc.vector.reciprocal_approx_fast(
                    zall[0:1, zlo:zlo + 4 * CH], zall[0:1, zlo:zlo + 4 * CH])
                for hh in range(2):
                    po = 64 * hh
                    zo = (4 * c + 2 * hh) * CH
                    zb = zpool.tile([64, 2 * CH], F32, tag="zbs", name="zbs")
                    nc.gpsimd.partition_broadcast(
                        zb[:], zall[0:1, zo:zo + 2 * CH])
                    for qi in range(NQC):
                        qsl = slice(qi * CH, (qi + 1) * CH)
                        nc.vector.tensor_mul(
                            numT[c][po:po + 64, qsl],
                            pav_q[qi][hh][0:64, 0:CH],
                            zb[:, qi * CH:(qi + 1) * CH])

            dump("d_numT", numT[0][:])

            # ---- phase D: O-proj + residual + LN2 ----------------------
            for q in range(NQB):
                ql = slice(128 * q, 128 * (q + 1))
                ps = pj_tile()
                for k in range(KC):
                    nc.tensor.matmul(ps[:], numT[k][:, ql], wo_s(k),
                                     start=(k == 0),
                                     stop=(nobias and k == KC - 1))
                if not nobias:
                    nc.tensor.matmul(ps[:], ones1b[0:1, 0:128],
                                     brow_t[0:1, D:2 * D],
                                     start=False, stop=True)
                xres = x_t((W + 128 * q) // 128)
                nc.vector.tensor_add(x2_t[q][:], ps[:], xres)

                xs2 = tpool.tile([128, D], BF16, tag="ln_xs2", name="ln_xs2",
                                 bufs=2)
                layer_norm(x2_t[q], xs2[:])
                pt = psT.tile([128, 512], BF16, tag="tp", name="tp")
                for c in range(KC):
                    nc.tensor.transpose(pt[:, 128 * c:128 * (c + 1)],
                                        xs2[:, 128 * c:128 * (c + 1)], id_t[:])
                    if ntp % 2 == 0:
                        nc.vector.tensor_copy(xs2T[c][:, ql],
                                              pt[:, 128 * c:128 * (c + 1)])
                    else:
                        nc.scalar.copy(xs2T[c][:, ql],
                                       pt[:, 128 * c:128 * (c + 1)])
                    ntp += 1

        dump("d_x2", x2_t[0][:])

        # ---- phase E: FFN ----------------------------------------------
        with ExitStack() as ffn:
            fpool = ffn.enter_context(tc.tile_pool(name="ffn", bufs=1))
            psF = ffn.enter_context(
                tc.tile_pool(name="psF", bufs=4, space="PSUM"))
            psF2 = ffn.enter_context(
                tc.tile_pool(name="psF2", bufs=3, space="PSUM"))

            h1gT = [fpool.tile([128, TPC], BF16, tag=f"h1gT{f}",
                               name=f"h1gT{f}") for f in range(FFC)]

            def ffn2(q):
                ql = slice(128 * q, 128 * (q + 1))
                ps2 = psF2.tile([128, D], F32, tag="f2", name="f2")
                for f in range(FFC):
                    nc.tensor.matmul(ps2[:],
                                     h1gT[f][:, ql],
                                     w2_t[:, f * D:(f + 1) * D],
                                     start=(f == 0),
                                     stop=(nobias and f == FFC - 1))
                if not nobias:
                    nc.tensor.matmul(ps2[:], ones1b[0:1, 0:128],
                                     brow_t[0:1, 2 * D:3 * D],
                                     start=False, stop=True)
                o = fpool.tile([128, D], F32, tag="fout", name="fout", bufs=2)
                nc.vector.tensor_add(o[:], ps2[:], x2_t[q][:])
                nc.sync.dma_start(out[ql, :], o[:])

            for qh in range(2):
                hsl = slice(CH * qh, CH * (qh + 1))
                for f in range(FFC):
                    ps = psF.tile([128, CH], F32, tag="f1", name="f1")
                    for k in range(KC):
                        nc.tensor.matmul(
                            ps[:],
                            w1_t[:, k * FF + 128 * f:k * FF + 128 * (f + 1)],
                            xs2T[k][:, hsl],
                            start=(k == 0), stop=(k == KC - 1))
                    nc.scalar.activation(h1gT[f][:, hsl], ps[:],
                                         AF.Gelu, bias=bf1_c(f))
                ffn2(2 * qh)
                ffn2(2 * qh + 1)
            dump("d_h1", h1gT[0][:])

    nc.compile()
    return nc


def _get_nc(W, uniform, nobias):
    key = (W, uniform, nobias)
    if key not in _BUILD_CACHE:
        _BUILD_CACHE[key] = _build(W, uniform, nobias)
    return _BUILD_CACHE[key]


# ---------------------------------------------------------------------------
# host side
# ---------------------------------------------------------------------------

def _bf16(a):
    return np.asarray(a, np.float32).astype(ml_dtypes.bfloat16)


def kernel(x, mask, wq, wk, wv, wo, bo, g1, b1, g2, b2, w1, bf1, w2, bf2,
           decay_logit):
    x = np.asarray(x, np.float32)
    mask = np.asarray(mask)
    g1 = np.asarray(g1, np.float64)
    b1 = np.asarray(b1, np.float64)
    g2 = np.asarray(g2, np.float64)
    b2 = np.asarray(b2, np.float64)

    d64 = 1.0 / (1.0 + np.exp(-np.asarray(decay_logit, np.float64)))
    d64 = np.clip(d64, 1e-8, None)
    dmax = float(d64.max())
    # Band width: the attention kernel truncates at |i-j| > ~W.  The omitted
    # mass relative to the normalizer z is ~ d^(W+1)/(1+d); keep it under
    # 1e-5, well below the bf16 matmul noise (~1e-3).
    if dmax >= 1.0 - 1e-12:
        W = 512  # decay ~1: widest supported band
    else:
        need = math.log(1e-5 * (1.0 + dmax)) / math.log(dmax)
        W = max(128, 128 * math.ceil(need / 128))
        W = min(W, 512)
    nW = W // 128
    L = TPC + 2 * W
    NT = L // 128
    uniform = bool(np.all(d64 == d64[0]))
    nobias = bool(np.all(b1 == 0) and np.all(np.asarray(bo) == 0)
                  and np.all(np.asarray(bf2) == 0))
    NHM = 1 if uniform else H
    NQP = 1 if uniform else KC

    nc = _get_nc(W, uniform, nobias)

    # folded weights (bf16)
    wq_f = _bf16(g1[:, None] * np.asarray(wq, np.float64))
    wk_f = _bf16(g1[:, None] * np.asarray(wk, np.float64))
    wv_f = _bf16(g1[:, None] * np.asarray(wv, np.float64))
    wo_f = _bf16(wo)
    w1_f = _bf16(g2[:, None] * np.asarray(w1, np.float64))
    w2_f = _bf16(w2)
    bq_h = _bf16(b1 @ np.asarray(wq, np.float64))
    bk_h = _bf16(b1 @ np.asarray(wk, np.float64))
    bv_h = _bf16(b1 @ np.asarray(wv, np.float64))
    bo_h = _bf16(bo)
    bf1_h = (b2 @ np.asarray(w1, np.float64) +
             np.asarray(bf1, np.float64)).astype(np.float32)
    bf2_h = _bf16(bf2)

    def pack_kblocks(w, blk, nb):
        # [nb*128, cols] -> [128, nb*cols] with block k at cols [k*cols, ...)
        return np.ascontiguousarray(
            w.reshape(nb, 128, blk).transpose(1, 0, 2).reshape(128, nb * blk))

    # (dmask appended to wa below, bf16)
    wb_h = np.concatenate([
        pack_kblocks(wv_f, 512, 4),
        pack_kblocks(wo_f, 512, 4)], axis=1)
    w1_h = pack_kblocks(w1_f, FF, 4)
    w2_h = pack_kblocks(w2_f, D, 16)
    brow_h = np.concatenate(
        [bv_h, bo_h, bf2_h, bk_h, bq_h]).reshape(1, 5 * D)
    brow_h = brow_h.astype(ml_dtypes.bfloat16)

    # decay-derived tables
    ld = np.log(d64)                                    # (H,)
    di = np.arange(TPC, dtype=np.float64) % CH
    qposb = np.zeros((NQP, 128, TPC), np.float32)
    qnegb = np.zeros((NQP, 128, TPC), np.float32)
    for c in range(NQP):
        for hh in range(2):
            h = 2 * c + hh
            qposb[c, 64 * hh:64 * hh + 64, :] = np.exp(ld[h] * di)[None, :]
            qnegb[c, 64 * hh:64 * hh + 64, :] = np.exp(-ld[h] * di)[None, :]
    if uniform:
        qposb[:, 64:, :] = qposb[:, :64, :]
        qnegb[:, 64:, :] = qnegb[:, :64, :]
    # diagonal-zone mask: m[dk, di] = d^{|di-dk| - di}, dk in [0,256)
    dic = np.arange(CH, dtype=np.float64)
    dk = np.arange(CH, dtype=np.float64)
    dmask = np.zeros((NHM, 2, 128, CH), np.float32)
    for h in range(NHM):
        m = np.exp(ld[h] * (np.abs(dic[None, :] - dk[:, None]) - dic[None, :]))
        dmask[h, 0] = m[:128, :]
        dmask[h, 1] = m[128:, :]
    cb_h = np.concatenate([
        qposb.transpose(1, 0, 2).reshape(128, NQP * TPC),
        qnegb.transpose(1, 0, 2).reshape(128, NQP * TPC)], axis=1)
    cb_h = np.ascontiguousarray(cb_h, np.float32)
    wa_h = np.concatenate([
        pack_kblocks(wk_f, 512, 4),
        pack_kblocks(wq_f, 512, 4),
        dmask.transpose(2, 0, 1, 3).reshape(128, NHM * 2 * CH)
             .astype(ml_dtypes.bfloat16)], axis=1)

    in_maps = []
    for core in range(NCORES):
        b = core // SH
        t0 = (core % SH) * TPC
        lo, hi = t0 - W, t0 + TPC + W
        xs = np.zeros((L, D), np.float32)
        pad = np.zeros((L,), np.float64)
        glo, ghi = max(lo, 0), min(hi, T)
        xs[glo - lo:ghi - lo] = x[b, glo:ghi]
        pad[glo - lo:ghi - lo] = (~mask[b, glo:ghi]).astype(np.float64)
        xbig = np.ascontiguousarray(
            xs.reshape(NT, 128, D).transpose(1, 0, 2).reshape(128, NT * D))

        vneg = np.zeros((H, L), np.float32)
        vpos = np.zeros((H, L), np.float32)
        vpln = np.zeros((H, L), np.float32)
        for h in range(H):
            vn = np.zeros(L)
            vp = np.zeros(L)
            for qi in range(NQC):
                s = W + qi * CH
                jj = np.arange(s - W, s)
                vn[jj] = np.exp(ld[h] * (s - jj))
                jj = np.arange(s + CH, s + CH + W)
                vp[jj] = np.exp(ld[h] * (CH + jj - (s + CH)))
            vneg[h] = (vn * pad).astype(np.float32)
            vpos[h] = (vp * pad).astype(np.float32)
            vpln[h] = pad.astype(np.float32)

        def _vlayout(a):
            return np.ascontiguousarray(
                a.reshape(H, L // 128, 128).transpose(2, 1, 0)
                 .reshape(128, (L // 128) * H))
        vneg, vpos, vpln = _vlayout(vneg), _vlayout(vpos), _vlayout(vpln)

        mu = xs.mean(axis=1)
        rr = 1.0 / np.sqrt(xs.var(axis=1) + 1e-5)
        mr = np.stack([mu, rr], axis=1).astype(np.float32)    # [L, 2]
        mr = mr.reshape(NT, 128, 2).transpose(1, 0, 2).reshape(128, 2 * NT)
        cs_h = np.concatenate([
            np.zeros((128, 8), np.float32),      # (bias cols retired)
            bf1_h.reshape(16, 128).T,
            vneg, vpos, vpln, mr], axis=1)
        cs_h = np.ascontiguousarray(cs_h, np.float32)

        nh = NT // 2
        im = {
            "xh0": np.ascontiguousarray(xbig[:, :nh * D]),
            "xh1": np.ascontiguousarray(xbig[:, nh * D:]),
            "ident": np.eye(128, dtype=ml_dtypes.bfloat16),
            "cs": cs_h, "cb": cb_h,
            "wa": wa_h, "wb": wb_h, "w1": w1_h, "w2": w2_h,
        }
        if not nobias:
            im["brow"] = brow_h
        in_maps.append(im)

    res = bass_utils.run_bass_kernel_spmd(nc, in_maps,
                                          core_ids=list(range(NCORES)))
    out = np.empty((B, T, D), np.float32)
    for core in range(NCORES):
        b = core // SH
        t0 = (core % SH) * TPC
        out[b, t0:t0 + TPC] = res.results[core]["out"]

    # Degenerate-mask patch: a query whose entire +-W neighbourhood is
    # masked has z ~ 0 on the device; the reference output is then
    # dominated by out-of-band keys that the banded kernel truncates.
    # Recompute those rows exactly on the host (never triggers for an
    # all-False mask).
    if mask.any():
        idx = np.arange(T)
        for b_ in range(B):
            keep = ~np.asarray(mask[b_])
            if keep.all():
                continue
            kpos = idx[keep]
            if len(kpos) == 0:
                dist = np.full(T, T)
            else:
                ins = np.searchsorted(kpos, idx)
                left = np.where(ins > 0, idx - kpos[np.clip(ins - 1, 0,
                                                            len(kpos) - 1)], T)
                right = np.where(ins < len(kpos),
                                 kpos[np.clip(ins, 0, len(kpos) - 1)] - idx, T)
                dist = np.minimum(left, right)
            need = dist > (W - 32)
            if need.any():
                out[b_, need] = _exact_rows(
                    np.asarray(x[b_], np.float64), keep.astype(np.float64),
                    np.where(need)[0], wq, wk, wv, wo, bo, g1, b1, g2, b2,
                    w1, bf1, w2, bf2, d64)
    return out


def _exact_rows(xb, pad, rows, wq, wk, wv, wo, bo, g1, b1, g2, b2,
                w1, bf1, w2, bf2, d64):
    """Reference math (float64) for the given query rows of one batch."""
    import scipy.special as _sp

    def ln(z, g, b):
        mu = z.mean(-1, keepdims=True)
        var = ((z - mu) ** 2).mean(-1, keepdims=True)
        return (z - mu) / np.sqrt(var + 1e-5) * g + b

    wq = np.asarray(wq, np.float64); wk = np.asarray(wk, np.float64)
    wv = np.asarray(wv, np.float64); wo = np.asarray(wo, np.float64)
    w1 = np.asarray(w1, np.float64); w2 = np.asarray(w2, np.float64)
    xn = ln(xb, np.asarray(g1, np.float64), np.asarray(b1, np.float64))

    def feat(z):
        return np.where(z > 0, z + 1.0, np.exp(np.minimum(z, 0.0)))

    Q = feat((xn[rows] @ wq).reshape(len(rows), H, HD))       # (R,H,hd)
    K = feat((xn @ wk).reshape(T, H, HD)) * pad[:, None, None]
    V = (xn @ wv).reshape(T, H, HD) * pad[:, None, None]
    dist = np.abs(rows[:, None] - np.arange(T)[None, :]).astype(np.float64)
    M = np.exp(dist[None] * np.log(d64)[:, None, None])       # (H,R,T)
    A = np.einsum('rhd,jhd->hrj', Q, K) * M
    z = np.clip(A.sum(-1, keepdims=True), 1e-6, None)
    o = np.einsum('hrj,jhd->rhd', A / z, V).reshape(len(rows), D)
    o = o @ wo + np.asarray(bo, np.float64)
    x2 = xb[rows] + o
    xn2 = ln(x2, np.asarray(g2, np.float64), np.asarray(b2, np.float64))
    h1 = xn2 @ w1 + np.asarray(bf1, np.float64)
    gl = 0.5 * h1 * (1.0 + _sp.erf(h1 / np.sqrt(2.0)))
    ffn = gl @ w2 + np.asarray(bf2, np.float64)
    return (x2 + ffn).astype(np.float32)


# revision 42
# speedup vs baseline: 1.0568x; 1.0568x over previous
"""Trainium2 Bass kernel for a bidirectional linear-attention transformer layer.

Contract: kernel(**inputs) takes the FULL unsharded inputs (as produced by the
problem's setup_inputs()) and returns the FULL (B, T, D) float32 output.

Sharding: token-parallel across 8 NeuronCores. Core c handles batch c//4,
query tokens [ (c%4)*512, (c%4)*512+512 ), with a +-W token halo (W=128 for
decay=0.9). The decay mask d^|i-j| factorizes per 256-token query chunk into
  - a Q-side scale d^{+di} (past/diag) or d^{-di} (future), baked into two
    pre-scaled copies of Q,
  - a K-side scale folded into V rows and into an extra "ones" column of V
    that simultaneously produces the normalization denominator z,
  - an elementwise 256x256 mask for the diagonal zone only.

For the off-diagonal (fully factorized) zones the attention is linearized:
    out_chunk += (K_blk^T V_blk')^T q_chunk        (G-trick)
so only the diagonal zone materializes scores.  All heavy matmuls run in
bf16 (fp32 PSUM accumulate); the residual path and statistics stay fp32.

Structure (per core):
  A  LN1 + PE transposes            -> xsT (bf16, [d, tok])
  B  K^T (diag cols), Qp/Qf, V+decay variants, K_tok, G matrices
  C1 diagonal QK scores             -> st tiles (bf16, masked)
  C2 AV chains (diag st + G terms)  -> numT (normalized, bf16)
  D  O-proj + residual + LN2        -> x2 (f32), xs2T (bf16)
  E  FFN1 + gelu + FFN2             -> out
Head pairs (2c, 2c+1) occupy PE row groups 0-63 / 64-127, so their K=64
matmuls execute concurrently when emitted adjacently.
"""

import math
import os
from contextlib import ExitStack

import numpy as np
import ml_dtypes

import concourse.bacc as bacc
import concourse.mybir as mybir
import concourse.tile as tile
from concourse import bass_utils
from concourse.alu_op_type import AluOpType

F32 = mybir.dt.float32
BF16 = mybir.dt.bfloat16
AF = mybir.ActivationFunctionType

B, T, D, H, FF = 2, 2048, 512, 8, 2048
HD = D // H          # 64
NCORES = 8
SH = 4               # token shards per batch
TPC = T // SH        # 512 query tokens per core
CH = 256             # query chunk width
NQC = TPC // CH      # 2 query chunks per core
KC = D // 128        # 4 contraction chunks over D
FFC = FF // 128      # 16 ff chunks

_BUILD_CACHE = {}


def _build(W, uniform, nobias):
    nW = W // 128            # past/future 128-blocks per q-chunk
    L = TPC + 2 * W          # local tokens per core
    NT = L // 128            # 128-token tiles per core
    NQB = TPC // 128         # 4 query 128-blocks
    NHM = 1 if uniform else H    # distinct decay tables needed
    NQP = 1 if uniform else KC

    CS_COLS = 24 + 3 * NT * H + 2 * NT        # fp32 pack 1
    O_VN = 24
    O_MR = 24 + 3 * NT * H
    O_QP = 0                                  # fp32 pack 2 offsets
    O_QN = NQP * TPC
    CB_COLS = 2 * NQP * TPC
    NPR = 1 if uniform else KC                # distinct head-pair masks
    O_DM = 2 * KC * 512                       # dmask lives in wa (bf16)
    WA_COLS = O_DM + NPR * 2 * 2 * CH

    nc = bacc.Bacc("TRN2", target_bir_lowering=False, debug=False)

    def dram_in(name, shape, dt=F32):
        return nc.dram_tensor(name, shape, dt, kind="ExternalInput").ap()

    # DMA issue order == declaration/use order below (HWDGE FIFO):
    xbig_a = dram_in("xh0", [128, (NT // 2) * D])     # x tokens, first half
    id_d = dram_in("ident", [128, 128], BF16)
    cs_d = dram_in("cs", [128, CS_COLS])              # small fp32 consts
    xbig_b = dram_in("xh1", [128, (NT - NT // 2) * D])
    wa_d = dram_in("wa", [128, WA_COLS], BF16)   # wk|wq|dmask
    cb_d = dram_in("cb", [128, CB_COLS])              # qpos|qneg|dmask
    wb_d = dram_in("wb", [128, 2 * KC * 512], BF16)   # wv|wo
    if not nobias:
        brow_d = dram_in("brow", [1, 5 * D], BF16)    # bv|bo|bf2|bk|bq rows
    w1_d = dram_in("w1", [128, KC * FF], BF16)
    w2_d = dram_in("w2", [128, FFC * D], BF16)

    out = nc.dram_tensor("out", [TPC, D], F32, kind="ExternalOutput").ap()

    dbg = os.environ.get("KDBG") == "1"
    dbg_d = {}
    if dbg:
        for nm, shape in [("d_xsT", [128, L]), ("d_kT", [128, TPC]),
                          ("d_qp", [128, TPC]), ("d_qf", [128, TPC]),
                          ("d_vaug", [128, H * (HD + 1)]),
                          ("d_ktok", [128, D]), ("d_numT", [128, TPC]),
                          ("d_x2", [128, D]), ("d_h1", [128, TPC])]:
            dbg_d[nm] = nc.dram_tensor(nm, shape, F32,
                                       kind="ExternalOutput").ap()

    with tile.TileContext(nc) as tc, ExitStack() as top:
        cpool = top.enter_context(tc.tile_pool(name="const", bufs=1))
        xpool = top.enter_context(tc.tile_pool(name="x", bufs=1))
        x2pool = top.enter_context(tc.tile_pool(name="x2", bufs=1))

        # ---- input DMAs (one per packed tensor) ------------------------
        xh0 = xpool.tile([128, (NT // 2) * D], F32, tag="xh0", name="xh0")
        nc.sync.dma_start(xh0[:], xbig_a[:])
        id_t = cpool.tile([128, 128], BF16, tag="ident", name="ident")
        nc.sync.dma_start(id_t[:], id_d[:])

        def x_t(i):
            if i < NT // 2:
                return xh0[:, i * D:(i + 1) * D]
            j = i - NT // 2
            return xh1[:, j * D:(j + 1) * D]

        cs_t = cpool.tile([128, CS_COLS], F32, tag="cs", name="cs")
        nc.sync.dma_start(cs_t[:], cs_d[:])
        xh1 = xpool.tile([128, (NT - NT // 2) * D], F32, tag="xh1", name="xh1")
        nc.sync.dma_start(xh1[:], xbig_b[:])
        wa_t = cpool.tile([128, WA_COLS], BF16, tag="wa", name="wa")
        nc.sync.dma_start(wa_t[:], wa_d[:])
        cb_t = cpool.tile([128, CB_COLS], F32, tag="cb", name="cb")
        nc.sync.dma_start(cb_t[:], cb_d[:])
        wb_t = cpool.tile([128, 2 * KC * 512], BF16, tag="wb", name="wb")
        nc.sync.dma_start(wb_t[:], wb_d[:])
        if not nobias:
            brow_t = cpool.tile([1, 5 * D], BF16, tag="brow", name="brow")
            nc.sync.dma_start(brow_t[:], brow_d[:])
        w1_t = cpool.tile([128, KC * FF], BF16, tag="w1", name="w1")
        nc.sync.dma_start(w1_t[:], w1_d[:])
        w2_t = cpool.tile([128, FFC * D], BF16, tag="w2", name="w2")
        nc.sync.dma_start(w2_t[:], w2_d[:])

        def wk_s(k, c):
            return wa_t[:, k * 512 + 128 * c:k * 512 + 128 * (c + 1)]

        def wq_s(k, c):
            return wa_t[:, 2048 + k * 512 + 128 * c:2048 + k * 512 + 128 * (c + 1)]

        def wk_full(k):
            return wa_t[:, k * 512:(k + 1) * 512]

        def wv_s(k):
            return wb_t[:, k * 512:(k + 1) * 512]

        def wo_s(k):
            return wb_t[:, 2048 + k * 512:2048 + k * 512 + 512]

        bf1_c = lambda f: cs_t[:, 8 + f:9 + f]
        vsc = {"n": lambda i, h: cs_t[:, O_VN + i * H + h:O_VN + i * H + h + 1],
               "f": lambda i, h: cs_t[:, O_VN + NT * H + i * H + h:
                                      O_VN + NT * H + i * H + h + 1],
               "p": lambda i, h: cs_t[:, O_VN + 2 * NT * H + i * H + h:
                                      O_VN + 2 * NT * H + i * H + h + 1]}
        mu_c = lambda i: cs_t[:, O_MR + 2 * i:O_MR + 2 * i + 1]
        rr_c = lambda i: cs_t[:, O_MR + 2 * i + 1:O_MR + 2 * i + 2]
        qpos_s = lambda c: cb_t[:, O_QP + (c % NQP) * TPC:
                                O_QP + (c % NQP) * TPC + TPC]
        qneg_s = lambda c: cb_t[:, O_QN + (c % NQP) * TPC:
                                O_QN + (c % NQP) * TPC + TPC]
        # paired mask: [dm(2c) | dm(2c+1)] per mi, 512 wide
        dm2_s = lambda c, j: wa_t[:, O_DM + ((c % NPR) * 2 + j) * 2 * CH:
                                  O_DM + ((c % NPR) * 2 + j) * 2 * CH + 2 * CH]

        eps_t = cpool.tile([128, 1], F32, tag="eps", name="eps")
        nc.gpsimd.memset(eps_t[:], 1e-5)
        ones1b = cpool.tile([1, 512], BF16, tag="on1b", name="on1b")
        nc.gpsimd.memset(ones1b[:], 1.0)
        # warm the ACT LUT tables while input DMAs run
        warm_t = cpool.tile([1, 1], F32, tag="warm", name="warm")
        for fn in (AF.Copy, AF.Sqrt, AF.Exp, AF.Relu, AF.Gelu):
            nc.scalar.activation(warm_t[:], eps_t[0:1, :], fn)
        # PE warm-up: keep the PE busy (and the HAM un-throttled) during the
        # initial DMA wait so real matmuls run at 2.4 GHz from the start.
        wrm = cpool.tile([128, 512], BF16, tag="wrm", name="wrm")
        nc.gpsimd.memset(wrm[:], 0.0)

        def dump(nm, ap):
            if not dbg:
                return
            t = x2pool.tile(list(ap.shape), F32, tag="dbg_" + nm,
                            name="dbg_" + nm)
            nc.vector.tensor_copy(t[:], ap)
            nc.sync.dma_start(dbg_d[nm][:], t[:])

        x2_t = [x2pool.tile([128, D], F32, tag=f"x2_{q}", name=f"x2_{q}")
                for q in range(NQB)]
        xs2T = [x2pool.tile([128, TPC], BF16, tag=f"xs2T{c}", name=f"xs2T{c}")
                for c in range(KC)]

        with ExitStack() as attn:
            apool = attn.enter_context(tc.tile_pool(name="attn", bufs=1))
            spool = attn.enter_context(tc.tile_pool(name="st", bufs=1))
            tpool = attn.enter_context(tc.tile_pool(name="tmp", bufs=4))
            zpool = attn.enter_context(tc.tile_pool(name="ztmp", bufs=4))
            psS = attn.enter_context(
                tc.tile_pool(name="psS", bufs=2, space="PSUM"))
            psPJ = attn.enter_context(
                tc.tile_pool(name="psPJ", bufs=2, space="PSUM"))
            psAV = attn.enter_context(
                tc.tile_pool(name="psAV", bufs=2, space="PSUM"))
            psT = attn.enter_context(
                tc.tile_pool(name="psT", bufs=2, space="PSUM"))

            npj = [0]

            def pj_tile():
                pool = (psS, psPJ)[npj[0] % 2]
                npj[0] += 1
                return pool.tile([128, 512], F32,
                                 tag="s" if pool is psS else "pj",
                                 name="s" if pool is psS else "pj")

            for _ in range(8):
                wps = pj_tile()
                nc.tensor.matmul(wps[:, 0:512], wrm[:, 0:128], wrm[:, 0:512],
                                 start=True, stop=True)

            # ---- phase A: LN1 + transpose ------------------------------
            def layer_norm(src, xs_out):
                st6 = tpool.tile([128, 6], F32, tag="ln_st6", name="ln_st6")
                nc.vector.bn_stats(st6[:], src)
                mv = tpool.tile([128, 2], F32, tag="ln_mv", name="ln_mv")
                nc.vector.bn_aggr(mv[:], st6[:])
                sd = tpool.tile([128, 1], F32, tag="ln_sd", name="ln_sd")
                nc.scalar.activation(sd[:], mv[:, 1:2], AF.Sqrt, bias=eps_t[:])
                rr = tpool.tile([128, 1], F32, tag="ln_rr", name="ln_rr")
                nc.vector.reciprocal_approx_fast(rr[:], sd[:])
                nc.vector.tensor_scalar(xs_out, src, mv[:, 0:1], rr[:],
                                        AluOpType.subtract, AluOpType.mult)

            xsT = [apool.tile([128, L], BF16, tag=f"xsT{c}", name=f"xsT{c}")
                   for c in range(KC)]
            ntp = 0
            for i in range(NT):
                xs = tpool.tile([128, D], BF16, tag="ln_xs", name="ln_xs",
                                bufs=2)
                nc.vector.tensor_scalar(xs[:], x_t(i), mu_c(i), rr_c(i),
                                        AluOpType.subtract, AluOpType.mult)
                pt = psT.tile([128, 512], BF16, tag="tp", name="tp")
                for c in range(KC):
                    nc.tensor.transpose(pt[:, 128 * c:128 * (c + 1)],
                                        xs[:, 128 * c:128 * (c + 1)], id_t[:])
                    if ntp % 2 == 0:
                        nc.vector.tensor_copy(xsT[c][:, 128 * i:128 * (i + 1)],
                                              pt[:, 128 * c:128 * (c + 1)])
                    else:
                        nc.scalar.copy(xsT[c][:, 128 * i:128 * (i + 1)],
                                       pt[:, 128 * c:128 * (c + 1)])
                    ntp += 1

            dump("d_xsT", xsT[0][:])

            # ---- phase B: K^T(diag), Qp/Qf, V variants, K_tok, G -------
            nfg = [0]

            def feat_from_psum(ps, out_ap, span):
                """out = elu(ps) + 1 elementwise; out bf16 (no bias --
                biases are folded in as K=1 matmul rows upstream)."""
                m = tpool.tile([128, 512], BF16, tag="feat_m", name="feat_m",
                               bufs=2)
                nc.vector.tensor_scalar_min(m[:, :span], ps[:, :span], 0.0)
                r = tpool.tile([128, 512], BF16, tag="feat_r", name="feat_r",
                               bufs=2)
                if nfg[0] % 2 == 0:
                    nc.scalar.activation(r[:, :span], ps[:, :span], AF.Relu)
                else:
                    nc.vector.tensor_relu(r[:, :span], ps[:, :span])
                nfg[0] += 1
                e = tpool.tile([128, 512], BF16, tag="feat_e", name="feat_e",
                               bufs=2)
                nc.scalar.activation(e[:, :span], m[:, :span], AF.Exp)
                nc.vector.tensor_add(out_ap, e[:, :span], r[:, :span])

            # (kT is built later by transposing K_tok blocks)
            kT = [apool.tile([128, TPC], BF16, tag=f"kT{c}", name=f"kT{c}")
                  for c in range(KC)]

            qp = [apool.tile([128, TPC], BF16, tag=f"qp{c}", name=f"qp{c}")
                  for c in range(KC)]
            qf = [apool.tile([128, TPC], BF16, tag=f"qf{c}", name=f"qf{c}")
                  for c in range(KC)]
            for c in range(KC):
                ps = pj_tile()
                for k in range(KC):
                    nc.tensor.matmul(
                        ps[:], wq_s(k, c), xsT[k][:, W:W + TPC],
                        start=(k == 0), stop=(nobias and k == KC - 1))
                if not nobias:
                    nc.tensor.matmul(
                        ps[:], brow_t[0:1, 4 * D + 128 * c:4 * D + 128 * (c + 1)],
                        ones1b[0:1, 0:TPC], start=False, stop=True)
                ft = tpool.tile([128, TPC], F32, tag="feat_q", name="feat_q",
                                bufs=2)
                feat_from_psum(ps, ft[:], TPC)
                nc.vector.tensor_mul(qp[c][:], ft[:], qpos_s(c))
                nc.vector.tensor_mul(qf[c][:], ft[:], qneg_s(c))

            dump("d_qp", qp[0][:])
            dump("d_qf", qf[0][:])

            # role maps for key 128-blocks
            pastkb, diagkb, futkb = set(), set(), set()
            for qi in range(NQC):
                s = W + qi * CH
                for j in range(nW):
                    pastkb.add((s - W + 128 * j) // 128)
                    futkb.add((s + CH + 128 * j) // 128)
                diagkb.add(s // 128)
                diagkb.add(s // 128 + 1)
            plainkb = sorted(pastkb | futkb)

            # K in token-major layout for every local block; the diagonal
            # blocks are additionally transposed into kT (feature-major).
            allkb = sorted(pastkb | futkb | diagkb)
            ktok = {}
            for i in allkb:
                ps = pj_tile()
                for k in range(KC):
                    nc.tensor.matmul(ps[:],
                                     xsT[k][:, 128 * i:128 * (i + 1)],
                                     wk_full(k),
                                     start=(k == 0),
                                     stop=(nobias and k == KC - 1))
                if not nobias:
                    nc.tensor.matmul(ps[:], ones1b[0:1, 0:128],
                                     brow_t[0:1, 3 * D:4 * D],
                                     start=False, stop=True)
                kt = apool.tile([128, D], BF16, tag=f"ktok{i}",
                                name=f"ktok{i}")
                feat_from_psum(ps, kt[:], D)
                ktok[i] = kt
                if i in diagkb:
                    pt = psT.tile([128, 512], BF16, tag="tp", name="tp")
                    for c in range(KC):
                        nc.tensor.transpose(pt[:, 128 * c:128 * (c + 1)],
                                            kt[:, 128 * c:128 * (c + 1)],
                                            id_t[:])
                        dst = kT[c][:, 128 * i - W:128 * (i + 1) - W]
                        if ntp % 2 == 0:
                            nc.vector.tensor_copy(dst, pt[:, 128 * c:128 * (c + 1)])
                        else:
                            nc.scalar.copy(dst, pt[:, 128 * c:128 * (c + 1)])
                        ntp += 1

            dump("d_kT", kT[0][:])


            # V psum -> decay-scaled variants (vaug) per (role, block)
            vaug = {}
            for i in range(NT):
                roles = []
                if i in pastkb:
                    roles.append("n")
                if i in diagkb:
                    roles.append("p")
                if i in futkb:
                    roles.append("f")
                if not roles:
                    continue
                ps = pj_tile()
                for k in range(KC):
                    nc.tensor.matmul(ps[:],
                                     xsT[k][:, 128 * i:128 * (i + 1)],
                                     wv_s(k),
                                     start=(k == 0),
                                     stop=(nobias and k == KC - 1))
                if not nobias:
                    nc.tensor.matmul(ps[:], ones1b[0:1, 0:128],
                                     brow_t[0:1, 0:D], start=False, stop=True)
                for nv, rname in enumerate(roles):
                    va = apool.tile([128, H * (HD + 1)], BF16,
                                    tag=f"vaug_{rname}{i}",
                                    name=f"vaug_{rname}{i}")
                    vaug[(rname, i)] = va
                    if uniform:
                        sc = vsc[rname](i, 0)
                        if (i + nv) % 2 == 0:
                            nc.scalar.activation(
                                va[:].rearrange("p (h d) -> p h d", h=H)[:, :, 0:64],
                                ps[:].rearrange("p (h d) -> p h d", h=H),
                                AF.Copy, scale=sc)
                        else:
                            nc.vector.tensor_scalar_mul(
                                va[:].rearrange("p (h d) -> p h d", h=H)[:, :, 0:64],
                                ps[:].rearrange("p (h d) -> p h d", h=H), sc)
                        nc.scalar.copy(
                            va[:].rearrange("p (h d) -> p h d", h=H)[:, :, 64:65],
                            sc.broadcast_to((128, H, 1)))
                    else:
                        for h in range(H):
                            sc = vsc[rname](i, h)
                            nc.scalar.activation(va[:, h * 65:h * 65 + 64],
                                                 ps[:, h * 64:(h + 1) * 64],
                                                 AF.Copy, scale=sc)
                            nc.scalar.copy(va[:, h * 65 + 64:h * 65 + 65], sc)

            if dbg:
                dump("d_vaug", vaug[("p", W // 128)][:])

            if dbg:
                dump("d_ktok", ktok[plainkb[0]][:])

            # G matrices: G[(role, kb, c)] [128, 65] bf16 -- head 2c in
            # partitions 0:64, head 2c+1 in 64:128.
            # G[f, m] = sum_key K_tok[key, f] * vaug[key, m]
            groles = []
            for i in sorted(pastkb):
                groles.append(("n", i))
            for i in sorted(futkb):
                groles.append(("f", i))
            gmat = {}
            ngc = 0
            for role, kb in groles:
                for c in range(KC):
                    gt = spool.tile([128, 65], BF16, tag=f"g{role}{kb}_{c}",
                                    name=f"g{role}{kb}_{c}")
                    gmat[(role, kb, c)] = gt
                    pg = pj_tile()
                    for hh in range(2):
                        h = 2 * c + hh
                        nc.tensor.matmul(
                            pg[64 * hh:64 * hh + 64, 0:65],
                            ktok[kb][:, 64 * h:64 * h + 64],
                            vaug[(role, kb)][:, h * 65:(h + 1) * 65],
                            start=True, stop=True,
                            tile_position=(0, 64 * hh))
                    if ngc % 2 == 0:
                        nc.vector.tensor_copy(gt[:], pg[0:128, 0:65])
                    else:
                        nc.scalar.copy(gt[:], pg[0:128, 0:65])
                    ngc += 1

            # ---- phase C1: diagonal QK scores (masked) -----------------
            # per qi the two diagonal blocks; head pairs emitted adjacently
            # so their K=64 matmuls run in different PE row groups.
            diag = []                     # (qi, kb, mi)
            for qi in range(NQC):
                s = W + qi * CH
                diag.append((qi, s // 128, 0))
                diag.append((qi, s // 128 + 1, 1))

            sts = {}       # (c, kb) -> paired st tile [128, 512]
            nst = 0
            numT = [apool.tile([128, TPC], BF16, tag=f"numT{c}",
                               name=f"numT{c}") for c in range(KC)]
            zall = zpool.tile([1, 16 * CH], F32, tag="zall", name="zall",
                              bufs=1)
            npv = [0]

            def qk_group(c):
                """diag QK for head pair (2c, 2c+1); separate PSUM banks
                (concurrent row-group matmuls may not share a bank), one
                paired st tile with a single masked mul."""
                nonlocal nst
                for qi, kb, mi in diag:
                    pss = [pj_tile(), pj_tile()]
                    for hh in range(2):
                        po = 64 * hh
                        nc.tensor.matmul(
                            pss[hh][:, 0:CH],
                            kT[c][po:po + 64,
                                  128 * kb - W:128 * (kb + 1) - W],
                            qp[c][po:po + 64, qi * CH:(qi + 1) * CH],
                            start=True, stop=True)
                    sr = tpool.tile([128, 2 * CH], BF16, tag="st_raw",
                                    name="st_raw", bufs=3)
                    for hh in range(2):
                        if (nst + hh) % 2 == 0:
                            nc.scalar.copy(sr[:, hh * CH:(hh + 1) * CH],
                                           pss[hh][:, 0:CH])
                        else:
                            nc.vector.tensor_copy(
                                sr[:, hh * CH:(hh + 1) * CH],
                                pss[hh][:, 0:CH])
                    st = spool.tile([128, 2 * CH], BF16,
                                    tag=f"st{c}_{kb}", name=f"st{c}_{kb}")
                    nc.vector.tensor_mul(st[:], sr[:], dm2_s(c, mi))
                    nst += 1
                    sts[(c, kb)] = st

            def av_group(c):
                pavq = []
                for qi in range(NQC):
                    s = W + qi * CH
                    qsl = slice(qi * CH, (qi + 1) * CH)
                    pavs = []
                    for hh in range(2):
                        if npv[0] % 2 == 0:
                            pav = psAV.tile([128, 2 * CH], F32, tag="av",
                                            name="av")
                        else:
                            pav = pj_tile()
                        npv[0] += 1
                        pavs.append(pav)
                    pavq.append(pavs)
                    kbs_d = [s // 128, s // 128 + 1]
                    kbs_n = [(s - W) // 128 + j for j in range(nW)]
                    kbs_f = [(s + CH) // 128 + j for j in range(nW)]
                    nterm = 2 + 2 * nW
                    for hh in range(2):
                        h = 2 * c + hh
                        po = 64 * hh
                        csl = slice(hh * CH, (hh + 1) * CH)
                        it = 0
                        for kb in kbs_d:
                            nc.tensor.matmul(
                                pavs[hh][0:65, 0:CH],
                                vaug[("p", kb)][:, h * 65:(h + 1) * 65],
                                sts[(c, kb)][:, csl],
                                start=(it == 0), stop=(it == nterm - 1))
                            it += 1
                        for kb in kbs_n:
                            nc.tensor.matmul(
                                pavs[hh][0:65, 0:CH],
                                gmat[("n", kb, c)][po:po + 64, :],
                                qp[c][po:po + 64, qsl],
                                start=(it == 0), stop=(it == nterm - 1))
                            it += 1
                        for kb in kbs_f:
                            nc.tensor.matmul(
                                pavs[hh][0:65, 0:CH],
                                gmat[("f", kb, c)][po:po + 64, :],
                                qf[c][po:po + 64, qsl],
                                start=(it == 0), stop=(it == nterm - 1))
                            it += 1
                        zi = 4 * c + 2 * qi + hh
                        nc.scalar.copy(zall[0:1, zi * CH:(zi + 1) * CH],
                                       pavs[hh][64:65, 0:CH])
                zlo = 4 * c * CH
                nc.vector.tensor_scalar_max(
                    zall[0:1, zlo:zlo + 4 * CH], zall[0:1, zlo:zlo + 4 * CH],
                    1e-6)
                nc.vector.reciprocal_approx_fast(
                    zall[0:1, zlo:zlo + 4 * CH], zall[0:1, zlo:zlo + 4 * CH])
                for qi in range(NQC):
                    qsl = slice(qi * CH, (qi + 1) * CH)
                    for hh in range(2):
                        po = 64 * hh
                        zo = (4 * c + 2 * qi + hh) * CH
                        zb = zpool.tile([64, CH], F32, tag="zbs", name="zbs")
                        nc.gpsimd.partition_broadcast(
                            zb[:], zall[0:1, zo:zo + CH])
                        nc.vector.tensor_mul(
                            numT[c][po:po + 64, qsl],
                            pavq[qi][hh][0:64, 0:CH],
                            zb[:])

            qk_group(0)
            for c in range(KC):
                if c + 1 < KC:
                    qk_group(c + 1)
                av_group(c)

            dump("d_numT", numT[0][:])

            # ---- phase D: O-proj + residual + LN2 ----------------------
            for q in range(NQB):
                ql = slice(128 * q, 128 * (q + 1))
                ps = pj_tile()
                for k in range(KC):
                    nc.tensor.matmul(ps[:], numT[k][:, ql], wo_s(k),
                                     start=(k == 0),
                                     stop=(nobias and k == KC - 1))
                if not nobias:
                    nc.tensor.matmul(ps[:], ones1b[0:1, 0:128],
                                     brow_t[0:1, D:2 * D],
                                     start=False, stop=True)
                xres = x_t((W + 128 * q) // 128)
                nc.vector.tensor_add(x2_t[q][:], ps[:], xres)

                xs2 = tpool.tile([128, D], BF16, tag="ln_xs2", name="ln_xs2",
                                 bufs=2)
                layer_norm(x2_t[q], xs2[:])
                pt = psT.tile([128, 512], BF16, tag="tp", name="tp")
                for c in range(KC):
                    nc.tensor.transpose(pt[:, 128 * c:128 * (c + 1)],
                                        xs2[:, 128 * c:128 * (c + 1)], id_t[:])
                    if ntp % 2 == 0:
                        nc.vector.tensor_copy(xs2T[c][:, ql],
                                              pt[:, 128 * c:128 * (c + 1)])
                    else:
                        nc.scalar.copy(xs2T[c][:, ql],
                                       pt[:, 128 * c:128 * (c + 1)])
                    ntp += 1

        dump("d_x2", x2_t[0][:])

        # ---- phase E: FFN ----------------------------------------------
        with ExitStack() as ffn:
            fpool = ffn.enter_context(tc.tile_pool(name="ffn", bufs=1))
            psF = ffn.enter_context(
                tc.tile_pool(name="psF", bufs=4, space="PSUM"))
            psF2 = ffn.enter_context(
                tc.tile_pool(name="psF2", bufs=3, space="PSUM"))

            h1gT = [fpool.tile([128, TPC], BF16, tag=f"h1gT{f}",
                               name=f"h1gT{f}") for f in range(FFC)]

            def ffn2(q):
                ql = slice(128 * q, 128 * (q + 1))
                ps2 = psF2.tile([128, D], F32, tag="f2", name="f2")
                for f in range(FFC):
                    nc.tensor.matmul(ps2[:],
                                     h1gT[f][:, ql],
                                     w2_t[:, f * D:(f + 1) * D],
                                     start=(f == 0),
                                     stop=(nobias and f == FFC - 1))
                if not nobias:
                    nc.tensor.matmul(ps2[:], ones1b[0:1, 0:128],
                                     brow_t[0:1, 2 * D:3 * D],
                                     start=False, stop=True)
                o = fpool.tile([128, D], F32, tag="fout", name="fout", bufs=2)
                nc.vector.tensor_add(o[:], ps2[:], x2_t[q][:])
                nc.sync.dma_start(out[ql, :], o[:])

            for qh in range(2):
                hsl = slice(CH * qh, CH * (qh + 1))
                for f in range(FFC):
                    ps = psF.tile([128, CH], F32, tag="f1", name="f1")
                    for k in range(KC):
                        nc.tensor.matmul(
                            ps[:],
                            w1_t[:, k * FF + 128 * f:k * FF + 128 * (f + 1)],
                            xs2T[k][:, hsl],
                            start=(k == 0), stop=(k == KC - 1))
                    nc.scalar.activation(h1gT[f][:, hsl], ps[:],
                                         AF.Gelu, bias=bf1_c(f))
                ffn2(2 * qh)
                ffn2(2 * qh + 1)
            dump("d_h1", h1gT[0][:])

    nc.compile()
    return nc


def _get_nc(W, uniform, nobias):
    key = (W, uniform, nobias)
    if key not in _BUILD_CACHE:
        _BUILD_CACHE[key] = _build(W, uniform, nobias)
    return _BUILD_CACHE[key]


# ---------------------------------------------------------------------------
# host side
# ---------------------------------------------------------------------------

def _bf16(a):
    return np.asarray(a, np.float32).astype(ml_dtypes.bfloat16)


def kernel(x, mask, wq, wk, wv, wo, bo, g1, b1, g2, b2, w1, bf1, w2, bf2,
           decay_logit):
    x = np.asarray(x, np.float32)
    mask = np.asarray(mask)
    g1 = np.asarray(g1, np.float64)
    b1 = np.asarray(b1, np.float64)
    g2 = np.asarray(g2, np.float64)
    b2 = np.asarray(b2, np.float64)

    d64 = 1.0 / (1.0 + np.exp(-np.asarray(decay_logit, np.float64)))
    d64 = np.clip(d64, 1e-8, None)
    dmax = float(d64.max())
    # Band width: the attention kernel truncates at |i-j| > ~W.  The omitted
    # mass relative to the normalizer z is ~ d^(W+1)/(1+d); keep it under
    # 1e-5, well below the bf16 matmul noise (~1e-3).
    if dmax >= 1.0 - 1e-12:
        W = 512  # decay ~1: widest supported band
    else:
        need = math.log(1e-5 * (1.0 + dmax)) / math.log(dmax)
        W = max(128, 128 * math.ceil(need / 128))
        W = min(W, 512)
    nW = W // 128
    L = TPC + 2 * W
    NT = L // 128
    uniform = bool(np.all(d64 == d64[0]))
    nobias = bool(np.all(b1 == 0) and np.all(np.asarray(bo) == 0)
                  and np.all(np.asarray(bf2) == 0))
    NHM = 1 if uniform else H
    NQP = 1 if uniform else KC

    nc = _get_nc(W, uniform, nobias)

    # folded weights (bf16)
    wq_f = _bf16(g1[:, None] * np.asarray(wq, np.float64))
    wk_f = _bf16(g1[:, None] * np.asarray(wk, np.float64))
    wv_f = _bf16(g1[:, None] * np.asarray(wv, np.float64))
    wo_f = _bf16(wo)
    w1_f = _bf16(g2[:, None] * np.asarray(w1, np.float64))
    w2_f = _bf16(w2)
    bq_h = _bf16(b1 @ np.asarray(wq, np.float64))
    bk_h = _bf16(b1 @ np.asarray(wk, np.float64))
    bv_h = _bf16(b1 @ np.asarray(wv, np.float64))
    bo_h = _bf16(bo)
    bf1_h = (b2 @ np.asarray(w1, np.float64) +
             np.asarray(bf1, np.float64)).astype(np.float32)
    bf2_h = _bf16(bf2)

    def pack_kblocks(w, blk, nb):
        # [nb*128, cols] -> [128, nb*cols] with block k at cols [k*cols, ...)
        return np.ascontiguousarray(
            w.reshape(nb, 128, blk).transpose(1, 0, 2).reshape(128, nb * blk))

    # (dmask appended to wa below, bf16)
    wb_h = np.concatenate([
        pack_kblocks(wv_f, 512, 4),
        pack_kblocks(wo_f, 512, 4)], axis=1)
    w1_h = pack_kblocks(w1_f, FF, 4)
    w2_h = pack_kblocks(w2_f, D, 16)
    brow_h = np.concatenate(
        [bv_h, bo_h, bf2_h, bk_h, bq_h]).reshape(1, 5 * D)
    brow_h = brow_h.astype(ml_dtypes.bfloat16)

    # decay-derived tables
    ld = np.log(d64)                                    # (H,)
    di = np.arange(TPC, dtype=np.float64) % CH
    qposb = np.zeros((NQP, 128, TPC), np.float32)
    qnegb = np.zeros((NQP, 128, TPC), np.float32)
    for c in range(NQP):
        for hh in range(2):
            h = 2 * c + hh
            qposb[c, 64 * hh:64 * hh + 64, :] = np.exp(ld[h] * di)[None, :]
            qnegb[c, 64 * hh:64 * hh + 64, :] = np.exp(-ld[h] * di)[None, :]
    if uniform:
        qposb[:, 64:, :] = qposb[:, :64, :]
        qnegb[:, 64:, :] = qnegb[:, :64, :]
    # diagonal-zone mask: m[dk, di] = d^{|di-dk| - di}, dk in [0,256)
    dic = np.arange(CH, dtype=np.float64)
    dk = np.arange(CH, dtype=np.float64)
    dmask = np.zeros((NHM, 2, 128, CH), np.float32)
    for h in range(NHM):
        m = np.exp(ld[h] * (np.abs(dic[None, :] - dk[:, None]) - dic[None, :]))
        dmask[h, 0] = m[:128, :]
        dmask[h, 1] = m[128:, :]
    cb_h = np.concatenate([
        qposb.transpose(1, 0, 2).reshape(128, NQP * TPC),
        qnegb.transpose(1, 0, 2).reshape(128, NQP * TPC)], axis=1)
    cb_h = np.ascontiguousarray(cb_h, np.float32)
    NPR = 1 if uniform else KC
    dm2 = np.zeros((NPR, 2, 128, 2 * CH), np.float32)
    for p in range(NPR):
        for mi in range(2):
            dm2[p, mi, :, :CH] = dmask[(2 * p) % NHM, mi]
            dm2[p, mi, :, CH:] = dmask[(2 * p + 1) % NHM, mi]
    wa_h = np.concatenate([
        pack_kblocks(wk_f, 512, 4),
        pack_kblocks(wq_f, 512, 4),
        dm2.transpose(2, 0, 1, 3).reshape(128, NPR * 2 * 2 * CH)
           .astype(ml_dtypes.bfloat16)], axis=1)

    in_maps = []
    for core in range(NCORES):
        b = core // SH
        t0 = (core % SH) * TPC
        lo, hi = t0 - W, t0 + TPC + W
        xs = np.zeros((L, D), np.float32)
        pad = np.zeros((L,), np.float64)
        glo, ghi = max(lo, 0), min(hi, T)
        xs[glo - lo:ghi - lo] = x[b, glo:ghi]
        pad[glo - lo:ghi - lo] = (~mask[b, glo:ghi]).astype(np.float64)
        xbig = np.ascontiguousarray(
            xs.reshape(NT, 128, D).transpose(1, 0, 2).reshape(128, NT * D))

        vneg = np.zeros((H, L), np.float32)
        vpos = np.zeros((H, L), np.float32)
        vpln = np.zeros((H, L), np.float32)
        for h in range(H):
            vn = np.zeros(L)
            vp = np.zeros(L)
            for qi in range(NQC):
                s = W + qi * CH
                jj = np.arange(s - W, s)
                vn[jj] = np.exp(ld[h] * (s - jj))
                jj = np.arange(s + CH, s + CH + W)
                vp[jj] = np.exp(ld[h] * (CH + jj - (s + CH)))
            vneg[h] = (vn * pad).astype(np.float32)
            vpos[h] = (vp * pad).astype(np.float32)
            vpln[h] = pad.astype(np.float32)

        def _vlayout(a):
            return np.ascontiguousarray(
                a.reshape(H, L // 128, 128).transpose(2, 1, 0)
                 .reshape(128, (L // 128) * H))
        vneg, vpos, vpln = _vlayout(vneg), _vlayout(vpos), _vlayout(vpln)

        mu = xs.mean(axis=1)
        rr = 1.0 / np.sqrt(xs.var(axis=1) + 1e-5)
        mr = np.stack([mu, rr], axis=1).astype(np.float32)    # [L, 2]
        mr = mr.reshape(NT, 128, 2).transpose(1, 0, 2).reshape(128, 2 * NT)
        cs_h = np.concatenate([
            np.zeros((128, 8), np.float32),      # (bias cols retired)
            bf1_h.reshape(16, 128).T,
            vneg, vpos, vpln, mr], axis=1)
        cs_h = np.ascontiguousarray(cs_h, np.float32)

        nh = NT // 2
        im = {
            "xh0": np.ascontiguousarray(xbig[:, :nh * D]),
            "xh1": np.ascontiguousarray(xbig[:, nh * D:]),
            "ident": np.eye(128, dtype=ml_dtypes.bfloat16),
            "cs": cs_h, "cb": cb_h,
            "wa": wa_h, "wb": wb_h, "w1": w1_h, "w2": w2_h,
        }
        if not nobias:
            im["brow"] = brow_h
        in_maps.append(im)

    res = bass_utils.run_bass_kernel_spmd(nc, in_maps,
                                          core_ids=list(range(NCORES)))
    out = np.empty((B, T, D), np.float32)
    for core in range(NCORES):
        b = core // SH
        t0 = (core % SH) * TPC
        out[b, t0:t0 + TPC] = res.results[core]["out"]

    # Degenerate-mask patch: a query whose entire +-W neighbourhood is
    # masked has z ~ 0 on the device; the reference output is then
    # dominated by out-of-band keys that the banded kernel truncates.
    # Recompute those rows exactly on the host (never triggers for an
    # all-False mask).
    if mask.any():
        idx = np.arange(T)
        for b_ in range(B):
            keep = ~np.asarray(mask[b_])
            if keep.all():
                continue
            kpos = idx[keep]
            if len(kpos) == 0:
                dist = np.full(T, T)
            else:
                ins = np.searchsorted(kpos, idx)
                left = np.where(ins > 0, idx - kpos[np.clip(ins - 1, 0,
                                                            len(kpos) - 1)], T)
                right = np.where(ins < len(kpos),
                                 kpos[np.clip(ins, 0, len(kpos) - 1)] - idx, T)
                dist = np.minimum(left, right)
            need = dist > (W - 32)
            if need.any():
                out[b_, need] = _exact_rows(
                    np.asarray(x[b_], np.float64), keep.astype(np.float64),
                    np.where(need)[0], wq, wk, wv, wo, bo, g1, b1, g2, b2,
                    w1, bf1, w2, bf2, d64)
    return out


def _exact_rows(xb, pad, rows, wq, wk, wv, wo, bo, g1, b1, g2, b2,
                w1, bf1, w2, bf2, d64):
    """Reference math (float64) for the given query rows of one batch."""
    import scipy.special as _sp

    def ln(z, g, b):
        mu = z.mean(-1, keepdims=True)
        var = ((z - mu) ** 2).mean(-1, keepdims=True)
        return (z - mu) / np.sqrt(var + 1e-5) * g + b

    wq = np.asarray(wq, np.float64); wk = np.asarray(wk, np.float64)
    wv = np.asarray(wv, np.float64); wo = np.asarray(wo, np.float64)
    w1 = np.asarray(w1, np.float64); w2 = np.asarray(w2, np.float64)
    xn = ln(xb, np.asarray(g1, np.float64), np.asarray(b1, np.float64))

    def feat(z):
        return np.where(z > 0, z + 1.0, np.exp(np.minimum(z, 0.0)))

    Q = feat((xn[rows] @ wq).reshape(len(rows), H, HD))       # (R,H,hd)
    K = feat((xn @ wk).reshape(T, H, HD)) * pad[:, None, None]
    V = (xn @ wv).reshape(T, H, HD) * pad[:, None, None]
    dist = np.abs(rows[:, None] - np.arange(T)[None, :]).astype(np.float64)
    M = np.exp(dist[None] * np.log(d64)[:, None, None])       # (H,R,T)
    A = np.einsum('rhd,jhd->hrj', Q, K) * M
    z = np.clip(A.sum(-1, keepdims=True), 1e-6, None)
    o = np.einsum('hrj,jhd->rhd', A / z, V).reshape(len(rows), D)
    o = o @ wo + np.asarray(bo, np.float64)
    x2 = xb[rows] + o
    xn2 = ln(x2, np.asarray(g2, np.float64), np.asarray(b2, np.float64))
    h1 = xn2 @ w1 + np.asarray(bf1, np.float64)
    gl = 0.5 * h1 * (1.0 + _sp.erf(h1 / np.sqrt(2.0)))
    ffn = gl @ w2 + np.asarray(bf2, np.float64)
    return (x2 + ffn).astype(np.float32)


# revision 43
# speedup vs baseline: 1.1154x; 1.0554x over previous
"""Trainium2 Bass kernel for a bidirectional linear-attention transformer layer.

Contract: kernel(**inputs) takes the FULL unsharded inputs (as produced by the
problem's setup_inputs()) and returns the FULL (B, T, D) float32 output.

Sharding: token-parallel across 8 NeuronCores. Core c handles batch c//4,
query tokens [ (c%4)*512, (c%4)*512+512 ), with a +-W token halo (W=128 for
decay=0.9). The decay mask d^|i-j| factorizes per 256-token query chunk into
  - a Q-side scale d^{+di} (past/diag) or d^{-di} (future), baked into two
    pre-scaled copies of Q,
  - a K-side scale folded into V rows and into an extra "ones" column of V
    that simultaneously produces the normalization denominator z,
  - an elementwise 256x256 mask for the diagonal zone only.

For the off-diagonal (fully factorized) zones the attention is linearized:
    out_chunk += (K_blk^T V_blk')^T q_chunk        (G-trick)
so only the diagonal zone materializes scores.  All heavy matmuls run in
bf16 (fp32 PSUM accumulate); the residual path and statistics stay fp32.

Structure (per core):
  A  LN1 + PE transposes            -> xsT (bf16, [d, tok])
  B  K^T (diag cols), Qp/Qf, V+decay variants, K_tok, G matrices
  C1 diagonal QK scores             -> st tiles (bf16, masked)
  C2 AV chains (diag st + G terms)  -> numT (normalized, bf16)
  D  O-proj + residual + LN2        -> x2 (f32), xs2T (bf16)
  E  FFN1 + gelu + FFN2             -> out
Head pairs (2c, 2c+1) occupy PE row groups 0-63 / 64-127, so their K=64
matmuls execute concurrently when emitted adjacently.
"""

import math
import os
from contextlib import ExitStack

import numpy as np
import ml_dtypes

import concourse.bacc as bacc
import concourse.mybir as mybir
import concourse.tile as tile
from concourse import bass_utils
from concourse.alu_op_type import AluOpType

F32 = mybir.dt.float32
BF16 = mybir.dt.bfloat16
AF = mybir.ActivationFunctionType

B, T, D, H, FF = 2, 2048, 512, 8, 2048
HD = D // H          # 64
NCORES = 8
SH = 4               # token shards per batch
TPC = T // SH        # 512 query tokens per core
CH = 256             # query chunk width
NQC = TPC // CH      # 2 query chunks per core
KC = D // 128        # 4 contraction chunks over D
FFC = FF // 128      # 16 ff chunks

_BUILD_CACHE = {}


def _build(W, uniform, nobias):
    nW = W // 128            # past/future 128-blocks per q-chunk
    L = TPC + 2 * W          # local tokens per core
    NT = L // 128            # 128-token tiles per core
    NQB = TPC // 128         # 4 query 128-blocks
    NHM = 1 if uniform else H    # distinct decay tables needed
    NQP = 1 if uniform else KC

    CS_COLS = 24 + 3 * NT * H + 2 * NT        # fp32 pack 1
    O_VN = 24
    O_MR = 24 + 3 * NT * H
    O_QP = 0                                  # fp32 pack 2 offsets
    O_QN = NQP * TPC
    CB_COLS = 2 * NQP * TPC
    NPR = 1 if uniform else KC                # distinct head-pair masks
    O_DM = 2 * KC * 512                       # dmask lives in wa (bf16)
    WA_COLS = O_DM + NPR * 2 * 2 * CH

    nc = bacc.Bacc("TRN2", target_bir_lowering=False, debug=False)

    def dram_in(name, shape, dt=F32):
        return nc.dram_tensor(name, shape, dt, kind="ExternalInput").ap()

    # DMA issue order == declaration/use order below (HWDGE FIFO):
    xbig_a = dram_in("xh0", [128, (NT // 2) * D])     # x tokens, first half
    id_d = dram_in("ident", [128, 128], BF16)
    cs_d = dram_in("cs", [128, CS_COLS])              # small fp32 consts
    xbig_b = dram_in("xh1", [128, (NT - NT // 2) * D])
    wa_d = dram_in("wa", [128, WA_COLS], BF16)   # wk|wq|dmask
    cb_d = dram_in("cb", [128, CB_COLS])              # qpos|qneg|dmask
    wb_d = dram_in("wb", [128, 2 * KC * 512], BF16)   # wv|wo
    if not nobias:
        brow_d = dram_in("brow", [1, 5 * D], BF16)    # bv|bo|bf2|bk|bq rows
    w1_d = dram_in("w1", [128, KC * FF], BF16)
    w2_d = dram_in("w2", [128, FFC * D], BF16)

    out = nc.dram_tensor("out", [TPC, D], F32, kind="ExternalOutput").ap()

    dbg = os.environ.get("KDBG") == "1"
    dbg_d = {}
    if dbg:
        for nm, shape in [("d_xsT", [128, L]), ("d_kT", [128, TPC]),
                          ("d_qp", [128, TPC]), ("d_qf", [128, TPC]),
                          ("d_vaug", [128, H * (HD + 1)]),
                          ("d_ktok", [128, D]), ("d_numT", [128, TPC]),
                          ("d_x2", [128, D]), ("d_h1", [128, TPC])]:
            dbg_d[nm] = nc.dram_tensor(nm, shape, F32,
                                       kind="ExternalOutput").ap()

    with tile.TileContext(nc) as tc, ExitStack() as top:
        cpool = top.enter_context(tc.tile_pool(name="const", bufs=1))
        xpool = top.enter_context(tc.tile_pool(name="x", bufs=1))
        x2pool = top.enter_context(tc.tile_pool(name="x2", bufs=1))

        # ---- input DMAs (one per packed tensor) ------------------------
        cs_t = cpool.tile([128, CS_COLS], F32, tag="cs", name="cs")
        nc.sync.dma_start(cs_t[:], cs_d[:])
        xh0 = xpool.tile([128, (NT // 2) * D], F32, tag="xh0", name="xh0")
        nc.sync.dma_start(xh0[:], xbig_a[:])
        id_t = cpool.tile([128, 128], BF16, tag="ident", name="ident")
        nc.sync.dma_start(id_t[:], id_d[:])

        def x_t(i):
            if i < NT // 2:
                return xh0[:, i * D:(i + 1) * D]
            j = i - NT // 2
            return xh1[:, j * D:(j + 1) * D]

        xh1 = xpool.tile([128, (NT - NT // 2) * D], F32, tag="xh1", name="xh1")
        nc.sync.dma_start(xh1[:], xbig_b[:])
        wa_t = cpool.tile([128, WA_COLS], BF16, tag="wa", name="wa")
        nc.sync.dma_start(wa_t[:], wa_d[:])
        cb_t = cpool.tile([128, CB_COLS], F32, tag="cb", name="cb")
        nc.sync.dma_start(cb_t[:], cb_d[:])
        wb_t = cpool.tile([128, 2 * KC * 512], BF16, tag="wb", name="wb")
        nc.sync.dma_start(wb_t[:], wb_d[:])
        if not nobias:
            brow_t = cpool.tile([1, 5 * D], BF16, tag="brow", name="brow")
            nc.sync.dma_start(brow_t[:], brow_d[:])
        w1_t = cpool.tile([128, KC * FF], BF16, tag="w1", name="w1")
        nc.sync.dma_start(w1_t[:], w1_d[:])
        w2_t = cpool.tile([128, FFC * D], BF16, tag="w2", name="w2")
        nc.sync.dma_start(w2_t[:], w2_d[:])

        def wk_s(k, c):
            return wa_t[:, k * 512 + 128 * c:k * 512 + 128 * (c + 1)]

        def wq_s(k, c):
            return wa_t[:, 2048 + k * 512 + 128 * c:2048 + k * 512 + 128 * (c + 1)]

        def wk_full(k):
            return wa_t[:, k * 512:(k + 1) * 512]

        def wv_s(k):
            return wb_t[:, k * 512:(k + 1) * 512]

        def wo_s(k):
            return wb_t[:, 2048 + k * 512:2048 + k * 512 + 512]

        bf1_c = lambda f: cs_t[:, 8 + f:9 + f]
        vsc = {"n": lambda i, h: cs_t[:, O_VN + i * H + h:O_VN + i * H + h + 1],
               "f": lambda i, h: cs_t[:, O_VN + NT * H + i * H + h:
                                      O_VN + NT * H + i * H + h + 1],
               "p": lambda i, h: cs_t[:, O_VN + 2 * NT * H + i * H + h:
                                      O_VN + 2 * NT * H + i * H + h + 1]}
        mu_c = lambda i: cs_t[:, O_MR + 2 * i:O_MR + 2 * i + 1]
        rr_c = lambda i: cs_t[:, O_MR + 2 * i + 1:O_MR + 2 * i + 2]
        qpos_s = lambda c: cb_t[:, O_QP + (c % NQP) * TPC:
                                O_QP + (c % NQP) * TPC + TPC]
        qneg_s = lambda c: cb_t[:, O_QN + (c % NQP) * TPC:
                                O_QN + (c % NQP) * TPC + TPC]
        # paired mask: [dm(2c) | dm(2c+1)] per mi, 512 wide
        dm2_s = lambda c, j: wa_t[:, O_DM + ((c % NPR) * 2 + j) * 2 * CH:
                                  O_DM + ((c % NPR) * 2 + j) * 2 * CH + 2 * CH]

        eps_t = cpool.tile([128, 1], F32, tag="eps", name="eps")
        nc.gpsimd.memset(eps_t[:], 1e-5)
        ones1b = cpool.tile([1, 512], BF16, tag="on1b", name="on1b")
        nc.gpsimd.memset(ones1b[:], 1.0)
        # warm the ACT LUT tables while input DMAs run
        warm_t = cpool.tile([1, 1], F32, tag="warm", name="warm")
        for fn in (AF.Copy, AF.Sqrt, AF.Exp, AF.Relu, AF.Gelu):
            nc.scalar.activation(warm_t[:], eps_t[0:1, :], fn)
        # PE warm-up: keep the PE busy (and the HAM un-throttled) during the
        # initial DMA wait so real matmuls run at 2.4 GHz from the start.
        wrm = cpool.tile([128, 512], BF16, tag="wrm", name="wrm")
        nc.gpsimd.memset(wrm[:], 0.0)
        # pre-load the Q7 PartitionBroadcast library during the DMA wait --
        # the first broadcast otherwise pays a ~7us LIBRARY_RELOAD mid-C2.
        pbw = cpool.tile([64, 8], F32, tag="pbw", name="pbw")
        nc.gpsimd.partition_broadcast(pbw[:], eps_t[0:1, 0:1].broadcast_to((1, 8)))

        def dump(nm, ap):
            if not dbg:
                return
            t = x2pool.tile(list(ap.shape), F32, tag="dbg_" + nm,
                            name="dbg_" + nm)
            nc.vector.tensor_copy(t[:], ap)
            nc.sync.dma_start(dbg_d[nm][:], t[:])

        x2_t = [x2pool.tile([128, D], F32, tag=f"x2_{q}", name=f"x2_{q}")
                for q in range(NQB)]
        xs2T = [x2pool.tile([128, TPC], BF16, tag=f"xs2T{c}", name=f"xs2T{c}")
                for c in range(KC)]

        with ExitStack() as attn:
            apool = attn.enter_context(tc.tile_pool(name="attn", bufs=1))
            spool = attn.enter_context(tc.tile_pool(name="st", bufs=1))
            tpool = attn.enter_context(tc.tile_pool(name="tmp", bufs=4))
            zpool = attn.enter_context(tc.tile_pool(name="ztmp", bufs=4))
            psS = attn.enter_context(
                tc.tile_pool(name="psS", bufs=2, space="PSUM"))
            psPJ = attn.enter_context(
                tc.tile_pool(name="psPJ", bufs=2, space="PSUM"))
            psAV = attn.enter_context(
                tc.tile_pool(name="psAV", bufs=2, space="PSUM"))
            psT = attn.enter_context(
                tc.tile_pool(name="psT", bufs=2, space="PSUM"))

            npj = [0]

            def pj_tile():
                pool = (psS, psPJ)[npj[0] % 2]
                npj[0] += 1
                return pool.tile([128, 512], F32,
                                 tag="s" if pool is psS else "pj",
                                 name="s" if pool is psS else "pj")

            for _ in range(8):
                wps = pj_tile()
                nc.tensor.matmul(wps[:, 0:512], wrm[:, 0:128], wrm[:, 0:512],
                                 start=True, stop=True)

            # ---- phase A: LN1 + transpose ------------------------------
            def layer_norm(src, xs_out):
                st6 = tpool.tile([128, 6], F32, tag="ln_st6", name="ln_st6")
                nc.vector.bn_stats(st6[:], src)
                mv = tpool.tile([128, 2], F32, tag="ln_mv", name="ln_mv")
                nc.vector.bn_aggr(mv[:], st6[:])
                sd = tpool.tile([128, 1], F32, tag="ln_sd", name="ln_sd")
                nc.scalar.activation(sd[:], mv[:, 1:2], AF.Sqrt, bias=eps_t[:])
                rr = tpool.tile([128, 1], F32, tag="ln_rr", name="ln_rr")
                nc.vector.reciprocal_approx_fast(rr[:], sd[:])
                nc.vector.tensor_scalar(xs_out, src, mv[:, 0:1], rr[:],
                                        AluOpType.subtract, AluOpType.mult)

            xsT = [apool.tile([128, L], BF16, tag=f"xsT{c}", name=f"xsT{c}")
                   for c in range(KC)]
            ntp = 0
            for i in range(NT):
                xs = tpool.tile([128, D], BF16, tag="ln_xs", name="ln_xs",
                                bufs=2)
                nc.vector.tensor_scalar(xs[:], x_t(i), mu_c(i), rr_c(i),
                                        AluOpType.subtract, AluOpType.mult)
                pt = psT.tile([128, 512], BF16, tag="tp", name="tp")
                for c in range(KC):
                    nc.tensor.transpose(pt[:, 128 * c:128 * (c + 1)],
                                        xs[:, 128 * c:128 * (c + 1)], id_t[:])
                    if ntp % 2 == 0:
                        nc.vector.tensor_copy(xsT[c][:, 128 * i:128 * (i + 1)],
                                              pt[:, 128 * c:128 * (c + 1)])
                    else:
                        nc.scalar.copy(xsT[c][:, 128 * i:128 * (i + 1)],
                                       pt[:, 128 * c:128 * (c + 1)])
                    ntp += 1

            dump("d_xsT", xsT[0][:])

            # ---- phase B: K^T(diag), Qp/Qf, V variants, K_tok, G -------
            nfg = [0]

            def feat_from_psum(ps, out_ap, span):
                """out = elu(ps) + 1 elementwise; out bf16 (no bias --
                biases are folded in as K=1 matmul rows upstream)."""
                m = tpool.tile([128, 512], BF16, tag="feat_m", name="feat_m",
                               bufs=2)
                nc.vector.tensor_scalar_min(m[:, :span], ps[:, :span], 0.0)
                r = tpool.tile([128, 512], BF16, tag="feat_r", name="feat_r",
                               bufs=2)
                if nfg[0] % 2 == 0:
                    nc.scalar.activation(r[:, :span], ps[:, :span], AF.Relu)
                else:
                    nc.vector.tensor_relu(r[:, :span], ps[:, :span])
                nfg[0] += 1
                e = tpool.tile([128, 512], BF16, tag="feat_e", name="feat_e",
                               bufs=2)
                nc.scalar.activation(e[:, :span], m[:, :span], AF.Exp)
                nc.vector.tensor_add(out_ap, e[:, :span], r[:, :span])

            # (kT is built later by transposing K_tok blocks)
            kT = [apool.tile([128, TPC], BF16, tag=f"kT{c}", name=f"kT{c}")
                  for c in range(KC)]

            qp = [apool.tile([128, TPC], BF16, tag=f"qp{c}", name=f"qp{c}")
                  for c in range(KC)]
            qf = [apool.tile([128, TPC], BF16, tag=f"qf{c}", name=f"qf{c}")
                  for c in range(KC)]
            for c in range(KC):
                ps = pj_tile()
                for k in range(KC):
                    nc.tensor.matmul(
                        ps[:], wq_s(k, c), xsT[k][:, W:W + TPC],
                        start=(k == 0), stop=(nobias and k == KC - 1))
                if not nobias:
                    nc.tensor.matmul(
                        ps[:], brow_t[0:1, 4 * D + 128 * c:4 * D + 128 * (c + 1)],
                        ones1b[0:1, 0:TPC], start=False, stop=True)
                ft = tpool.tile([128, TPC], F32, tag="feat_q", name="feat_q",
                                bufs=2)
                feat_from_psum(ps, ft[:], TPC)
                nc.vector.tensor_mul(qp[c][:], ft[:], qpos_s(c))
                nc.vector.tensor_mul(qf[c][:], ft[:], qneg_s(c))

            dump("d_qp", qp[0][:])
            dump("d_qf", qf[0][:])

            # role maps for key 128-blocks
            pastkb, diagkb, futkb = set(), set(), set()
            for qi in range(NQC):
                s = W + qi * CH
                for j in range(nW):
                    pastkb.add((s - W + 128 * j) // 128)
                    futkb.add((s + CH + 128 * j) // 128)
                diagkb.add(s // 128)
                diagkb.add(s // 128 + 1)
            plainkb = sorted(pastkb | futkb)

            # K in token-major layout for every local block; the diagonal
            # blocks are additionally transposed into kT (feature-major).
            allkb = sorted(pastkb | futkb | diagkb)
            ktok = {}
            for i in allkb:
                ps = pj_tile()
                for k in range(KC):
                    nc.tensor.matmul(ps[:],
                                     xsT[k][:, 128 * i:128 * (i + 1)],
                                     wk_full(k),
                                     start=(k == 0),
                                     stop=(nobias and k == KC - 1))
                if not nobias:
                    nc.tensor.matmul(ps[:], ones1b[0:1, 0:128],
                                     brow_t[0:1, 3 * D:4 * D],
                                     start=False, stop=True)
                kt = apool.tile([128, D], BF16, tag=f"ktok{i}",
                                name=f"ktok{i}")
                feat_from_psum(ps, kt[:], D)
                ktok[i] = kt
                if i in diagkb:
                    pt = psT.tile([128, 512], BF16, tag="tp", name="tp")
                    for c in range(KC):
                        nc.tensor.transpose(pt[:, 128 * c:128 * (c + 1)],
                                            kt[:, 128 * c:128 * (c + 1)],
                                            id_t[:])
                        dst = kT[c][:, 128 * i - W:128 * (i + 1) - W]
                        if ntp % 2 == 0:
                            nc.vector.tensor_copy(dst, pt[:, 128 * c:128 * (c + 1)])
                        else:
                            nc.scalar.copy(dst, pt[:, 128 * c:128 * (c + 1)])
                        ntp += 1

            dump("d_kT", kT[0][:])


            # V psum -> decay-scaled variants (vaug) per (role, block)
            vaug = {}
            for i in range(NT):
                roles = []
                if i in pastkb:
                    roles.append("n")
                if i in diagkb:
                    roles.append("p")
                if i in futkb:
                    roles.append("f")
                if not roles:
                    continue
                ps = pj_tile()
                for k in range(KC):
                    nc.tensor.matmul(ps[:],
                                     xsT[k][:, 128 * i:128 * (i + 1)],
                                     wv_s(k),
                                     start=(k == 0),
                                     stop=(nobias and k == KC - 1))
                if not nobias:
                    nc.tensor.matmul(ps[:], ones1b[0:1, 0:128],
                                     brow_t[0:1, 0:D], start=False, stop=True)
                for nv, rname in enumerate(roles):
                    va = apool.tile([128, H * (HD + 1)], BF16,
                                    tag=f"vaug_{rname}{i}",
                                    name=f"vaug_{rname}{i}")
                    vaug[(rname, i)] = va
                    if uniform:
                        sc = vsc[rname](i, 0)
                        if (i + nv) % 2 == 0:
                            nc.scalar.activation(
                                va[:].rearrange("p (h d) -> p h d", h=H)[:, :, 0:64],
                                ps[:].rearrange("p (h d) -> p h d", h=H),
                                AF.Copy, scale=sc)
                        else:
                            nc.vector.tensor_scalar_mul(
                                va[:].rearrange("p (h d) -> p h d", h=H)[:, :, 0:64],
                                ps[:].rearrange("p (h d) -> p h d", h=H), sc)
                        nc.scalar.copy(
                            va[:].rearrange("p (h d) -> p h d", h=H)[:, :, 64:65],
                            sc.broadcast_to((128, H, 1)))
                    else:
                        for h in range(H):
                            sc = vsc[rname](i, h)
                            nc.scalar.activation(va[:, h * 65:h * 65 + 64],
                                                 ps[:, h * 64:(h + 1) * 64],
                                                 AF.Copy, scale=sc)
                            nc.scalar.copy(va[:, h * 65 + 64:h * 65 + 65], sc)

            if dbg:
                dump("d_vaug", vaug[("p", W // 128)][:])

            if dbg:
                dump("d_ktok", ktok[plainkb[0]][:])

            # G matrices: G[(role, kb, c)] [128, 65] bf16 -- head 2c in
            # partitions 0:64, head 2c+1 in 64:128.
            # G[f, m] = sum_key K_tok[key, f] * vaug[key, m]
            groles = []
            for i in sorted(pastkb):
                groles.append(("n", i))
            for i in sorted(futkb):
                groles.append(("f", i))
            gmat = {}
            ngc = 0
            for role, kb in groles:
                for c in range(KC):
                    gt = spool.tile([128, 65], BF16, tag=f"g{role}{kb}_{c}",
                                    name=f"g{role}{kb}_{c}")
                    gmat[(role, kb, c)] = gt
                    pg = pj_tile()
                    for hh in range(2):
                        h = 2 * c + hh
                        nc.tensor.matmul(
                            pg[64 * hh:64 * hh + 64, 0:65],
                            ktok[kb][:, 64 * h:64 * h + 64],
                            vaug[(role, kb)][:, h * 65:(h + 1) * 65],
                            start=True, stop=True,
                            tile_position=(0, 64 * hh))
                    if ngc % 2 == 0:
                        nc.vector.tensor_copy(gt[:], pg[0:128, 0:65])
                    else:
                        nc.scalar.copy(gt[:], pg[0:128, 0:65])
                    ngc += 1

            # ---- phase C1: diagonal QK scores (masked) -----------------
            # per qi the two diagonal blocks; head pairs emitted adjacently
            # so their K=64 matmuls run in different PE row groups.
            diag = []                     # (qi, kb, mi)
            for qi in range(NQC):
                s = W + qi * CH
                diag.append((qi, s // 128, 0))
                diag.append((qi, s // 128 + 1, 1))

            sts = {}       # (c, kb) -> paired st tile [128, 512]
            nst = 0
            numT = [apool.tile([128, TPC], BF16, tag=f"numT{c}",
                               name=f"numT{c}") for c in range(KC)]
            zall = zpool.tile([1, 16 * CH], F32, tag="zall", name="zall",
                              bufs=1)
            npv = [0]

            def qk_group(c):
                """diag QK for head pair (2c, 2c+1); separate PSUM banks
                (concurrent row-group matmuls may not share a bank), one
                paired st tile with a single masked mul."""
                nonlocal nst
                for qi, kb, mi in diag:
                    pss = [pj_tile(), pj_tile()]
                    for hh in range(2):
                        po = 64 * hh
                        nc.tensor.matmul(
                            pss[hh][:, 0:CH],
                            kT[c][po:po + 64,
                                  128 * kb - W:128 * (kb + 1) - W],
                            qp[c][po:po + 64, qi * CH:(qi + 1) * CH],
                            start=True, stop=True)
                    sr = tpool.tile([128, 2 * CH], BF16, tag="st_raw",
                                    name="st_raw", bufs=3)
                    for hh in range(2):
                        if (nst + hh) % 2 == 0:
                            nc.scalar.copy(sr[:, hh * CH:(hh + 1) * CH],
                                           pss[hh][:, 0:CH])
                        else:
                            nc.vector.tensor_copy(
                                sr[:, hh * CH:(hh + 1) * CH],
                                pss[hh][:, 0:CH])
                    st = spool.tile([128, 2 * CH], BF16,
                                    tag=f"st{c}_{kb}", name=f"st{c}_{kb}")
                    nc.vector.tensor_mul(st[:], sr[:], dm2_s(c, mi))
                    nst += 1
                    sts[(c, kb)] = st

            def av_group(c):
                pavq = []
                for qi in range(NQC):
                    s = W + qi * CH
                    qsl = slice(qi * CH, (qi + 1) * CH)
                    pavs = []
                    for hh in range(2):
                        if npv[0] % 2 == 0:
                            pav = psAV.tile([128, 2 * CH], F32, tag="av",
                                            name="av")
                        else:
                            pav = pj_tile()
                        npv[0] += 1
                        pavs.append(pav)
                    pavq.append(pavs)
                    kbs_d = [s // 128, s // 128 + 1]
                    kbs_n = [(s - W) // 128 + j for j in range(nW)]
                    kbs_f = [(s + CH) // 128 + j for j in range(nW)]
                    nterm = 2 + 2 * nW
                    for hh in range(2):
                        h = 2 * c + hh
                        po = 64 * hh
                        csl = slice(hh * CH, (hh + 1) * CH)
                        it = 0
                        for kb in kbs_d:
                            nc.tensor.matmul(
                                pavs[hh][0:65, 0:CH],
                                vaug[("p", kb)][:, h * 65:(h + 1) * 65],
                                sts[(c, kb)][:, csl],
                                start=(it == 0), stop=(it == nterm - 1))
                            it += 1
                        for kb in kbs_n:
                            nc.tensor.matmul(
                                pavs[hh][0:65, 0:CH],
                                gmat[("n", kb, c)][po:po + 64, :],
                                qp[c][po:po + 64, qsl],
                                start=(it == 0), stop=(it == nterm - 1))
                            it += 1
                        for kb in kbs_f:
                            nc.tensor.matmul(
                                pavs[hh][0:65, 0:CH],
                                gmat[("f", kb, c)][po:po + 64, :],
                                qf[c][po:po + 64, qsl],
                                start=(it == 0), stop=(it == nterm - 1))
                            it += 1
                        zi = 4 * c + 2 * qi + hh
                        nc.scalar.copy(zall[0:1, zi * CH:(zi + 1) * CH],
                                       pavs[hh][64:65, 0:CH])
                zlo = 4 * c * CH
                nc.vector.tensor_scalar_max(
                    zall[0:1, zlo:zlo + 4 * CH], zall[0:1, zlo:zlo + 4 * CH],
                    1e-6)
                nc.vector.reciprocal_approx_fast(
                    zall[0:1, zlo:zlo + 4 * CH], zall[0:1, zlo:zlo + 4 * CH])
                for qi in range(NQC):
                    qsl = slice(qi * CH, (qi + 1) * CH)
                    for hh in range(2):
                        po = 64 * hh
                        zo = (4 * c + 2 * qi + hh) * CH
                        zb = zpool.tile([64, CH], F32, tag="zbs", name="zbs")
                        nc.gpsimd.partition_broadcast(
                            zb[:], zall[0:1, zo:zo + CH])
                        nc.vector.tensor_mul(
                            numT[c][po:po + 64, qsl],
                            pavq[qi][hh][0:64, 0:CH],
                            zb[:])

            qk_group(0)
            for c in range(KC):
                if c + 1 < KC:
                    qk_group(c + 1)
                av_group(c)

            dump("d_numT", numT[0][:])

            # ---- phase D: O-proj + residual + LN2 ----------------------
            for q in range(NQB):
                ql = slice(128 * q, 128 * (q + 1))
                ps = pj_tile()
                for k in range(KC):
                    nc.tensor.matmul(ps[:], numT[k][:, ql], wo_s(k),
                                     start=(k == 0),
                                     stop=(nobias and k == KC - 1))
                if not nobias:
                    nc.tensor.matmul(ps[:], ones1b[0:1, 0:128],
                                     brow_t[0:1, D:2 * D],
                                     start=False, stop=True)
                xres = x_t((W + 128 * q) // 128)
                nc.vector.tensor_add(x2_t[q][:], ps[:], xres)

                xs2 = tpool.tile([128, D], BF16, tag="ln_xs2", name="ln_xs2",
                                 bufs=2)
                layer_norm(x2_t[q], xs2[:])
                pt = psT.tile([128, 512], BF16, tag="tp", name="tp")
                for c in range(KC):
                    nc.tensor.transpose(pt[:, 128 * c:128 * (c + 1)],
                                        xs2[:, 128 * c:128 * (c + 1)], id_t[:])
                    if ntp % 2 == 0:
                        nc.vector.tensor_copy(xs2T[c][:, ql],
                                              pt[:, 128 * c:128 * (c + 1)])
                    else:
                        nc.scalar.copy(xs2T[c][:, ql],
                                       pt[:, 128 * c:128 * (c + 1)])
                    ntp += 1

        dump("d_x2", x2_t[0][:])

        # ---- phase E: FFN ----------------------------------------------
        with ExitStack() as ffn:
            fpool = ffn.enter_context(tc.tile_pool(name="ffn", bufs=1))
            psF = ffn.enter_context(
                tc.tile_pool(name="psF", bufs=4, space="PSUM"))
            psF2 = ffn.enter_context(
                tc.tile_pool(name="psF2", bufs=3, space="PSUM"))

            h1gT = [fpool.tile([128, TPC], BF16, tag=f"h1gT{f}",
                               name=f"h1gT{f}") for f in range(FFC)]

            def ffn2(q):
                ql = slice(128 * q, 128 * (q + 1))
                ps2 = psF2.tile([128, D], F32, tag="f2", name="f2")
                for f in range(FFC):
                    nc.tensor.matmul(ps2[:],
                                     h1gT[f][:, ql],
                                     w2_t[:, f * D:(f + 1) * D],
                                     start=(f == 0),
                                     stop=(nobias and f == FFC - 1))
                if not nobias:
                    nc.tensor.matmul(ps2[:], ones1b[0:1, 0:128],
                                     brow_t[0:1, 2 * D:3 * D],
                                     start=False, stop=True)
                o = fpool.tile([128, D], F32, tag="fout", name="fout", bufs=2)
                nc.vector.tensor_add(o[:], ps2[:], x2_t[q][:])
                nc.sync.dma_start(out[ql, :], o[:])

            for qh in range(2):
                hsl = slice(CH * qh, CH * (qh + 1))
                for f in range(FFC):
                    ps = psF.tile([128, CH], F32, tag="f1", name="f1")
                    for k in range(KC):
                        nc.tensor.matmul(
                            ps[:],
                            w1_t[:, k * FF + 128 * f:k * FF + 128 * (f + 1)],
                            xs2T[k][:, hsl],
                            start=(k == 0), stop=(k == KC - 1))
                    nc.scalar.activation(h1gT[f][:, hsl], ps[:],
                                         AF.Gelu, bias=bf1_c(f))
                ffn2(2 * qh)
                ffn2(2 * qh + 1)
            dump("d_h1", h1gT[0][:])

    nc.compile()
    return nc


def _get_nc(W, uniform, nobias):
    key = (W, uniform, nobias)
    if key not in _BUILD_CACHE:
        _BUILD_CACHE[key] = _build(W, uniform, nobias)
    return _BUILD_CACHE[key]


# ---------------------------------------------------------------------------
# host side
# ---------------------------------------------------------------------------

def _bf16(a):
    return np.asarray(a, np.float32).astype(ml_dtypes.bfloat16)


def kernel(x, mask, wq, wk, wv, wo, bo, g1, b1, g2, b2, w1, bf1, w2, bf2,
           decay_logit):
    x = np.asarray(x, np.float32)
    mask = np.asarray(mask)
    g1 = np.asarray(g1, np.float64)
    b1 = np.asarray(b1, np.float64)
    g2 = np.asarray(g2, np.float64)
    b2 = np.asarray(b2, np.float64)

    d64 = 1.0 / (1.0 + np.exp(-np.asarray(decay_logit, np.float64)))
    d64 = np.clip(d64, 1e-8, None)
    dmax = float(d64.max())
    # Band width: the attention kernel truncates at |i-j| > ~W.  The omitted
    # mass relative to the normalizer z is ~ d^(W+1)/(1+d); keep it under
    # 1e-5, well below the bf16 matmul noise (~1e-3).
    if dmax >= 1.0 - 1e-12:
        W = 512  # decay ~1: widest supported band
    else:
        need = math.log(1e-5 * (1.0 + dmax)) / math.log(dmax)
        W = max(128, 128 * math.ceil(need / 128))
        W = min(W, 512)
    nW = W // 128
    L = TPC + 2 * W
    NT = L // 128
    uniform = bool(np.all(d64 == d64[0]))
    nobias = bool(np.all(b1 == 0) and np.all(np.asarray(bo) == 0)
                  and np.all(np.asarray(bf2) == 0))
    NHM = 1 if uniform else H
    NQP = 1 if uniform else KC

    nc = _get_nc(W, uniform, nobias)

    # folded weights (bf16)
    wq_f = _bf16(g1[:, None] * np.asarray(wq, np.float64))
    wk_f = _bf16(g1[:, None] * np.asarray(wk, np.float64))
    wv_f = _bf16(g1[:, None] * np.asarray(wv, np.float64))
    wo_f = _bf16(wo)
    w1_f = _bf16(g2[:, None] * np.asarray(w1, np.float64))
    w2_f = _bf16(w2)
    bq_h = _bf16(b1 @ np.asarray(wq, np.float64))
    bk_h = _bf16(b1 @ np.asarray(wk, np.float64))
    bv_h = _bf16(b1 @ np.asarray(wv, np.float64))
    bo_h = _bf16(bo)
    bf1_h = (b2 @ np.asarray(w1, np.float64) +
             np.asarray(bf1, np.float64)).astype(np.float32)
    bf2_h = _bf16(bf2)

    def pack_kblocks(w, blk, nb):
        # [nb*128, cols] -> [128, nb*cols] with block k at cols [k*cols, ...)
        return np.ascontiguousarray(
            w.reshape(nb, 128, blk).transpose(1, 0, 2).reshape(128, nb * blk))

    # (dmask appended to wa below, bf16)
    wb_h = np.concatenate([
        pack_kblocks(wv_f, 512, 4),
        pack_kblocks(wo_f, 512, 4)], axis=1)
    w1_h = pack_kblocks(w1_f, FF, 4)
    w2_h = pack_kblocks(w2_f, D, 16)
    brow_h = np.concatenate(
        [bv_h, bo_h, bf2_h, bk_h, bq_h]).reshape(1, 5 * D)
    brow_h = brow_h.astype(ml_dtypes.bfloat16)

    # decay-derived tables
    ld = np.log(d64)                                    # (H,)
    di = np.arange(TPC, dtype=np.float64) % CH
    qposb = np.zeros((NQP, 128, TPC), np.float32)
    qnegb = np.zeros((NQP, 128, TPC), np.float32)
    for c in range(NQP):
        for hh in range(2):
            h = 2 * c + hh
            qposb[c, 64 * hh:64 * hh + 64, :] = np.exp(ld[h] * di)[None, :]
            qnegb[c, 64 * hh:64 * hh + 64, :] = np.exp(-ld[h] * di)[None, :]
    if uniform:
        qposb[:, 64:, :] = qposb[:, :64, :]
        qnegb[:, 64:, :] = qnegb[:, :64, :]
    # diagonal-zone mask: m[dk, di] = d^{|di-dk| - di}, dk in [0,256)
    dic = np.arange(CH, dtype=np.float64)
    dk = np.arange(CH, dtype=np.float64)
    dmask = np.zeros((NHM, 2, 128, CH), np.float32)
    for h in range(NHM):
        m = np.exp(ld[h] * (np.abs(dic[None, :] - dk[:, None]) - dic[None, :]))
        dmask[h, 0] = m[:128, :]
        dmask[h, 1] = m[128:, :]
    cb_h = np.concatenate([
        qposb.transpose(1, 0, 2).reshape(128, NQP * TPC),
        qnegb.transpose(1, 0, 2).reshape(128, NQP * TPC)], axis=1)
    cb_h = np.ascontiguousarray(cb_h, np.float32)
    NPR = 1 if uniform else KC
    dm2 = np.zeros((NPR, 2, 128, 2 * CH), np.float32)
    for p in range(NPR):
        for mi in range(2):
            dm2[p, mi, :, :CH] = dmask[(2 * p) % NHM, mi]
            dm2[p, mi, :, CH:] = dmask[(2 * p + 1) % NHM, mi]
    wa_h = np.concatenate([
        pack_kblocks(wk_f, 512, 4),
        pack_kblocks(wq_f, 512, 4),
        dm2.transpose(2, 0, 1, 3).reshape(128, NPR * 2 * 2 * CH)
           .astype(ml_dtypes.bfloat16)], axis=1)

    in_maps = []
    for core in range(NCORES):
        b = core // SH
        t0 = (core % SH) * TPC
        lo, hi = t0 - W, t0 + TPC + W
        xs = np.zeros((L, D), np.float32)
        pad = np.zeros((L,), np.float64)
        glo, ghi = max(lo, 0), min(hi, T)
        xs[glo - lo:ghi - lo] = x[b, glo:ghi]
        pad[glo - lo:ghi - lo] = (~mask[b, glo:ghi]).astype(np.float64)
        xbig = np.ascontiguousarray(
            xs.reshape(NT, 128, D).transpose(1, 0, 2).reshape(128, NT * D))

        vneg = np.zeros((H, L), np.float32)
        vpos = np.zeros((H, L), np.float32)
        vpln = np.zeros((H, L), np.float32)
        for h in range(H):
            vn = np.zeros(L)
            vp = np.zeros(L)
            for qi in range(NQC):
                s = W + qi * CH
                jj = np.arange(s - W, s)
                vn[jj] = np.exp(ld[h] * (s - jj))
                jj = np.arange(s + CH, s + CH + W)
                vp[jj] = np.exp(ld[h] * (CH + jj - (s + CH)))
            vneg[h] = (vn * pad).astype(np.float32)
            vpos[h] = (vp * pad).astype(np.float32)
            vpln[h] = pad.astype(np.float32)

        def _vlayout(a):
            return np.ascontiguousarray(
                a.reshape(H, L // 128, 128).transpose(2, 1, 0)
                 .reshape(128, (L // 128) * H))
        vneg, vpos, vpln = _vlayout(vneg), _vlayout(vpos), _vlayout(vpln)

        mu = xs.mean(axis=1)
        rr = 1.0 / np.sqrt(xs.var(axis=1) + 1e-5)
        mr = np.stack([mu, rr], axis=1).astype(np.float32)    # [L, 2]
        mr = mr.reshape(NT, 128, 2).transpose(1, 0, 2).reshape(128, 2 * NT)
        cs_h = np.concatenate([
            np.zeros((128, 8), np.float32),      # (bias cols retired)
            bf1_h.reshape(16, 128).T,
            vneg, vpos, vpln, mr], axis=1)
        cs_h = np.ascontiguousarray(cs_h, np.float32)

        nh = NT // 2
        im = {
            "xh0": np.ascontiguousarray(xbig[:, :nh * D]),
            "xh1": np.ascontiguousarray(xbig[:, nh * D:]),
            "ident": np.eye(128, dtype=ml_dtypes.bfloat16),
            "cs": cs_h, "cb": cb_h,
            "wa": wa_h, "wb": wb_h, "w1": w1_h, "w2": w2_h,
        }
        if not nobias:
            im["brow"] = brow_h
        in_maps.append(im)

    res = bass_utils.run_bass_kernel_spmd(nc, in_maps,
                                          core_ids=list(range(NCORES)))
    out = np.empty((B, T, D), np.float32)
    for core in range(NCORES):
        b = core // SH
        t0 = (core % SH) * TPC
        out[b, t0:t0 + TPC] = res.results[core]["out"]

    # Degenerate-mask patch: a query whose entire +-W neighbourhood is
    # masked has z ~ 0 on the device; the reference output is then
    # dominated by out-of-band keys that the banded kernel truncates.
    # Recompute those rows exactly on the host (never triggers for an
    # all-False mask).
    if mask.any():
        idx = np.arange(T)
        for b_ in range(B):
            keep = ~np.asarray(mask[b_])
            if keep.all():
                continue
            kpos = idx[keep]
            if len(kpos) == 0:
                dist = np.full(T, T)
            else:
                ins = np.searchsorted(kpos, idx)
                left = np.where(ins > 0, idx - kpos[np.clip(ins - 1, 0,
                                                            len(kpos) - 1)], T)
                right = np.where(ins < len(kpos),
                                 kpos[np.clip(ins, 0, len(kpos) - 1)] - idx, T)
                dist = np.minimum(left, right)
            need = dist > (W - 32)
            if need.any():
                out[b_, need] = _exact_rows(
                    np.asarray(x[b_], np.float64), keep.astype(np.float64),
                    np.where(need)[0], wq, wk, wv, wo, bo, g1, b1, g2, b2,
                    w1, bf1, w2, bf2, d64)
    return out


def _exact_rows(xb, pad, rows, wq, wk, wv, wo, bo, g1, b1, g2, b2,
                w1, bf1, w2, bf2, d64):
    """Reference math (float64) for the given query rows of one batch."""
    import scipy.special as _sp

    def ln(z, g, b):
        mu = z.mean(-1, keepdims=True)
        var = ((z - mu) ** 2).mean(-1, keepdims=True)
        return (z - mu) / np.sqrt(var + 1e-5) * g + b

    wq = np.asarray(wq, np.float64); wk = np.asarray(wk, np.float64)
    wv = np.asarray(wv, np.float64); wo = np.asarray(wo, np.float64)
    w1 = np.asarray(w1, np.float64); w2 = np.asarray(w2, np.float64)
    xn = ln(xb, np.asarray(g1, np.float64), np.asarray(b1, np.float64))

    def feat(z):
        return np.where(z > 0, z + 1.0, np.exp(np.minimum(z, 0.0)))

    Q = feat((xn[rows] @ wq).reshape(len(rows), H, HD))       # (R,H,hd)
    K = feat((xn @ wk).reshape(T, H, HD)) * pad[:, None, None]
    V = (xn @ wv).reshape(T, H, HD) * pad[:, None, None]
    dist = np.abs(rows[:, None] - np.arange(T)[None, :]).astype(np.float64)
    M = np.exp(dist[None] * np.log(d64)[:, None, None])       # (H,R,T)
    A = np.einsum('rhd,jhd->hrj', Q, K) * M
    z = np.clip(A.sum(-1, keepdims=True), 1e-6, None)
    o = np.einsum('hrj,jhd->rhd', A / z, V).reshape(len(rows), D)
    o = o @ wo + np.asarray(bo, np.float64)
    x2 = xb[rows] + o
    xn2 = ln(x2, np.asarray(g2, np.float64), np.asarray(b2, np.float64))
    h1 = xn2 @ w1 + np.asarray(bf1, np.float64)
    gl = 0.5 * h1 * (1.0 + _sp.erf(h1 / np.sqrt(2.0)))
    ffn = gl @ w2 + np.asarray(bf2, np.float64)
    return (x2 + ffn).astype(np.float32)


# revision 44
# speedup vs baseline: 1.1311x; 1.0142x over previous
"""Trainium2 Bass kernel for a bidirectional linear-attention transformer layer.

Contract: kernel(**inputs) takes the FULL unsharded inputs (as produced by the
problem's setup_inputs()) and returns the FULL (B, T, D) float32 output.

Sharding: token-parallel across 8 NeuronCores. Core c handles batch c//4,
query tokens [ (c%4)*512, (c%4)*512+512 ), with a +-W token halo (W=128 for
decay=0.9). The decay mask d^|i-j| factorizes per 256-token query chunk into
  - a Q-side scale d^{+di} (past/diag) or d^{-di} (future), baked into two
    pre-scaled copies of Q,
  - a K-side scale folded into V rows and into an extra "ones" column of V
    that simultaneously produces the normalization denominator z,
  - an elementwise 256x256 mask for the diagonal zone only.

For the off-diagonal (fully factorized) zones the attention is linearized:
    out_chunk += (K_blk^T V_blk')^T q_chunk        (G-trick)
so only the diagonal zone materializes scores.  All heavy matmuls run in
bf16 (fp32 PSUM accumulate); the residual path and statistics stay fp32.

Structure (per core):
  A  LN1 + PE transposes            -> xsT (bf16, [d, tok])
  B  K^T (diag cols), Qp/Qf, V+decay variants, K_tok, G matrices
  C1 diagonal QK scores             -> st tiles (bf16, masked)
  C2 AV chains (diag st + G terms)  -> numT (normalized, bf16)
  D  O-proj + residual + LN2        -> x2 (f32), xs2T (bf16)
  E  FFN1 + gelu + FFN2             -> out
Head pairs (2c, 2c+1) occupy PE row groups 0-63 / 64-127, so their K=64
matmuls execute concurrently when emitted adjacently.
"""

import math
import os
from contextlib import ExitStack

import numpy as np
import ml_dtypes

import concourse.bacc as bacc
import concourse.mybir as mybir
import concourse.tile as tile
from concourse import bass_utils
from concourse.alu_op_type import AluOpType

F32 = mybir.dt.float32
BF16 = mybir.dt.bfloat16
AF = mybir.ActivationFunctionType

B, T, D, H, FF = 2, 2048, 512, 8, 2048
HD = D // H          # 64
NCORES = 8
SH = 4               # token shards per batch
TPC = T // SH        # 512 query tokens per core
CH = 256             # query chunk width
NQC = TPC // CH      # 2 query chunks per core
KC = D // 128        # 4 contraction chunks over D
FFC = FF // 128      # 16 ff chunks

_BUILD_CACHE = {}


def _build(W, uniform, nobias):
    nW = W // 128            # past/future 128-blocks per q-chunk
    L = TPC + 2 * W          # local tokens per core
    NT = L // 128            # 128-token tiles per core
    NQB = TPC // 128         # 4 query 128-blocks
    NHM = 1 if uniform else H    # distinct decay tables needed
    NQP = 1 if uniform else KC

    CS_COLS = 24 + 3 * NT * H + 2 * NT        # fp32 pack 1
    O_VN = 24
    O_MR = 24 + 3 * NT * H
    O_QP = 0                                  # fp32 pack 2 offsets
    O_QN = NQP * TPC
    CB_COLS = 2 * NQP * TPC
    NPR = 1 if uniform else KC                # distinct head-pair masks
    O_DM = 2 * KC * 512                       # dmask lives in wa (bf16)
    WA_COLS = O_DM + NPR * 2 * 2 * CH

    nc = bacc.Bacc("TRN2", target_bir_lowering=False, debug=False)

    def dram_in(name, shape, dt=F32):
        return nc.dram_tensor(name, shape, dt, kind="ExternalInput").ap()

    # DMA issue order == declaration/use order below (HWDGE FIFO):
    xbig_a = dram_in("xh0", [128, (NT // 2) * D])     # x tokens, first half
    id_d = dram_in("ident", [128, 128], BF16)
    cs_d = dram_in("cs", [128, CS_COLS])              # small fp32 consts
    xbig_b = dram_in("xh1", [128, (NT - NT // 2) * D])
    wa_d = dram_in("wa", [128, WA_COLS], BF16)   # wk|wq|dmask
    cb_d = dram_in("cb", [128, CB_COLS], BF16)        # qpos|qneg
    wb_d = dram_in("wb", [128, 2 * KC * 512], BF16)   # wv|wo
    if not nobias:
        brow_d = dram_in("brow", [1, 5 * D], BF16)    # bv|bo|bf2|bk|bq rows
    w1_d = dram_in("w1", [128, KC * FF], BF16)
    w2_d = dram_in("w2", [128, FFC * D], BF16)

    out = nc.dram_tensor("out", [TPC, D], F32, kind="ExternalOutput").ap()

    dbg = os.environ.get("KDBG") == "1"
    dbg_d = {}
    if dbg:
        for nm, shape in [("d_xsT", [128, L]), ("d_kT", [128, TPC]),
                          ("d_qp", [128, TPC]), ("d_qf", [128, TPC]),
                          ("d_vaug", [128, H * (HD + 1)]),
                          ("d_ktok", [128, D]), ("d_numT", [128, TPC]),
                          ("d_x2", [128, D]), ("d_h1", [128, TPC])]:
            dbg_d[nm] = nc.dram_tensor(nm, shape, F32,
                                       kind="ExternalOutput").ap()

    with tile.TileContext(nc) as tc, ExitStack() as top:
        cpool = top.enter_context(tc.tile_pool(name="const", bufs=1))
        xpool = top.enter_context(tc.tile_pool(name="x", bufs=1))
        x2pool = top.enter_context(tc.tile_pool(name="x2", bufs=1))

        # ---- input DMAs (one per packed tensor) ------------------------
        cs_t = cpool.tile([128, CS_COLS], F32, tag="cs", name="cs")
        nc.sync.dma_start(cs_t[:], cs_d[:])
        xh0 = xpool.tile([128, (NT // 2) * D], F32, tag="xh0", name="xh0")
        nc.sync.dma_start(xh0[:], xbig_a[:])
        id_t = cpool.tile([128, 128], BF16, tag="ident", name="ident")
        nc.sync.dma_start(id_t[:], id_d[:])

        def x_t(i):
            if i < NT // 2:
                return xh0[:, i * D:(i + 1) * D]
            j = i - NT // 2
            return xh1[:, j * D:(j + 1) * D]

        xh1 = xpool.tile([128, (NT - NT // 2) * D], F32, tag="xh1", name="xh1")
        nc.sync.dma_start(xh1[:], xbig_b[:])
        wa_t = cpool.tile([128, WA_COLS], BF16, tag="wa", name="wa")
        nc.sync.dma_start(wa_t[:], wa_d[:])
        cb_t = cpool.tile([128, CB_COLS], BF16, tag="cb", name="cb")
        nc.sync.dma_start(cb_t[:], cb_d[:])
        wb_t = cpool.tile([128, 2 * KC * 512], BF16, tag="wb", name="wb")
        nc.sync.dma_start(wb_t[:], wb_d[:])
        if not nobias:
            brow_t = cpool.tile([1, 5 * D], BF16, tag="brow", name="brow")
            nc.sync.dma_start(brow_t[:], brow_d[:])
        w1_t = cpool.tile([128, KC * FF], BF16, tag="w1", name="w1")
        nc.sync.dma_start(w1_t[:], w1_d[:])
        w2_t = cpool.tile([128, FFC * D], BF16, tag="w2", name="w2")
        nc.sync.dma_start(w2_t[:], w2_d[:])

        def wk_s(k, c):
            return wa_t[:, k * 512 + 128 * c:k * 512 + 128 * (c + 1)]

        def wq_s(k, c):
            return wa_t[:, 2048 + k * 512 + 128 * c:2048 + k * 512 + 128 * (c + 1)]

        def wk_full(k):
            return wa_t[:, k * 512:(k + 1) * 512]

        def wv_s(k):
            return wb_t[:, k * 512:(k + 1) * 512]

        def wo_s(k):
            return wb_t[:, 2048 + k * 512:2048 + k * 512 + 512]

        bf1_c = lambda f: cs_t[:, 8 + f:9 + f]
        vsc = {"n": lambda i, h: cs_t[:, O_VN + i * H + h:O_VN + i * H + h + 1],
               "f": lambda i, h: cs_t[:, O_VN + NT * H + i * H + h:
                                      O_VN + NT * H + i * H + h + 1],
               "p": lambda i, h: cs_t[:, O_VN + 2 * NT * H + i * H + h:
                                      O_VN + 2 * NT * H + i * H + h + 1]}
        mu_c = lambda i: cs_t[:, O_MR + 2 * i:O_MR + 2 * i + 1]
        rr_c = lambda i: cs_t[:, O_MR + 2 * i + 1:O_MR + 2 * i + 2]
        qpos_s = lambda c: cb_t[:, O_QP + (c % NQP) * TPC:
                                O_QP + (c % NQP) * TPC + TPC]
        qneg_s = lambda c: cb_t[:, O_QN + (c % NQP) * TPC:
                                O_QN + (c % NQP) * TPC + TPC]
        # paired mask: [dm(2c) | dm(2c+1)] per mi, 512 wide
        dm2_s = lambda c, j: wa_t[:, O_DM + ((c % NPR) * 2 + j) * 2 * CH:
                                  O_DM + ((c % NPR) * 2 + j) * 2 * CH + 2 * CH]

        eps_t = cpool.tile([128, 1], F32, tag="eps", name="eps")
        nc.gpsimd.memset(eps_t[:], 1e-5)
        ones1b = cpool.tile([1, 512], BF16, tag="on1b", name="on1b")
        nc.gpsimd.memset(ones1b[:], 1.0)
        # warm the ACT LUT tables while input DMAs run
        warm_t = cpool.tile([1, 1], F32, tag="warm", name="warm")
        for fn in (AF.Copy, AF.Sqrt, AF.Exp, AF.Relu, AF.Gelu):
            nc.scalar.activation(warm_t[:], eps_t[0:1, :], fn)
        # PE warm-up: keep the PE busy (and the HAM un-throttled) during the
        # initial DMA wait so real matmuls run at 2.4 GHz from the start.
        wrm = cpool.tile([128, 512], BF16, tag="wrm", name="wrm")
        nc.gpsimd.memset(wrm[:], 0.0)
        # pre-load the Q7 PartitionBroadcast library during the DMA wait --
        # the first broadcast otherwise pays a ~7us LIBRARY_RELOAD mid-C2.
        pbw = cpool.tile([64, 8], F32, tag="pbw", name="pbw")
        nc.gpsimd.partition_broadcast(pbw[:], eps_t[0:1, 0:1].broadcast_to((1, 8)))

        def dump(nm, ap):
            if not dbg:
                return
            t = x2pool.tile(list(ap.shape), F32, tag="dbg_" + nm,
                            name="dbg_" + nm)
            nc.vector.tensor_copy(t[:], ap)
            nc.sync.dma_start(dbg_d[nm][:], t[:])

        x2_t = [x2pool.tile([128, D], F32, tag=f"x2_{q}", name=f"x2_{q}")
                for q in range(NQB)]
        xs2T = [x2pool.tile([128, TPC], BF16, tag=f"xs2T{c}", name=f"xs2T{c}")
                for c in range(KC)]

        with ExitStack() as attn:
            apool = attn.enter_context(tc.tile_pool(name="attn", bufs=1))
            spool = attn.enter_context(tc.tile_pool(name="st", bufs=1))
            tpool = attn.enter_context(tc.tile_pool(name="tmp", bufs=4))
            zpool = attn.enter_context(tc.tile_pool(name="ztmp", bufs=4))
            psS = attn.enter_context(
                tc.tile_pool(name="psS", bufs=2, space="PSUM"))
            psPJ = attn.enter_context(
                tc.tile_pool(name="psPJ", bufs=2, space="PSUM"))
            psAV = attn.enter_context(
                tc.tile_pool(name="psAV", bufs=2, space="PSUM"))
            psT = attn.enter_context(
                tc.tile_pool(name="psT", bufs=2, space="PSUM"))

            npj = [0]

            def pj_tile():
                pool = (psS, psPJ)[npj[0] % 2]
                npj[0] += 1
                return pool.tile([128, 512], F32,
                                 tag="s" if pool is psS else "pj",
                                 name="s" if pool is psS else "pj")

            for _ in range(8):
                wps = pj_tile()
                nc.tensor.matmul(wps[:, 0:512], wrm[:, 0:128], wrm[:, 0:512],
                                 start=True, stop=True)

            # ---- phase A: LN1 + transpose ------------------------------
            def layer_norm(src, xs_out):
                st6 = tpool.tile([128, 6], F32, tag="ln_st6", name="ln_st6")
                nc.vector.bn_stats(st6[:], src)
                mv = tpool.tile([128, 2], F32, tag="ln_mv", name="ln_mv")
                nc.vector.bn_aggr(mv[:], st6[:])
                sd = tpool.tile([128, 1], F32, tag="ln_sd", name="ln_sd")
                nc.scalar.activation(sd[:], mv[:, 1:2], AF.Sqrt, bias=eps_t[:])
                rr = tpool.tile([128, 1], F32, tag="ln_rr", name="ln_rr")
                nc.vector.reciprocal_approx_fast(rr[:], sd[:])
                nc.vector.tensor_scalar(xs_out, src, mv[:, 0:1], rr[:],
                                        AluOpType.subtract, AluOpType.mult)

            xsT = [apool.tile([128, L], BF16, tag=f"xsT{c}", name=f"xsT{c}")
                   for c in range(KC)]
            ntp = 0
            for i in range(NT):
                xs = tpool.tile([128, D], BF16, tag="ln_xs", name="ln_xs",
                                bufs=2)
                nc.vector.tensor_scalar(xs[:], x_t(i), mu_c(i), rr_c(i),
                                        AluOpType.subtract, AluOpType.mult)
                pt = psT.tile([128, 512], BF16, tag="tp", name="tp")
                for c in range(KC):
                    nc.tensor.transpose(pt[:, 128 * c:128 * (c + 1)],
                                        xs[:, 128 * c:128 * (c + 1)], id_t[:])
                    if ntp % 2 == 0:
                        nc.vector.tensor_copy(xsT[c][:, 128 * i:128 * (i + 1)],
                                              pt[:, 128 * c:128 * (c + 1)])
                    else:
                        nc.scalar.copy(xsT[c][:, 128 * i:128 * (i + 1)],
                                       pt[:, 128 * c:128 * (c + 1)])
                    ntp += 1

            dump("d_xsT", xsT[0][:])

            # ---- phase B: K^T(diag), Qp/Qf, V variants, K_tok, G -------
            nfg = [0]

            def feat_from_psum(ps, out_ap, span):
                """out = elu(ps) + 1 elementwise; out bf16 (no bias --
                biases are folded in as K=1 matmul rows upstream)."""
                m = tpool.tile([128, 512], BF16, tag="feat_m", name="feat_m",
                               bufs=2)
                nc.vector.tensor_scalar_min(m[:, :span], ps[:, :span], 0.0)
                r = tpool.tile([128, 512], BF16, tag="feat_r", name="feat_r",
                               bufs=2)
                if nfg[0] % 2 == 0:
                    nc.scalar.activation(r[:, :span], ps[:, :span], AF.Relu)
                else:
                    nc.vector.tensor_relu(r[:, :span], ps[:, :span])
                nfg[0] += 1
                e = tpool.tile([128, 512], BF16, tag="feat_e", name="feat_e",
                               bufs=2)
                nc.scalar.activation(e[:, :span], m[:, :span], AF.Exp)
                nc.vector.tensor_add(out_ap, e[:, :span], r[:, :span])

            # (kT is built later by transposing K_tok blocks)
            kT = [apool.tile([128, TPC], BF16, tag=f"kT{c}", name=f"kT{c}")
                  for c in range(KC)]

            qp = [apool.tile([128, TPC], BF16, tag=f"qp{c}", name=f"qp{c}")
                  for c in range(KC)]
            qf = [apool.tile([128, TPC], BF16, tag=f"qf{c}", name=f"qf{c}")
                  for c in range(KC)]
            for c in range(KC):
                ps = pj_tile()
                for k in range(KC):
                    nc.tensor.matmul(
                        ps[:], wq_s(k, c), xsT[k][:, W:W + TPC],
                        start=(k == 0), stop=(nobias and k == KC - 1))
                if not nobias:
                    nc.tensor.matmul(
                        ps[:], brow_t[0:1, 4 * D + 128 * c:4 * D + 128 * (c + 1)],
                        ones1b[0:1, 0:TPC], start=False, stop=True)
                ft = tpool.tile([128, TPC], BF16, tag="feat_q", name="feat_q",
                                bufs=2)
                feat_from_psum(ps, ft[:], TPC)
                nc.vector.tensor_mul(qp[c][:], ft[:], qpos_s(c))
                nc.vector.tensor_mul(qf[c][:], ft[:], qneg_s(c))

            dump("d_qp", qp[0][:])
            dump("d_qf", qf[0][:])

            # role maps for key 128-blocks
            pastkb, diagkb, futkb = set(), set(), set()
            for qi in range(NQC):
                s = W + qi * CH
                for j in range(nW):
                    pastkb.add((s - W + 128 * j) // 128)
                    futkb.add((s + CH + 128 * j) // 128)
                diagkb.add(s // 128)
                diagkb.add(s // 128 + 1)
            plainkb = sorted(pastkb | futkb)

            # K in token-major layout for every local block; the diagonal
            # blocks are additionally transposed into kT (feature-major).
            allkb = sorted(pastkb | futkb | diagkb)
            ktok = {}
            for i in allkb:
                ps = pj_tile()
                for k in range(KC):
                    nc.tensor.matmul(ps[:],
                                     xsT[k][:, 128 * i:128 * (i + 1)],
                                     wk_full(k),
                                     start=(k == 0),
                                     stop=(nobias and k == KC - 1))
                if not nobias:
                    nc.tensor.matmul(ps[:], ones1b[0:1, 0:128],
                                     brow_t[0:1, 3 * D:4 * D],
                                     start=False, stop=True)
                kt = apool.tile([128, D], BF16, tag=f"ktok{i}",
                                name=f"ktok{i}")
                feat_from_psum(ps, kt[:], D)
                ktok[i] = kt
                if i in diagkb:
                    pt = psT.tile([128, 512], BF16, tag="tp", name="tp")
                    for c in range(KC):
                        nc.tensor.transpose(pt[:, 128 * c:128 * (c + 1)],
                                            kt[:, 128 * c:128 * (c + 1)],
                                            id_t[:])
                        dst = kT[c][:, 128 * i - W:128 * (i + 1) - W]
                        if ntp % 2 == 0:
                            nc.vector.tensor_copy(dst, pt[:, 128 * c:128 * (c + 1)])
                        else:
                            nc.scalar.copy(dst, pt[:, 128 * c:128 * (c + 1)])
                        ntp += 1

            dump("d_kT", kT[0][:])


            # V psum -> decay-scaled variants (vaug) per (role, block)
            vaug = {}
            for i in range(NT):
                roles = []
                if i in pastkb:
                    roles.append("n")
                if i in diagkb:
                    roles.append("p")
                if i in futkb:
                    roles.append("f")
                if not roles:
                    continue
                ps = pj_tile()
                for k in range(KC):
                    nc.tensor.matmul(ps[:],
                                     xsT[k][:, 128 * i:128 * (i + 1)],
                                     wv_s(k),
                                     start=(k == 0),
                                     stop=(nobias and k == KC - 1))
                if not nobias:
                    nc.tensor.matmul(ps[:], ones1b[0:1, 0:128],
                                     brow_t[0:1, 0:D], start=False, stop=True)
                for nv, rname in enumerate(roles):
                    va = apool.tile([128, H * (HD + 1)], BF16,
                                    tag=f"vaug_{rname}{i}",
                                    name=f"vaug_{rname}{i}")
                    vaug[(rname, i)] = va
                    if uniform:
                        sc = vsc[rname](i, 0)
                        if (i + nv) % 2 == 0:
                            nc.scalar.activation(
                                va[:].rearrange("p (h d) -> p h d", h=H)[:, :, 0:64],
                                ps[:].rearrange("p (h d) -> p h d", h=H),
                                AF.Copy, scale=sc)
                        else:
                            nc.vector.tensor_scalar_mul(
                                va[:].rearrange("p (h d) -> p h d", h=H)[:, :, 0:64],
                                ps[:].rearrange("p (h d) -> p h d", h=H), sc)
                        nc.scalar.copy(
                            va[:].rearrange("p (h d) -> p h d", h=H)[:, :, 64:65],
                            sc.broadcast_to((128, H, 1)))
                    else:
                        for h in range(H):
                            sc = vsc[rname](i, h)
                            nc.scalar.activation(va[:, h * 65:h * 65 + 64],
                                                 ps[:, h * 64:(h + 1) * 64],
                                                 AF.Copy, scale=sc)
                            nc.scalar.copy(va[:, h * 65 + 64:h * 65 + 65], sc)

            if dbg:
                dump("d_vaug", vaug[("p", W // 128)][:])

            if dbg:
                dump("d_ktok", ktok[plainkb[0]][:])

            # G matrices: G[(role, kb, c)] [128, 65] bf16 -- head 2c in
            # partitions 0:64, head 2c+1 in 64:128.
            # G[f, m] = sum_key K_tok[key, f] * vaug[key, m]
            groles = []
            for i in sorted(pastkb):
                groles.append(("n", i))
            for i in sorted(futkb):
                groles.append(("f", i))
            gmat = {}
            ngc = 0
            for role, kb in groles:
                for c in range(KC):
                    gt = spool.tile([128, 65], BF16, tag=f"g{role}{kb}_{c}",
                                    name=f"g{role}{kb}_{c}")
                    gmat[(role, kb, c)] = gt
                    pg = pj_tile()
                    for hh in range(2):
                        h = 2 * c + hh
                        nc.tensor.matmul(
                            pg[64 * hh:64 * hh + 64, 0:65],
                            ktok[kb][:, 64 * h:64 * h + 64],
                            vaug[(role, kb)][:, h * 65:(h + 1) * 65],
                            start=True, stop=True,
                            tile_position=(0, 64 * hh))
                    if ngc % 2 == 0:
                        nc.vector.tensor_copy(gt[:], pg[0:128, 0:65])
                    else:
                        nc.scalar.copy(gt[:], pg[0:128, 0:65])
                    ngc += 1

            # ---- phase C1: diagonal QK scores (masked) -----------------
            # per qi the two diagonal blocks; head pairs emitted adjacently
            # so their K=64 matmuls run in different PE row groups.
            diag = []                     # (qi, kb, mi)
            for qi in range(NQC):
                s = W + qi * CH
                diag.append((qi, s // 128, 0))
                diag.append((qi, s // 128 + 1, 1))

            sts = {}       # (c, kb) -> paired st tile [128, 512]
            nst = 0
            numT = [apool.tile([128, TPC], BF16, tag=f"numT{c}",
                               name=f"numT{c}") for c in range(KC)]
            zall = zpool.tile([1, 16 * CH], F32, tag="zall", name="zall",
                              bufs=1)
            npv = [0]

            def qk_group(c):
                """diag QK for head pair (2c, 2c+1); separate PSUM banks
                (concurrent row-group matmuls may not share a bank), one
                paired st tile with a single masked mul."""
                nonlocal nst
                for qi, kb, mi in diag:
                    pss = [pj_tile(), pj_tile()]
                    for hh in range(2):
                        po = 64 * hh
                        nc.tensor.matmul(
                            pss[hh][:, 0:CH],
                            kT[c][po:po + 64,
                                  128 * kb - W:128 * (kb + 1) - W],
                            qp[c][po:po + 64, qi * CH:(qi + 1) * CH],
                            start=True, stop=True)
                    sr = tpool.tile([128, 2 * CH], BF16, tag="st_raw",
                                    name="st_raw", bufs=3)
                    for hh in range(2):
                        if (nst + hh) % 2 == 0:
                            nc.scalar.copy(sr[:, hh * CH:(hh + 1) * CH],
                                           pss[hh][:, 0:CH])
                        else:
                            nc.vector.tensor_copy(
                                sr[:, hh * CH:(hh + 1) * CH],
                                pss[hh][:, 0:CH])
                    st = spool.tile([128, 2 * CH], BF16,
                                    tag=f"st{c}_{kb}", name=f"st{c}_{kb}")
                    nc.vector.tensor_mul(st[:], sr[:], dm2_s(c, mi))
                    nst += 1
                    sts[(c, kb)] = st

            def av_group(c):
                pavq = []
                for qi in range(NQC):
                    s = W + qi * CH
                    qsl = slice(qi * CH, (qi + 1) * CH)
                    pavs = []
                    for hh in range(2):
                        if npv[0] % 2 == 0:
                            pav = psAV.tile([128, 2 * CH], F32, tag="av",
                                            name="av")
                        else:
                            pav = pj_tile()
                        npv[0] += 1
                        pavs.append(pav)
                    pavq.append(pavs)
                    kbs_d = [s // 128, s // 128 + 1]
                    kbs_n = [(s - W) // 128 + j for j in range(nW)]
                    kbs_f = [(s + CH) // 128 + j for j in range(nW)]
                    nterm = 2 + 2 * nW
                    for hh in range(2):
                        h = 2 * c + hh
                        po = 64 * hh
                        csl = slice(hh * CH, (hh + 1) * CH)
                        it = 0
                        for kb in kbs_d:
                            nc.tensor.matmul(
                                pavs[hh][0:65, 0:CH],
                                vaug[("p", kb)][:, h * 65:(h + 1) * 65],
                                sts[(c, kb)][:, csl],
                                start=(it == 0), stop=(it == nterm - 1))
                            it += 1
                        for kb in kbs_n:
                            nc.tensor.matmul(
                                pavs[hh][0:65, 0:CH],
                                gmat[("n", kb, c)][po:po + 64, :],
                                qp[c][po:po + 64, qsl],
                                start=(it == 0), stop=(it == nterm - 1))
                            it += 1
                        for kb in kbs_f:
                            nc.tensor.matmul(
                                pavs[hh][0:65, 0:CH],
                                gmat[("f", kb, c)][po:po + 64, :],
                                qf[c][po:po + 64, qsl],
                                start=(it == 0), stop=(it == nterm - 1))
                            it += 1
                        zi = 4 * c + 2 * qi + hh
                        nc.scalar.copy(zall[0:1, zi * CH:(zi + 1) * CH],
                                       pavs[hh][64:65, 0:CH])
                zlo = 4 * c * CH
                nc.vector.tensor_scalar_max(
                    zall[0:1, zlo:zlo + 4 * CH], zall[0:1, zlo:zlo + 4 * CH],
                    1e-6)
                nc.vector.reciprocal_approx_fast(
                    zall[0:1, zlo:zlo + 4 * CH], zall[0:1, zlo:zlo + 4 * CH])
                for qi in range(NQC):
                    qsl = slice(qi * CH, (qi + 1) * CH)
                    for hh in range(2):
                        po = 64 * hh
                        zo = (4 * c + 2 * qi + hh) * CH
                        zb = zpool.tile([64, CH], F32, tag="zbs", name="zbs")
                        nc.gpsimd.partition_broadcast(
                            zb[:], zall[0:1, zo:zo + CH])
                        nc.vector.tensor_mul(
                            numT[c][po:po + 64, qsl],
                            pavq[qi][hh][0:64, 0:CH],
                            zb[:])

            qk_group(0)
            for c in range(KC):
                if c + 1 < KC:
                    qk_group(c + 1)
                av_group(c)

            dump("d_numT", numT[0][:])

            # ---- phase D: O-proj + residual + LN2 ----------------------
            # all four O-projections stream first; LN2 + transposes follow
            # so the PE is not gated by each block's LN chain.
            for q in range(NQB):
                ql = slice(128 * q, 128 * (q + 1))
                ps = pj_tile()
                for k in range(KC):
                    nc.tensor.matmul(ps[:], numT[k][:, ql], wo_s(k),
                                     start=(k == 0),
                                     stop=(nobias and k == KC - 1))
                if not nobias:
                    nc.tensor.matmul(ps[:], ones1b[0:1, 0:128],
                                     brow_t[0:1, D:2 * D],
                                     start=False, stop=True)
                xres = x_t((W + 128 * q) // 128)
                nc.vector.tensor_add(x2_t[q][:], ps[:], xres)

            for q in range(NQB):
                ql = slice(128 * q, 128 * (q + 1))
                xs2 = tpool.tile([128, D], BF16, tag="ln_xs2", name="ln_xs2",
                                 bufs=2)
                layer_norm(x2_t[q], xs2[:])
                pt = psT.tile([128, 512], BF16, tag="tp", name="tp")
                for c in range(KC):
                    nc.tensor.transpose(pt[:, 128 * c:128 * (c + 1)],
                                        xs2[:, 128 * c:128 * (c + 1)], id_t[:])
                    if ntp % 2 == 0:
                        nc.vector.tensor_copy(xs2T[c][:, ql],
                                              pt[:, 128 * c:128 * (c + 1)])
                    else:
                        nc.scalar.copy(xs2T[c][:, ql],
                                       pt[:, 128 * c:128 * (c + 1)])
                    ntp += 1

        dump("d_x2", x2_t[0][:])

        # ---- phase E: FFN ----------------------------------------------
        with ExitStack() as ffn:
            fpool = ffn.enter_context(tc.tile_pool(name="ffn", bufs=1))
            psF = ffn.enter_context(
                tc.tile_pool(name="psF", bufs=4, space="PSUM"))
            psF2 = ffn.enter_context(
                tc.tile_pool(name="psF2", bufs=3, space="PSUM"))

            h1gT = [fpool.tile([128, TPC], BF16, tag=f"h1gT{f}",
                               name=f"h1gT{f}") for f in range(FFC)]

            def ffn2(q):
                ql = slice(128 * q, 128 * (q + 1))
                ps2 = psF2.tile([128, D], F32, tag="f2", name="f2")
                for f in range(FFC):
                    nc.tensor.matmul(ps2[:],
                                     h1gT[f][:, ql],
                                     w2_t[:, f * D:(f + 1) * D],
                                     start=(f == 0),
                                     stop=(nobias and f == FFC - 1))
                if not nobias:
                    nc.tensor.matmul(ps2[:], ones1b[0:1, 0:128],
                                     brow_t[0:1, 2 * D:3 * D],
                                     start=False, stop=True)
                o = fpool.tile([128, D], F32, tag="fout", name="fout", bufs=2)
                nc.vector.tensor_add(o[:], ps2[:], x2_t[q][:])
                nc.sync.dma_start(out[ql, :], o[:])

            for qh in range(2):
                hsl = slice(CH * qh, CH * (qh + 1))
                for f in range(FFC):
                    ps = psF.tile([128, CH], F32, tag="f1", name="f1")
                    for k in range(KC):
                        nc.tensor.matmul(
                            ps[:],
                            w1_t[:, k * FF + 128 * f:k * FF + 128 * (f + 1)],
                            xs2T[k][:, hsl],
                            start=(k == 0), stop=(k == KC - 1))
                    nc.scalar.activation(h1gT[f][:, hsl], ps[:],
                                         AF.Gelu, bias=bf1_c(f))
                ffn2(2 * qh)
                ffn2(2 * qh + 1)
            dump("d_h1", h1gT[0][:])

    nc.compile()
    return nc


def _get_nc(W, uniform, nobias):
    key = (W, uniform, nobias)
    if key not in _BUILD_CACHE:
        _BUILD_CACHE[key] = _build(W, uniform, nobias)
    return _BUILD_CACHE[key]


# ---------------------------------------------------------------------------
# host side
# ---------------------------------------------------------------------------

def _bf16(a):
    return np.asarray(a, np.float32).astype(ml_dtypes.bfloat16)


def kernel(x, mask, wq, wk, wv, wo, bo, g1, b1, g2, b2, w1, bf1, w2, bf2,
           decay_logit):
    x = np.asarray(x, np.float32)
    mask = np.asarray(mask)
    g1 = np.asarray(g1, np.float64)
    b1 = np.asarray(b1, np.float64)
    g2 = np.asarray(g2, np.float64)
    b2 = np.asarray(b2, np.float64)

    d64 = 1.0 / (1.0 + np.exp(-np.asarray(decay_logit, np.float64)))
    d64 = np.clip(d64, 1e-8, None)
    dmax = float(d64.max())
    # Band width: the attention kernel truncates at |i-j| > ~W.  The omitted
    # mass relative to the normalizer z is ~ d^(W+1)/(1+d); keep it under
    # 1e-5, well below the bf16 matmul noise (~1e-3).
    if dmax >= 1.0 - 1e-12:
        W = 512  # decay ~1: widest supported band
    else:
        need = math.log(1e-5 * (1.0 + dmax)) / math.log(dmax)
        W = max(128, 128 * math.ceil(need / 128))
        W = min(W, 512)
    nW = W // 128
    L = TPC + 2 * W
    NT = L // 128
    uniform = bool(np.all(d64 == d64[0]))
    nobias = bool(np.all(b1 == 0) and np.all(np.asarray(bo) == 0)
                  and np.all(np.asarray(bf2) == 0))
    NHM = 1 if uniform else H
    NQP = 1 if uniform else KC

    nc = _get_nc(W, uniform, nobias)

    # folded weights (bf16)
    wq_f = _bf16(g1[:, None] * np.asarray(wq, np.float64))
    wk_f = _bf16(g1[:, None] * np.asarray(wk, np.float64))
    wv_f = _bf16(g1[:, None] * np.asarray(wv, np.float64))
    wo_f = _bf16(wo)
    w1_f = _bf16(g2[:, None] * np.asarray(w1, np.float64))
    w2_f = _bf16(w2)
    bq_h = _bf16(b1 @ np.asarray(wq, np.float64))
    bk_h = _bf16(b1 @ np.asarray(wk, np.float64))
    bv_h = _bf16(b1 @ np.asarray(wv, np.float64))
    bo_h = _bf16(bo)
    bf1_h = (b2 @ np.asarray(w1, np.float64) +
             np.asarray(bf1, np.float64)).astype(np.float32)
    bf2_h = _bf16(bf2)

    def pack_kblocks(w, blk, nb):
        # [nb*128, cols] -> [128, nb*cols] with block k at cols [k*cols, ...)
        return np.ascontiguousarray(
            w.reshape(nb, 128, blk).transpose(1, 0, 2).reshape(128, nb * blk))

    # (dmask appended to wa below, bf16)
    wb_h = np.concatenate([
        pack_kblocks(wv_f, 512, 4),
        pack_kblocks(wo_f, 512, 4)], axis=1)
    w1_h = pack_kblocks(w1_f, FF, 4)
    w2_h = pack_kblocks(w2_f, D, 16)
    brow_h = np.concatenate(
        [bv_h, bo_h, bf2_h, bk_h, bq_h]).reshape(1, 5 * D)
    brow_h = brow_h.astype(ml_dtypes.bfloat16)

    # decay-derived tables
    ld = np.log(d64)                                    # (H,)
    di = np.arange(TPC, dtype=np.float64) % CH
    qposb = np.zeros((NQP, 128, TPC), np.float32)
    qnegb = np.zeros((NQP, 128, TPC), np.float32)
    for c in range(NQP):
        for hh in range(2):
            h = 2 * c + hh
            qposb[c, 64 * hh:64 * hh + 64, :] = np.exp(ld[h] * di)[None, :]
            qnegb[c, 64 * hh:64 * hh + 64, :] = np.exp(-ld[h] * di)[None, :]
    if uniform:
        qposb[:, 64:, :] = qposb[:, :64, :]
        qnegb[:, 64:, :] = qnegb[:, :64, :]
    # diagonal-zone mask: m[dk, di] = d^{|di-dk| - di}, dk in [0,256)
    dic = np.arange(CH, dtype=np.float64)
    dk = np.arange(CH, dtype=np.float64)
    dmask = np.zeros((NHM, 2, 128, CH), np.float32)
    for h in range(NHM):
        m = np.exp(ld[h] * (np.abs(dic[None, :] - dk[:, None]) - dic[None, :]))
        dmask[h, 0] = m[:128, :]
        dmask[h, 1] = m[128:, :]
    cb_h = np.concatenate([
        qposb.transpose(1, 0, 2).reshape(128, NQP * TPC),
        qnegb.transpose(1, 0, 2).reshape(128, NQP * TPC)], axis=1)
    cb_h = np.ascontiguousarray(cb_h).astype(ml_dtypes.bfloat16)
    NPR = 1 if uniform else KC
    dm2 = np.zeros((NPR, 2, 128, 2 * CH), np.float32)
    for p in range(NPR):
        for mi in range(2):
            dm2[p, mi, :, :CH] = dmask[(2 * p) % NHM, mi]
            dm2[p, mi, :, CH:] = dmask[(2 * p + 1) % NHM, mi]
    wa_h = np.concatenate([
        pack_kblocks(wk_f, 512, 4),
        pack_kblocks(wq_f, 512, 4),
        dm2.transpose(2, 0, 1, 3).reshape(128, NPR * 2 * 2 * CH)
           .astype(ml_dtypes.bfloat16)], axis=1)

    in_maps = []
    for core in range(NCORES):
        b = core // SH
        t0 = (core % SH) * TPC
        lo, hi = t0 - W, t0 + TPC + W
        xs = np.zeros((L, D), np.float32)
        pad = np.zeros((L,), np.float64)
        glo, ghi = max(lo, 0), min(hi, T)
        xs[glo - lo:ghi - lo] = x[b, glo:ghi]
        pad[glo - lo:ghi - lo] = (~mask[b, glo:ghi]).astype(np.float64)
        xbig = np.ascontiguousarray(
            xs.reshape(NT, 128, D).transpose(1, 0, 2).reshape(128, NT * D))

        vneg = np.zeros((H, L), np.float32)
        vpos = np.zeros((H, L), np.float32)
        vpln = np.zeros((H, L), np.float32)
        for h in range(H):
            vn = np.zeros(L)
            vp = np.zeros(L)
            for qi in range(NQC):
                s = W + qi * CH
                jj = np.arange(s - W, s)
                vn[jj] = np.exp(ld[h] * (s - jj))
                jj = np.arange(s + CH, s + CH + W)
                vp[jj] = np.exp(ld[h] * (CH + jj - (s + CH)))
            vneg[h] = (vn * pad).astype(np.float32)
            vpos[h] = (vp * pad).astype(np.float32)
            vpln[h] = pad.astype(np.float32)

        def _vlayout(a):
            return np.ascontiguousarray(
                a.reshape(H, L // 128, 128).transpose(2, 1, 0)
                 .reshape(128, (L // 128) * H))
        vneg, vpos, vpln = _vlayout(vneg), _vlayout(vpos), _vlayout(vpln)

        mu = xs.mean(axis=1)
        rr = 1.0 / np.sqrt(xs.var(axis=1) + 1e-5)
        mr = np.stack([mu, rr], axis=1).astype(np.float32)    # [L, 2]
        mr = mr.reshape(NT, 128, 2).transpose(1, 0, 2).reshape(128, 2 * NT)
        cs_h = np.concatenate([
            np.zeros((128, 8), np.float32),      # (bias cols retired)
            bf1_h.reshape(16, 128).T,
            vneg, vpos, vpln, mr], axis=1)
        cs_h = np.ascontiguousarray(cs_h, np.float32)

        nh = NT // 2
        im = {
            "xh0": np.ascontiguousarray(xbig[:, :nh * D]),
            "xh1": np.ascontiguousarray(xbig[:, nh * D:]),
            "ident": np.eye(128, dtype=ml_dtypes.bfloat16),
            "cs": cs_h, "cb": cb_h,
            "wa": wa_h, "wb": wb_h, "w1": w1_h, "w2": w2_h,
        }
        if not nobias:
            im["brow"] = brow_h
        in_maps.append(im)

    res = bass_utils.run_bass_kernel_spmd(nc, in_maps,
                                          core_ids=list(range(NCORES)))
    out = np.empty((B, T, D), np.float32)
    for core in range(NCORES):
        b = core // SH
        t0 = (core % SH) * TPC
        out[b, t0:t0 + TPC] = res.results[core]["out"]

    # Degenerate-mask patch: a query whose entire +-W neighbourhood is
    # masked has z ~ 0 on the device; the reference output is then
    # dominated by out-of-band keys that the banded kernel truncates.
    # Recompute those rows exactly on the host (never triggers for an
    # all-False mask).
    if mask.any():
        idx = np.arange(T)
        for b_ in range(B):
            keep = ~np.asarray(mask[b_])
            if keep.all():
                continue
            kpos = idx[keep]
            if len(kpos) == 0:
                dist = np.full(T, T)
            else:
                ins = np.searchsorted(kpos, idx)
                left = np.where(ins > 0, idx - kpos[np.clip(ins - 1, 0,
                                                            len(kpos) - 1)], T)
                right = np.where(ins < len(kpos),
                                 kpos[np.clip(ins, 0, len(kpos) - 1)] - idx, T)
                dist = np.minimum(left, right)
            need = dist > (W - 32)
            if need.any():
                out[b_, need] = _exact_rows(
                    np.asarray(x[b_], np.float64), keep.astype(np.float64),
                    np.where(need)[0], wq, wk, wv, wo, bo, g1, b1, g2, b2,
                    w1, bf1, w2, bf2, d64)
    return out


def _exact_rows(xb, pad, rows, wq, wk, wv, wo, bo, g1, b1, g2, b2,
                w1, bf1, w2, bf2, d64):
    """Reference math (float64) for the given query rows of one batch."""
    import scipy.special as _sp

    def ln(z, g, b):
        mu = z.mean(-1, keepdims=True)
        var = ((z - mu) ** 2).mean(-1, keepdims=True)
        return (z - mu) / np.sqrt(var + 1e-5) * g + b

    wq = np.asarray(wq, np.float64); wk = np.asarray(wk, np.float64)
    wv = np.asarray(wv, np.float64); wo = np.asarray(wo, np.float64)
    w1 = np.asarray(w1, np.float64); w2 = np.asarray(w2, np.float64)
    xn = ln(xb, np.asarray(g1, np.float64), np.asarray(b1, np.float64))

    def feat(z):
        return np.where(z > 0, z + 1.0, np.exp(np.minimum(z, 0.0)))

    Q = feat((xn[rows] @ wq).reshape(len(rows), H, HD))       # (R,H,hd)
    K = feat((xn @ wk).reshape(T, H, HD)) * pad[:, None, None]
    V = (xn @ wv).reshape(T, H, HD) * pad[:, None, None]
    dist = np.abs(rows[:, None] - np.arange(T)[None, :]).astype(np.float64)
    M = np.exp(dist[None] * np.log(d64)[:, None, None])       # (H,R,T)
    A = np.einsum('rhd,jhd->hrj', Q, K) * M
    z = np.clip(A.sum(-1, keepdims=True), 1e-6, None)
    o = np.einsum('hrj,jhd->rhd', A / z, V).reshape(len(rows), D)
    o = o @ wo + np.asarray(bo, np.float64)
    x2 = xb[rows] + o
    xn2 = ln(x2, np.asarray(g2, np.float64), np.asarray(b2, np.float64))
    h1 = xn2 @ w1 + np.asarray(bf1, np.float64)
    gl = 0.5 * h1 * (1.0 + _sp.erf(h1 / np.sqrt(2.0)))
    ffn = gl @ w2 + np.asarray(bf2, np.float64)
    return (x2 + ffn).astype(np.float32)


# revision 45
# speedup vs baseline: 1.1582x; 1.0239x over previous
"""Trainium2 Bass kernel for a bidirectional linear-attention transformer layer.

Contract: kernel(**inputs) takes the FULL unsharded inputs (as produced by the
problem's setup_inputs()) and returns the FULL (B, T, D) float32 output.

Sharding: token-parallel across 8 NeuronCores. Core c handles batch c//4,
query tokens [ (c%4)*512, (c%4)*512+512 ), with a +-W token halo (W=128 for
decay=0.9). The decay mask d^|i-j| factorizes per 256-token query chunk into
  - a Q-side scale d^{+di} (past/diag) or d^{-di} (future), baked into two
    pre-scaled copies of Q,
  - a K-side scale folded into V rows and into an extra "ones" column of V
    that simultaneously produces the normalization denominator z,
  - an elementwise 256x256 mask for the diagonal zone only.

For the off-diagonal (fully factorized) zones the attention is linearized:
    out_chunk += (K_blk^T V_blk')^T q_chunk        (G-trick)
so only the diagonal zone materializes scores.  All heavy matmuls run in
bf16 (fp32 PSUM accumulate); the residual path and statistics stay fp32.

Structure (per core):
  A  LN1 + PE transposes            -> xsT (bf16, [d, tok])
  B  K^T (diag cols), Qp/Qf, V+decay variants, K_tok, G matrices
  C1 diagonal QK scores             -> st tiles (bf16, masked)
  C2 AV chains (diag st + G terms)  -> numT (normalized, bf16)
  D  O-proj + residual + LN2        -> x2 (f32), xs2T (bf16)
  E  FFN1 + gelu + FFN2             -> out
Head pairs (2c, 2c+1) occupy PE row groups 0-63 / 64-127, so their K=64
matmuls execute concurrently when emitted adjacently.
"""

import math
import os
from contextlib import ExitStack

import numpy as np
import ml_dtypes

import concourse.bacc as bacc
import concourse.mybir as mybir
import concourse.tile as tile
from concourse import bass_utils
from concourse.alu_op_type import AluOpType

F32 = mybir.dt.float32
BF16 = mybir.dt.bfloat16
AF = mybir.ActivationFunctionType

B, T, D, H, FF = 2, 2048, 512, 8, 2048
HD = D // H          # 64
NCORES = 8
SH = 4               # token shards per batch
TPC = T // SH        # 512 query tokens per core
CH = 256             # query chunk width
NQC = TPC // CH      # 2 query chunks per core
KC = D // 128        # 4 contraction chunks over D
FFC = FF // 128      # 16 ff chunks

_BUILD_CACHE = {}


def _build(W, uniform, nobias):
    nW = W // 128            # past/future 128-blocks per q-chunk
    L = TPC + 2 * W          # local tokens per core
    NT = L // 128            # 128-token tiles per core
    NQB = TPC // 128         # 4 query 128-blocks
    NHM = 1 if uniform else H    # distinct decay tables needed
    NQP = 1 if uniform else KC

    CS_COLS = 24 + 3 * NT * H + 2 * NT        # fp32 pack 1
    O_VN = 24
    O_MR = 24 + 3 * NT * H
    O_QP = 0                                  # fp32 pack 2 offsets
    O_QN = NQP * TPC
    CB_COLS = 2 * NQP * TPC
    NPR = 1 if uniform else KC                # distinct head-pair masks
    O_DM = 2 * KC * 512                       # dmask lives in wa (bf16)
    WA_COLS = O_DM + NPR * 2 * 2 * CH

    nc = bacc.Bacc("TRN2", target_bir_lowering=False, debug=False)

    def dram_in(name, shape, dt=F32):
        return nc.dram_tensor(name, shape, dt, kind="ExternalInput").ap()

    # DMA issue order == declaration/use order below (HWDGE FIFO):
    xbig_a = dram_in("xh0", [128, (NT // 2) * D])     # x tokens, first half
    id_d = dram_in("ident", [128, 128], BF16)
    cs_d = dram_in("cs", [128, CS_COLS])              # small fp32 consts
    xbig_b = dram_in("xh1", [128, (NT - NT // 2) * D])
    wa_d = dram_in("wa", [128, WA_COLS], BF16)   # wk|wq|dmask
    cb_d = dram_in("cb", [128, CB_COLS], BF16)        # qpos|qneg
    wb_d = dram_in("wb", [128, 2 * KC * 512], BF16)   # wv|wo
    if not nobias:
        brow_d = dram_in("brow", [1, 5 * D], BF16)    # bv|bo|bf2|bk|bq rows
    w1_d = dram_in("w1", [128, KC * FF], BF16)
    w2_d = dram_in("w2", [128, FFC * D], BF16)

    out = nc.dram_tensor("out", [TPC, D], F32, kind="ExternalOutput").ap()

    dbg = os.environ.get("KDBG") == "1"
    dbg_d = {}
    if dbg:
        for nm, shape in [("d_xsT", [128, L]), ("d_kT", [128, TPC]),
                          ("d_qp", [128, TPC]), ("d_qf", [128, TPC]),
                          ("d_vaug", [128, H * (HD + 1)]),
                          ("d_ktok", [128, D]), ("d_numT", [128, TPC]),
                          ("d_x2", [128, D]), ("d_h1", [128, TPC])]:
            dbg_d[nm] = nc.dram_tensor(nm, shape, F32,
                                       kind="ExternalOutput").ap()

    with tile.TileContext(nc) as tc, ExitStack() as top:
        cpool = top.enter_context(tc.tile_pool(name="const", bufs=1))
        xpool = top.enter_context(tc.tile_pool(name="x", bufs=1))
        x2pool = top.enter_context(tc.tile_pool(name="x2", bufs=1))

        # ---- input DMAs (one per packed tensor) ------------------------
        cs_t = cpool.tile([128, CS_COLS], F32, tag="cs", name="cs")
        nc.sync.dma_start(cs_t[:], cs_d[:])
        xh0 = xpool.tile([128, (NT // 2) * D], F32, tag="xh0", name="xh0")
        nc.sync.dma_start(xh0[:], xbig_a[:])
        id_t = cpool.tile([128, 128], BF16, tag="ident", name="ident")
        nc.sync.dma_start(id_t[:], id_d[:])

        def x_t(i):
            if i < NT // 2:
                return xh0[:, i * D:(i + 1) * D]
            j = i - NT // 2
            return xh1[:, j * D:(j + 1) * D]

        xh1 = xpool.tile([128, (NT - NT // 2) * D], F32, tag="xh1", name="xh1")
        nc.sync.dma_start(xh1[:], xbig_b[:])
        wa_t = cpool.tile([128, WA_COLS], BF16, tag="wa", name="wa")
        nc.sync.dma_start(wa_t[:], wa_d[:])
        cb_t = cpool.tile([128, CB_COLS], BF16, tag="cb", name="cb")
        nc.sync.dma_start(cb_t[:], cb_d[:])
        wb_t = cpool.tile([128, 2 * KC * 512], BF16, tag="wb", name="wb")
        nc.sync.dma_start(wb_t[:], wb_d[:])
        if not nobias:
            brow_t = cpool.tile([1, 5 * D], BF16, tag="brow", name="brow")
            nc.sync.dma_start(brow_t[:], brow_d[:])
        w1_t = cpool.tile([128, KC * FF], BF16, tag="w1", name="w1")
        nc.sync.dma_start(w1_t[:], w1_d[:])
        w2_t = cpool.tile([128, FFC * D], BF16, tag="w2", name="w2")
        nc.sync.dma_start(w2_t[:], w2_d[:])

        def wk_s(k, c):
            return wa_t[:, k * 512 + 128 * c:k * 512 + 128 * (c + 1)]

        def wq_s(k, c):
            return wa_t[:, 2048 + k * 512 + 128 * c:2048 + k * 512 + 128 * (c + 1)]

        def wk_full(k):
            return wa_t[:, k * 512:(k + 1) * 512]

        def wv_s(k):
            return wb_t[:, k * 512:(k + 1) * 512]

        def wo_s(k):
            return wb_t[:, 2048 + k * 512:2048 + k * 512 + 512]

        bf1_c = lambda f: cs_t[:, 8 + f:9 + f]
        vsc = {"n": lambda i, h: cs_t[:, O_VN + i * H + h:O_VN + i * H + h + 1],
               "f": lambda i, h: cs_t[:, O_VN + NT * H + i * H + h:
                                      O_VN + NT * H + i * H + h + 1],
               "p": lambda i, h: cs_t[:, O_VN + 2 * NT * H + i * H + h:
                                      O_VN + 2 * NT * H + i * H + h + 1]}
        mu_c = lambda i: cs_t[:, O_MR + 2 * i:O_MR + 2 * i + 1]
        rr_c = lambda i: cs_t[:, O_MR + 2 * i + 1:O_MR + 2 * i + 2]
        qpos_s = lambda c: cb_t[:, O_QP + (c % NQP) * TPC:
                                O_QP + (c % NQP) * TPC + TPC]
        qneg_s = lambda c: cb_t[:, O_QN + (c % NQP) * TPC:
                                O_QN + (c % NQP) * TPC + TPC]
        # paired mask: [dm(2c) | dm(2c+1)] per mi, 512 wide
        dm2_s = lambda c, j: wa_t[:, O_DM + ((c % NPR) * 2 + j) * 2 * CH:
                                  O_DM + ((c % NPR) * 2 + j) * 2 * CH + 2 * CH]

        eps_t = cpool.tile([128, 1], F32, tag="eps", name="eps")
        nc.gpsimd.memset(eps_t[:], 1e-5)
        ones1b = cpool.tile([1, 512], BF16, tag="on1b", name="on1b")
        nc.gpsimd.memset(ones1b[:], 1.0)
        # warm the ACT LUT tables while input DMAs run
        warm_t = cpool.tile([1, 1], F32, tag="warm", name="warm")
        for fn in (AF.Copy, AF.Sqrt, AF.Exp, AF.Relu, AF.Gelu):
            nc.scalar.activation(warm_t[:], eps_t[0:1, :], fn)
        # PE warm-up: keep the PE busy (and the HAM un-throttled) during the
        # initial DMA wait so real matmuls run at 2.4 GHz from the start.
        wrm = cpool.tile([128, 512], BF16, tag="wrm", name="wrm")
        nc.gpsimd.memset(wrm[:], 0.0)
        # pre-load the Q7 PartitionBroadcast library during the DMA wait --
        # the first broadcast otherwise pays a ~7us LIBRARY_RELOAD mid-C2.
        pbw = cpool.tile([64, 8], F32, tag="pbw", name="pbw")
        nc.gpsimd.partition_broadcast(pbw[:], eps_t[0:1, 0:1].broadcast_to((1, 8)))

        def dump(nm, ap):
            if not dbg:
                return
            t = x2pool.tile(list(ap.shape), F32, tag="dbg_" + nm,
                            name="dbg_" + nm)
            nc.vector.tensor_copy(t[:], ap)
            nc.sync.dma_start(dbg_d[nm][:], t[:])

        x2_t = [x2pool.tile([128, D], F32, tag=f"x2_{q}", name=f"x2_{q}")
                for q in range(NQB)]
        xs2T = [x2pool.tile([128, TPC], BF16, tag=f"xs2T{c}", name=f"xs2T{c}")
                for c in range(KC)]

        with ExitStack() as attn:
            apool = attn.enter_context(tc.tile_pool(name="attn", bufs=1))
            spool = attn.enter_context(tc.tile_pool(name="st", bufs=1))
            tpool = attn.enter_context(tc.tile_pool(name="tmp", bufs=4))
            zpool = attn.enter_context(tc.tile_pool(name="ztmp", bufs=4))
            psS = attn.enter_context(
                tc.tile_pool(name="psS", bufs=2, space="PSUM"))
            psPJ = attn.enter_context(
                tc.tile_pool(name="psPJ", bufs=2, space="PSUM"))
            psAV = attn.enter_context(
                tc.tile_pool(name="psAV", bufs=2, space="PSUM"))
            psT = attn.enter_context(
                tc.tile_pool(name="psT", bufs=2, space="PSUM"))

            npj = [0]

            def pj_tile():
                pool = (psS, psPJ)[npj[0] % 2]
                npj[0] += 1
                return pool.tile([128, 512], F32,
                                 tag="s" if pool is psS else "pj",
                                 name="s" if pool is psS else "pj")

            for _ in range(8):
                wps = pj_tile()
                nc.tensor.matmul(wps[:, 0:512], wrm[:, 0:128], wrm[:, 0:512],
                                 start=True, stop=True)

            # ---- phase A: LN1 + transpose ------------------------------
            def layer_norm(src, xs_out):
                st6 = tpool.tile([128, 6], F32, tag="ln_st6", name="ln_st6")
                nc.vector.bn_stats(st6[:], src)
                mv = tpool.tile([128, 2], F32, tag="ln_mv", name="ln_mv")
                nc.vector.bn_aggr(mv[:], st6[:])
                sd = tpool.tile([128, 1], F32, tag="ln_sd", name="ln_sd")
                nc.scalar.activation(sd[:], mv[:, 1:2], AF.Sqrt, bias=eps_t[:])
                rr = tpool.tile([128, 1], F32, tag="ln_rr", name="ln_rr")
                nc.vector.reciprocal_approx_fast(rr[:], sd[:])
                nc.vector.tensor_scalar(xs_out, src, mv[:, 0:1], rr[:],
                                        AluOpType.subtract, AluOpType.mult)

            xsT = [apool.tile([128, L], BF16, tag=f"xsT{c}", name=f"xsT{c}")
                   for c in range(KC)]
            ntp = 0
            for i in range(NT):
                xs = tpool.tile([128, D], BF16, tag="ln_xs", name="ln_xs",
                                bufs=2)
                nc.vector.tensor_scalar(xs[:], x_t(i), mu_c(i), rr_c(i),
                                        AluOpType.subtract, AluOpType.mult)
                pt = psT.tile([128, 512], BF16, tag="tp", name="tp")
                for c in range(KC):
                    nc.tensor.transpose(pt[:, 128 * c:128 * (c + 1)],
                                        xs[:, 128 * c:128 * (c + 1)], id_t[:])
                    if ntp % 2 == 0:
                        nc.vector.tensor_copy(xsT[c][:, 128 * i:128 * (i + 1)],
                                              pt[:, 128 * c:128 * (c + 1)])
                    else:
                        nc.scalar.copy(xsT[c][:, 128 * i:128 * (i + 1)],
                                       pt[:, 128 * c:128 * (c + 1)])
                    ntp += 1

            dump("d_xsT", xsT[0][:])

            # ---- phase B: K^T(diag), Qp/Qf, V variants, K_tok, G -------
            nfg = [0]

            def feat_from_psum(ps, out_ap, span):
                """out = elu(ps) + 1 elementwise; out bf16 (no bias --
                biases are folded in as K=1 matmul rows upstream)."""
                m = tpool.tile([128, 512], BF16, tag="feat_m", name="feat_m",
                               bufs=2)
                nc.vector.tensor_scalar_min(m[:, :span], ps[:, :span], 0.0)
                r = tpool.tile([128, 512], BF16, tag="feat_r", name="feat_r",
                               bufs=2)
                if nfg[0] % 2 == 0:
                    nc.scalar.activation(r[:, :span], ps[:, :span], AF.Relu)
                else:
                    nc.vector.tensor_relu(r[:, :span], ps[:, :span])
                nfg[0] += 1
                e = tpool.tile([128, 512], BF16, tag="feat_e", name="feat_e",
                               bufs=2)
                nc.scalar.activation(e[:, :span], m[:, :span], AF.Exp)
                nc.vector.tensor_add(out_ap, e[:, :span], r[:, :span])

            # (kT is built later by transposing K_tok blocks)
            kT = [apool.tile([128, TPC], BF16, tag=f"kT{c}", name=f"kT{c}")
                  for c in range(KC)]

            qp = [apool.tile([128, TPC], BF16, tag=f"qp{c}", name=f"qp{c}")
                  for c in range(KC)]
            qf = [apool.tile([128, TPC], BF16, tag=f"qf{c}", name=f"qf{c}")
                  for c in range(KC)]
            for c in range(KC):
                ps = pj_tile()
                for k in range(KC):
                    nc.tensor.matmul(
                        ps[:], wq_s(k, c), xsT[k][:, W:W + TPC],
                        start=(k == 0), stop=(nobias and k == KC - 1))
                if not nobias:
                    nc.tensor.matmul(
                        ps[:], brow_t[0:1, 4 * D + 128 * c:4 * D + 128 * (c + 1)],
                        ones1b[0:1, 0:TPC], start=False, stop=True)
                ft = tpool.tile([128, TPC], BF16, tag="feat_q", name="feat_q",
                                bufs=2)
                feat_from_psum(ps, ft[:], TPC)
                nc.vector.tensor_mul(qp[c][:], ft[:], qpos_s(c))
                nc.vector.tensor_mul(qf[c][:], ft[:], qneg_s(c))

            dump("d_qp", qp[0][:])
            dump("d_qf", qf[0][:])

            # role maps for key 128-blocks
            pastkb, diagkb, futkb = set(), set(), set()
            for qi in range(NQC):
                s = W + qi * CH
                for j in range(nW):
                    pastkb.add((s - W + 128 * j) // 128)
                    futkb.add((s + CH + 128 * j) // 128)
                diagkb.add(s // 128)
                diagkb.add(s // 128 + 1)
            plainkb = sorted(pastkb | futkb)

            # K in token-major layout for every local block; the diagonal
            # blocks are additionally transposed into kT (feature-major).
            allkb = sorted(pastkb | futkb | diagkb)
            ktok = {}
            for i in allkb:
                ps = pj_tile()
                for k in range(KC):
                    nc.tensor.matmul(ps[:],
                                     xsT[k][:, 128 * i:128 * (i + 1)],
                                     wk_full(k),
                                     start=(k == 0),
                                     stop=(nobias and k == KC - 1))
                if not nobias:
                    nc.tensor.matmul(ps[:], ones1b[0:1, 0:128],
                                     brow_t[0:1, 3 * D:4 * D],
                                     start=False, stop=True)
                kt = apool.tile([128, D], BF16, tag=f"ktok{i}",
                                name=f"ktok{i}")
                feat_from_psum(ps, kt[:], D)
                ktok[i] = kt
                if i in diagkb:
                    pt = psT.tile([128, 512], BF16, tag="tp", name="tp")
                    for c in range(KC):
                        nc.tensor.transpose(pt[:, 128 * c:128 * (c + 1)],
                                            kt[:, 128 * c:128 * (c + 1)],
                                            id_t[:])
                        dst = kT[c][:, 128 * i - W:128 * (i + 1) - W]
                        if ntp % 2 == 0:
                            nc.vector.tensor_copy(dst, pt[:, 128 * c:128 * (c + 1)])
                        else:
                            nc.scalar.copy(dst, pt[:, 128 * c:128 * (c + 1)])
                        ntp += 1

            dump("d_kT", kT[0][:])


            # V psum -> decay-scaled variants (vaug) per (role, block)
            vaug = {}
            for i in range(NT):
                roles = []
                if i in pastkb:
                    roles.append("n")
                if i in diagkb:
                    roles.append("p")
                if i in futkb:
                    roles.append("f")
                if not roles:
                    continue
                ps = pj_tile()
                for k in range(KC):
                    nc.tensor.matmul(ps[:],
                                     xsT[k][:, 128 * i:128 * (i + 1)],
                                     wv_s(k),
                                     start=(k == 0),
                                     stop=(nobias and k == KC - 1))
                if not nobias:
                    nc.tensor.matmul(ps[:], ones1b[0:1, 0:128],
                                     brow_t[0:1, 0:D], start=False, stop=True)
                for nv, rname in enumerate(roles):
                    va = apool.tile([128, H * (HD + 1)], BF16,
                                    tag=f"vaug_{rname}{i}",
                                    name=f"vaug_{rname}{i}")
                    vaug[(rname, i)] = va
                    if uniform:
                        sc = vsc[rname](i, 0)
                        if (i + nv) % 2 == 0:
                            nc.scalar.activation(
                                va[:].rearrange("p (h d) -> p h d", h=H)[:, :, 0:64],
                                ps[:].rearrange("p (h d) -> p h d", h=H),
                                AF.Copy, scale=sc)
                        else:
                            nc.vector.tensor_scalar_mul(
                                va[:].rearrange("p (h d) -> p h d", h=H)[:, :, 0:64],
                                ps[:].rearrange("p (h d) -> p h d", h=H), sc)
                        nc.scalar.copy(
                            va[:].rearrange("p (h d) -> p h d", h=H)[:, :, 64:65],
                            sc.broadcast_to((128, H, 1)))
                    else:
                        for h in range(H):
                            sc = vsc[rname](i, h)
                            nc.scalar.activation(va[:, h * 65:h * 65 + 64],
                                                 ps[:, h * 64:(h + 1) * 64],
                                                 AF.Copy, scale=sc)
                            nc.scalar.copy(va[:, h * 65 + 64:h * 65 + 65], sc)

            if dbg:
                dump("d_vaug", vaug[("p", W // 128)][:])

            if dbg:
                dump("d_ktok", ktok[plainkb[0]][:])

            # G matrices: G[(role, kb, c)] [128, 65] bf16 -- head 2c in
            # partitions 0:64, head 2c+1 in 64:128.
            # G[f, m] = sum_key K_tok[key, f] * vaug[key, m]
            groles = []
            for i in sorted(pastkb):
                groles.append(("n", i))
            for i in sorted(futkb):
                groles.append(("f", i))
            gmat = {}
            ngc = 0
            for role, kb in groles:
                for c in range(KC):
                    gt = spool.tile([128, 65], BF16, tag=f"g{role}{kb}_{c}",
                                    name=f"g{role}{kb}_{c}")
                    gmat[(role, kb, c)] = gt
                    pg = pj_tile()
                    for hh in range(2):
                        h = 2 * c + hh
                        nc.tensor.matmul(
                            pg[64 * hh:64 * hh + 64, 0:65],
                            ktok[kb][:, 64 * h:64 * h + 64],
                            vaug[(role, kb)][:, h * 65:(h + 1) * 65],
                            start=True, stop=True,
                            tile_position=(0, 64 * hh))
                    if ngc % 2 == 0:
                        nc.vector.tensor_copy(gt[:], pg[0:128, 0:65])
                    else:
                        nc.scalar.copy(gt[:], pg[0:128, 0:65])
                    ngc += 1

            # ---- phase C1: diagonal QK scores (masked) -----------------
            # per qi the two diagonal blocks; head pairs emitted adjacently
            # so their K=64 matmuls run in different PE row groups.
            diag = []                     # (qi, kb, mi)
            for qi in range(NQC):
                s = W + qi * CH
                diag.append((qi, s // 128, 0))
                diag.append((qi, s // 128 + 1, 1))

            sts = {}       # (c, kb) -> paired st tile [128, 512]
            nst = 0
            numT = [apool.tile([128, TPC], BF16, tag=f"numT{c}",
                               name=f"numT{c}") for c in range(KC)]
            zall = zpool.tile([1, 16 * CH], F32, tag="zall", name="zall",
                              bufs=1)
            npv = [0]

            def qk_group(c):
                """diag QK for head pair (2c, 2c+1); separate PSUM banks
                (concurrent row-group matmuls may not share a bank), one
                paired st tile with a single masked mul."""
                nonlocal nst
                for qi, kb, mi in diag:
                    pss = [pj_tile(), pj_tile()]
                    for hh in range(2):
                        po = 64 * hh
                        nc.tensor.matmul(
                            pss[hh][:, 0:CH],
                            kT[c][po:po + 64,
                                  128 * kb - W:128 * (kb + 1) - W],
                            qp[c][po:po + 64, qi * CH:(qi + 1) * CH],
                            start=True, stop=True)
                    sr = tpool.tile([128, 2 * CH], BF16, tag="st_raw",
                                    name="st_raw", bufs=3)
                    for hh in range(2):
                        if (nst + hh) % 2 == 0:
                            nc.scalar.copy(sr[:, hh * CH:(hh + 1) * CH],
                                           pss[hh][:, 0:CH])
                        else:
                            nc.vector.tensor_copy(
                                sr[:, hh * CH:(hh + 1) * CH],
                                pss[hh][:, 0:CH])
                    st = spool.tile([128, 2 * CH], BF16,
                                    tag=f"st{c}_{kb}", name=f"st{c}_{kb}")
                    nc.vector.tensor_mul(st[:], sr[:], dm2_s(c, mi))
                    nst += 1
                    sts[(c, kb)] = st

            def av_group(c):
                for qi in range(NQC):
                    s = W + qi * CH
                    qsl = slice(qi * CH, (qi + 1) * CH)
                    pavs = []
                    for hh in range(2):
                        if npv[0] % 2 == 0:
                            pav = psAV.tile([128, 2 * CH], F32, tag="av",
                                            name="av")
                        else:
                            pav = pj_tile()
                        npv[0] += 1
                        pavs.append(pav)
                    kbs_d = [s // 128, s // 128 + 1]
                    kbs_n = [(s - W) // 128 + j for j in range(nW)]
                    kbs_f = [(s + CH) // 128 + j for j in range(nW)]
                    nterm = 2 + 2 * nW
                    for hh in range(2):
                        h = 2 * c + hh
                        po = 64 * hh
                        csl = slice(hh * CH, (hh + 1) * CH)
                        it = 0
                        for kb in kbs_d:
                            nc.tensor.matmul(
                                pavs[hh][0:65, 0:CH],
                                vaug[("p", kb)][:, h * 65:(h + 1) * 65],
                                sts[(c, kb)][:, csl],
                                start=(it == 0), stop=(it == nterm - 1))
                            it += 1
                        for kb in kbs_n:
                            nc.tensor.matmul(
                                pavs[hh][0:65, 0:CH],
                                gmat[("n", kb, c)][po:po + 64, :],
                                qp[c][po:po + 64, qsl],
                                start=(it == 0), stop=(it == nterm - 1))
                            it += 1
                        for kb in kbs_f:
                            nc.tensor.matmul(
                                pavs[hh][0:65, 0:CH],
                                gmat[("f", kb, c)][po:po + 64, :],
                                qf[c][po:po + 64, qsl],
                                start=(it == 0), stop=(it == nterm - 1))
                            it += 1
                        zi = 4 * c + 2 * qi + hh
                        nc.scalar.copy(zall[0:1, zi * CH:(zi + 1) * CH],
                                       pavs[hh][64:65, 0:CH])
                    zlo = (4 * c + 2 * qi) * CH
                    nc.vector.tensor_scalar_max(
                        zall[0:1, zlo:zlo + 2 * CH],
                        zall[0:1, zlo:zlo + 2 * CH], 1e-6)
                    nc.vector.reciprocal_approx_fast(
                        zall[0:1, zlo:zlo + 2 * CH],
                        zall[0:1, zlo:zlo + 2 * CH])
                    for hh in range(2):
                        po = 64 * hh
                        zo = (4 * c + 2 * qi + hh) * CH
                        zb = zpool.tile([64, CH], F32, tag="zbs", name="zbs")
                        nc.gpsimd.partition_broadcast(
                            zb[:], zall[0:1, zo:zo + CH])
                        nc.vector.tensor_mul(
                            numT[c][po:po + 64, qsl],
                            pavs[hh][0:64, 0:CH],
                            zb[:])

            qk_group(0)
            for c in range(KC):
                if c + 1 < KC:
                    qk_group(c + 1)
                av_group(c)

            dump("d_numT", numT[0][:])

            # ---- phase D: O-proj + residual + LN2 ----------------------
            # all four O-projections stream first; LN2 + transposes follow
            # so the PE is not gated by each block's LN chain.
            for q in range(NQB):
                ql = slice(128 * q, 128 * (q + 1))
                ps = pj_tile()
                for k in range(KC):
                    nc.tensor.matmul(ps[:], numT[k][:, ql], wo_s(k),
                                     start=(k == 0),
                                     stop=(nobias and k == KC - 1))
                if not nobias:
                    nc.tensor.matmul(ps[:], ones1b[0:1, 0:128],
                                     brow_t[0:1, D:2 * D],
                                     start=False, stop=True)
                xres = x_t((W + 128 * q) // 128)
                nc.vector.tensor_add(x2_t[q][:], ps[:], xres)

            for q in range(NQB):
                ql = slice(128 * q, 128 * (q + 1))
                xs2 = tpool.tile([128, D], BF16, tag="ln_xs2", name="ln_xs2",
                                 bufs=2)
                layer_norm(x2_t[q], xs2[:])
                pt = psT.tile([128, 512], BF16, tag="tp", name="tp")
                for c in range(KC):
                    nc.tensor.transpose(pt[:, 128 * c:128 * (c + 1)],
                                        xs2[:, 128 * c:128 * (c + 1)], id_t[:])
                    if ntp % 2 == 0:
                        nc.vector.tensor_copy(xs2T[c][:, ql],
                                              pt[:, 128 * c:128 * (c + 1)])
                    else:
                        nc.scalar.copy(xs2T[c][:, ql],
                                       pt[:, 128 * c:128 * (c + 1)])
                    ntp += 1

        dump("d_x2", x2_t[0][:])

        # ---- phase E: FFN ----------------------------------------------
        with ExitStack() as ffn:
            fpool = ffn.enter_context(tc.tile_pool(name="ffn", bufs=1))
            psF = ffn.enter_context(
                tc.tile_pool(name="psF", bufs=4, space="PSUM"))
            psF2 = ffn.enter_context(
                tc.tile_pool(name="psF2", bufs=3, space="PSUM"))

            h1gT = [fpool.tile([128, TPC], BF16, tag=f"h1gT{f}",
                               name=f"h1gT{f}") for f in range(FFC)]

            def ffn2(q):
                ql = slice(128 * q, 128 * (q + 1))
                ps2 = psF2.tile([128, D], F32, tag="f2", name="f2")
                for f in range(FFC):
                    nc.tensor.matmul(ps2[:],
                                     h1gT[f][:, ql],
                                     w2_t[:, f * D:(f + 1) * D],
                                     start=(f == 0),
                                     stop=(nobias and f == FFC - 1))
                if not nobias:
                    nc.tensor.matmul(ps2[:], ones1b[0:1, 0:128],
                                     brow_t[0:1, 2 * D:3 * D],
                                     start=False, stop=True)
                o = fpool.tile([128, D], F32, tag="fout", name="fout", bufs=2)
                nc.vector.tensor_add(o[:], ps2[:], x2_t[q][:])
                nc.sync.dma_start(out[ql, :], o[:])

            for qh in range(2):
                hsl = slice(CH * qh, CH * (qh + 1))
                for f in range(FFC):
                    ps = psF.tile([128, CH], F32, tag="f1", name="f1")
                    for k in range(KC):
                        nc.tensor.matmul(
                            ps[:],
                            w1_t[:, k * FF + 128 * f:k * FF + 128 * (f + 1)],
                            xs2T[k][:, hsl],
                            start=(k == 0), stop=(k == KC - 1))
                    nc.scalar.activation(h1gT[f][:, hsl], ps[:],
                                         AF.Gelu, bias=bf1_c(f))
                ffn2(2 * qh)
                ffn2(2 * qh + 1)
            dump("d_h1", h1gT[0][:])

    nc.compile()
    return nc


def _get_nc(W, uniform, nobias):
    key = (W, uniform, nobias)
    if key not in _BUILD_CACHE:
        _BUILD_CACHE[key] = _build(W, uniform, nobias)
    return _BUILD_CACHE[key]


# ---------------------------------------------------------------------------
# host side
# ---------------------------------------------------------------------------

def _bf16(a):
    return np.asarray(a, np.float32).astype(ml_dtypes.bfloat16)


def kernel(x, mask, wq, wk, wv, wo, bo, g1, b1, g2, b2, w1, bf1, w2, bf2,
           decay_logit):
    x = np.asarray(x, np.float32)
    mask = np.asarray(mask)
    g1 = np.asarray(g1, np.float64)
    b1 = np.asarray(b1, np.float64)
    g2 = np.asarray(g2, np.float64)
    b2 = np.asarray(b2, np.float64)

    d64 = 1.0 / (1.0 + np.exp(-np.asarray(decay_logit, np.float64)))
    d64 = np.clip(d64, 1e-8, None)
    dmax = float(d64.max())
    # Band width: the attention kernel truncates at |i-j| > ~W.  The omitted
    # mass relative to the normalizer z is ~ d^(W+1)/(1+d); keep it under
    # 1e-5, well below the bf16 matmul noise (~1e-3).
    if dmax >= 1.0 - 1e-12:
        W = 512  # decay ~1: widest supported band
    else:
        need = math.log(1e-5 * (1.0 + dmax)) / math.log(dmax)
        W = max(128, 128 * math.ceil(need / 128))
        W = min(W, 512)
    nW = W // 128
    L = TPC + 2 * W
    NT = L // 128
    uniform = bool(np.all(d64 == d64[0]))
    nobias = bool(np.all(b1 == 0) and np.all(np.asarray(bo) == 0)
                  and np.all(np.asarray(bf2) == 0))
    NHM = 1 if uniform else H
    NQP = 1 if uniform else KC

    nc = _get_nc(W, uniform, nobias)

    # folded weights (bf16)
    wq_f = _bf16(g1[:, None] * np.asarray(wq, np.float64))
    wk_f = _bf16(g1[:, None] * np.asarray(wk, np.float64))
    wv_f = _bf16(g1[:, None] * np.asarray(wv, np.float64))
    wo_f = _bf16(wo)
    w1_f = _bf16(g2[:, None] * np.asarray(w1, np.float64))
    w2_f = _bf16(w2)
    bq_h = _bf16(b1 @ np.asarray(wq, np.float64))
    bk_h = _bf16(b1 @ np.asarray(wk, np.float64))
    bv_h = _bf16(b1 @ np.asarray(wv, np.float64))
    bo_h = _bf16(bo)
    bf1_h = (b2 @ np.asarray(w1, np.float64) +
             np.asarray(bf1, np.float64)).astype(np.float32)
    bf2_h = _bf16(bf2)

    def pack_kblocks(w, blk, nb):
        # [nb*128, cols] -> [128, nb*cols] with block k at cols [k*cols, ...)
        return np.ascontiguousarray(
            w.reshape(nb, 128, blk).transpose(1, 0, 2).reshape(128, nb * blk))

    # (dmask appended to wa below, bf16)
    wb_h = np.concatenate([
        pack_kblocks(wv_f, 512, 4),
        pack_kblocks(wo_f, 512, 4)], axis=1)
    w1_h = pack_kblocks(w1_f, FF, 4)
    w2_h = pack_kblocks(w2_f, D, 16)
    brow_h = np.concatenate(
        [bv_h, bo_h, bf2_h, bk_h, bq_h]).reshape(1, 5 * D)
    brow_h = brow_h.astype(ml_dtypes.bfloat16)

    # decay-derived tables
    ld = np.log(d64)                                    # (H,)
    di = np.arange(TPC, dtype=np.float64) % CH
    qposb = np.zeros((NQP, 128, TPC), np.float32)
    qnegb = np.zeros((NQP, 128, TPC), np.float32)
    for c in range(NQP):
        for hh in range(2):
            h = 2 * c + hh
            qposb[c, 64 * hh:64 * hh + 64, :] = np.exp(ld[h] * di)[None, :]
            qnegb[c, 64 * hh:64 * hh + 64, :] = np.exp(-ld[h] * di)[None, :]
    if uniform:
        qposb[:, 64:, :] = qposb[:, :64, :]
        qnegb[:, 64:, :] = qnegb[:, :64, :]
    # diagonal-zone mask: m[dk, di] = d^{|di-dk| - di}, dk in [0,256)
    dic = np.arange(CH, dtype=np.float64)
    dk = np.arange(CH, dtype=np.float64)
    dmask = np.zeros((NHM, 2, 128, CH), np.float32)
    for h in range(NHM):
        m = np.exp(ld[h] * (np.abs(dic[None, :] - dk[:, None]) - dic[None, :]))
        dmask[h, 0] = m[:128, :]
        dmask[h, 1] = m[128:, :]
    cb_h = np.concatenate([
        qposb.transpose(1, 0, 2).reshape(128, NQP * TPC),
        qnegb.transpose(1, 0, 2).reshape(128, NQP * TPC)], axis=1)
    cb_h = np.ascontiguousarray(cb_h).astype(ml_dtypes.bfloat16)
    NPR = 1 if uniform else KC
    dm2 = np.zeros((NPR, 2, 128, 2 * CH), np.float32)
    for p in range(NPR):
        for mi in range(2):
            dm2[p, mi, :, :CH] = dmask[(2 * p) % NHM, mi]
            dm2[p, mi, :, CH:] = dmask[(2 * p + 1) % NHM, mi]
    wa_h = np.concatenate([
        pack_kblocks(wk_f, 512, 4),
        pack_kblocks(wq_f, 512, 4),
        dm2.transpose(2, 0, 1, 3).reshape(128, NPR * 2 * 2 * CH)
           .astype(ml_dtypes.bfloat16)], axis=1)

    in_maps = []
    for core in range(NCORES):
        b = core // SH
        t0 = (core % SH) * TPC
        lo, hi = t0 - W, t0 + TPC + W
        xs = np.zeros((L, D), np.float32)
        pad = np.zeros((L,), np.float64)
        glo, ghi = max(lo, 0), min(hi, T)
        xs[glo - lo:ghi - lo] = x[b, glo:ghi]
        pad[glo - lo:ghi - lo] = (~mask[b, glo:ghi]).astype(np.float64)
        xbig = np.ascontiguousarray(
            xs.reshape(NT, 128, D).transpose(1, 0, 2).reshape(128, NT * D))

        vneg = np.zeros((H, L), np.float32)
        vpos = np.zeros((H, L), np.float32)
        vpln = np.zeros((H, L), np.float32)
        for h in range(H):
            vn = np.zeros(L)
            vp = np.zeros(L)
            for qi in range(NQC):
                s = W + qi * CH
                jj = np.arange(s - W, s)
                vn[jj] = np.exp(ld[h] * (s - jj))
                jj = np.arange(s + CH, s + CH + W)
                vp[jj] = np.exp(ld[h] * (CH + jj - (s + CH)))
            vneg[h] = (vn * pad).astype(np.float32)
            vpos[h] = (vp * pad).astype(np.float32)
            vpln[h] = pad.astype(np.float32)

        def _vlayout(a):
            return np.ascontiguousarray(
                a.reshape(H, L // 128, 128).transpose(2, 1, 0)
                 .reshape(128, (L // 128) * H))
        vneg, vpos, vpln = _vlayout(vneg), _vlayout(vpos), _vlayout(vpln)

        mu = xs.mean(axis=1)
        rr = 1.0 / np.sqrt(xs.var(axis=1) + 1e-5)
        mr = np.stack([mu, rr], axis=1).astype(np.float32)    # [L, 2]
        mr = mr.reshape(NT, 128, 2).transpose(1, 0, 2).reshape(128, 2 * NT)
        cs_h = np.concatenate([
            np.zeros((128, 8), np.float32),      # (bias cols retired)
            bf1_h.reshape(16, 128).T,
            vneg, vpos, vpln, mr], axis=1)
        cs_h = np.ascontiguousarray(cs_h, np.float32)

        nh = NT // 2
        im = {
            "xh0": np.ascontiguousarray(xbig[:, :nh * D]),
            "xh1": np.ascontiguousarray(xbig[:, nh * D:]),
            "ident": np.eye(128, dtype=ml_dtypes.bfloat16),
            "cs": cs_h, "cb": cb_h,
            "wa": wa_h, "wb": wb_h, "w1": w1_h, "w2": w2_h,
        }
        if not nobias:
            im["brow"] = brow_h
        in_maps.append(im)

    res = bass_utils.run_bass_kernel_spmd(nc, in_maps,
                                          core_ids=list(range(NCORES)))
    out = np.empty((B, T, D), np.float32)
    for core in range(NCORES):
        b = core // SH
        t0 = (core % SH) * TPC
        out[b, t0:t0 + TPC] = res.results[core]["out"]

    # Degenerate-mask patch: a query whose entire +-W neighbourhood is
    # masked has z ~ 0 on the device; the reference output is then
    # dominated by out-of-band keys that the banded kernel truncates.
    # Recompute those rows exactly on the host (never triggers for an
    # all-False mask).
    if mask.any():
        idx = np.arange(T)
        for b_ in range(B):
            keep = ~np.asarray(mask[b_])
            if keep.all():
                continue
            kpos = idx[keep]
            if len(kpos) == 0:
                dist = np.full(T, T)
            else:
                ins = np.searchsorted(kpos, idx)
                left = np.where(ins > 0, idx - kpos[np.clip(ins - 1, 0,
                                                            len(kpos) - 1)], T)
                right = np.where(ins < len(kpos),
                                 kpos[np.clip(ins, 0, len(kpos) - 1)] - idx, T)
                dist = np.minimum(left, right)
            need = dist > (W - 32)
            if need.any():
                out[b_, need] = _exact_rows(
                    np.asarray(x[b_], np.float64), keep.astype(np.float64),
                    np.where(need)[0], wq, wk, wv, wo, bo, g1, b1, g2, b2,
                    w1, bf1, w2, bf2, d64)
    return out


def _exact_rows(xb, pad, rows, wq, wk, wv, wo, bo, g1, b1, g2, b2,
                w1, bf1, w2, bf2, d64):
    """Reference math (float64) for the given query rows of one batch."""
    import scipy.special as _sp

    def ln(z, g, b):
        mu = z.mean(-1, keepdims=True)
        var = ((z - mu) ** 2).mean(-1, keepdims=True)
        return (z - mu) / np.sqrt(var + 1e-5) * g + b

    wq = np.asarray(wq, np.float64); wk = np.asarray(wk, np.float64)
    wv = np.asarray(wv, np.float64); wo = np.asarray(wo, np.float64)
    w1 = np.asarray(w1, np.float64); w2 = np.asarray(w2, np.float64)
    xn = ln(xb, np.asarray(g1, np.float64), np.asarray(b1, np.float64))

    def feat(z):
        return np.where(z > 0, z + 1.0, np.exp(np.minimum(z, 0.0)))

    Q = feat((xn[rows] @ wq).reshape(len(rows), H, HD))       # (R,H,hd)
    K = feat((xn @ wk).reshape(T, H, HD)) * pad[:, None, None]
    V = (xn @ wv).reshape(T, H, HD) * pad[:, None, None]
    dist = np.abs(rows[:, None] - np.arange(T)[None, :]).astype(np.float64)
    M = np.exp(dist[None] * np.log(d64)[:, None, None])       # (H,R,T)
    A = np.einsum('rhd,jhd->hrj', Q, K) * M
    z = np.clip(A.sum(-1, keepdims=True), 1e-6, None)
    o = np.einsum('hrj,jhd->rhd', A / z, V).reshape(len(rows), D)
    o = o @ wo + np.asarray(bo, np.float64)
    x2 = xb[rows] + o
    xn2 = ln(x2, np.asarray(g2, np.float64), np.asarray(b2, np.float64))
    h1 = xn2 @ w1 + np.asarray(bf1, np.float64)
    gl = 0.5 * h1 * (1.0 + _sp.erf(h1 / np.sqrt(2.0)))
    ffn = gl @ w2 + np.asarray(bf2, np.float64)
    return (x2 + ffn).astype(np.float32)
